# revision 1
# baseline (speedup 1.0000x reference)
"""2-layer GCN on 8 Trainium2 NeuronCores (Bass/Tile).

Math: gcn_conv(x, W, b) = D^-1/2 (A+I) D^-1/2 (x W) + b.  Propagation
commutes with the weight matmul, so layer 1 aggregates raw x
(h = relu((A_hat x) W1 + b1)) and layer 2 projects first
(out = A_hat (h W2) + b2), minimizing per-edge gather bytes.

Distribution: destination nodes sharded 8 ways (12500/core); each core
gathers source features for its own edges from a full local feature table
(x is an input; g = h W2 is assembled with one 8-rank AllGather).

Per-core aggregation, per layer:
- 5 block-major dma_gather streams (int16 idx limit => 4 source-range
  tables of <=25000 rows, plus a SELF stream for self-loops reading
  core-local tables: x_own / g_shard).
- Edges are laid out in fractional per-(tile, block) cells balanced by a
  per-core node permutation (greedy vector bin packing), ~1.6% padding;
  boundary chunks are consumed by two matmuls with foreign lanes zeroed.
- Consumption is tile-major: bf16 one-hot scatter matrices
  (tensor_scalar is_equal*norm) feed PE matmuls msgs^T @ onehot
  (bf16 -> fp32 PSUM); each destination tile accumulates all its cells
  in one PSUM bank, then a single ACT-engine escape feeds the inline
  projection (W1 -> relu -> W2 -> bf16 g row) or the bias+output write.

g is stored 128-wide (W2 zero-padded) so bf16 rows are 256B-aligned for
dma_gather, and g_full keeps shard-major order so one shared edge
schedule serves both layers (idx tensors differ only in value).
"""
import sys

sys.path.insert(0, "/opt/trn_rl_repo")
import numpy as np
import ml_dtypes

BF16 = ml_dtypes.bfloat16
NC = 8
CIN, CH, COUT = 128, 128, 64
CALL_CHUNKS = 40  # gather-call granularity (chunks of 128 edges)
K_OH = 8  # one-hot matrices generated per DVE op pair
PSUM_ACC_BUFS = 6  # concurrent per-tile accumulation banks
WBW = 386  # wb cols: W1[0:128] W2pad[128:256] iota[256:384] b1[384] b2[385]


def _balance(nv, caps, k_tb):
    """Best-fit-decreasing node->tile assignment for one core.

    nv: [NSH, NBLK] per-node block in-degree vectors.
    caps: [TILES] node slots per tile.  k_tb: [TILES, NBLK] chunk targets.
    Returns pos[NSH] (node -> global slot = tile*128 + slot_in_tile) or None
    if infeasible.
    """
    NSH, NBLK = nv.shape
    TILES = caps.shape[0]
    capv = k_tb.astype(np.float64)  # edge capacity per cell [TILES, NBLK]
    np.maximum(capv, 1e-9, out=capv)
    usedv = np.zeros((TILES, NBLK), dtype=np.float64)
    slots = caps.astype(np.float64)
    usect = np.zeros(TILES, dtype=np.int64)
    pos = np.empty(NSH, dtype=np.int64)
    order = np.argsort(-nv.sum(1), kind="stable")
    for n in order:
        v = nv[n].astype(np.float64)
        post = (usedv + v[None, :]) / capv  # post-placement fill ratios
        feas = (post <= 1.0).all(1) & (usect < caps)
        if not feas.any():
            return None
        # balance: place where the worst post-fill ratio (incl slots) is lowest
        score = np.maximum(post.max(1), (usect + 1) / caps)
        score[~feas] = np.inf
        t = int(np.argmin(score))
        usedv[t] += v
        pos[n] = t * 128 + usect[t]
        usect[t] += 1
    return pos


def _schedule(edge_index, n_nodes):
    """Static SPMD schedule + per-core edge arrays.

    Nodes are permuted within each core's shard (best-fit-decreasing bin
    packing) so per-(tile, block) edge counts fit a shared chunk budget with
    ~1% padding instead of ~35% from max-over-cores + ceil-to-128.
    """
    N = n_nodes
    NSH = (N + NC - 1) // NC  # dst nodes per core
    TILES = (NSH + 127) // 128
    NBLK = -(-N // 25000) if N > 32768 else 1
    BLK = -(-N // NBLK)  # src rows per gather table
    assert BLK <= 32767, (N, NBLK, BLK)

    src = np.asarray(edge_index[0], dtype=np.int64)
    dst = np.asarray(edge_index[1], dtype=np.int64)
    E = src.shape[0]
    deg = np.bincount(dst, minlength=N).astype(np.float64) + 1.0
    dinv = 1.0 / np.sqrt(deg)
    # self-loops are handled in a dedicated block (gathered from core-local
    # tables), so the streamed edge set here excludes them
    s_all = src
    d_all = dst
    w_all = (dinv[s_all] * dinv[d_all]).astype(np.float32)

    core = d_all // NSH
    j_all = d_all - core * NSH  # node index within dst core
    blk = s_all // BLK

    # per-core per-node block in-degree vectors (self-loops excluded)
    nv_flat = np.bincount(
        (core * NSH + j_all) * NBLK + blk, minlength=NC * NSH * NBLK
    )
    nv = nv_flat.reshape(NC, NSH, NBLK)
    B = nv.sum(axis=1)  # [NC, NBLK] edges per (core, block)

    caps = np.full(TILES, 128, dtype=np.int64)
    caps[TILES - 1] = NSH - 128 * (TILES - 1)

    # shared edge-capacity budget per block (multiple of 128), distributed
    # over tiles proportionally to node slots as integer cell capacities
    slack = 512
    for _attempt in range(8):
        TOTb = ((B.max(axis=0) + slack + 127) // 128) * 128  # [NBLK]
        captb = np.zeros((NBLK, TILES), dtype=np.int64)
        for b in range(NBLK):
            tgt = TOTb[b] * caps / NSH
            base = np.floor(tgt).astype(np.int64)
            rem = int(TOTb[b] - base.sum())
            order = np.argsort(-(tgt - base))
            base[order[:rem]] += 1
            captb[b] = base
        pos_all = np.empty((NC, NSH), dtype=np.int64)
        ok = True
        for r in range(NC):
            pos = _balance(nv[r], caps, captb.T)
            if pos is None:
                ok = False
                break
            pos_all[r] = pos
        if ok:
            break
        slack *= 2
    assert ok, "balance failed"

    # fractional cell layout: block 0 = SELF (one tile-aligned chunk per
    # tile, gathered from core-local tables); blocks 1..NBLK = src ranges.
    # Cells pack back-to-back within each block at arbitrary offsets;
    # boundary chunks are consumed by two matmuls (one per adjacent tile)
    # with norms zeroed for foreign lanes.
    captb = captb.astype(np.int64)  # [NBLK, TILES]
    cap_self = np.full((1, TILES), 128, dtype=np.int64)
    captb = np.concatenate([cap_self, captb], axis=0)  # [NBLK+1, TILES]
    NBLK1 = NBLK + 1
    Kb = captb.sum(axis=1) // 128  # chunks per block
    C = int(Kb.sum())
    nkey = NBLK1 * TILES
    off_flat = np.zeros(nkey + 1, dtype=np.int64)
    np.cumsum(captb.reshape(-1), out=off_flat[1:])
    CAP = int(off_flat[-1])
    assert CAP == C * 128

    # matmul table: (b, t, chunk, lo, hi) in TILE-MAJOR consumption order
    # (all of tile t's cells across blocks accumulate in one PSUM bank);
    # gather streams remain block-major (chunk numbering is global).
    mm = []
    for t in range(TILES):
        for b in range(NBLK1):
            o = int(off_flat[b * TILES + t])
            cap = int(captb[b, t])
            if cap == 0:
                continue
            c0, c1 = o >> 7, (o + cap - 1) >> 7
            for c in range(c0, c1 + 1):
                lo = max(o, c * 128)
                hi = min(o + cap, (c + 1) * 128)
                mm.append((b, t, c, lo, hi))
    mm = np.array(mm, dtype=np.int64)
    M = mm.shape[0]

    # permuted dst position of every edge
    pos_e = pos_all[core, j_all]
    tile_of = pos_e >> 7
    dstl = (pos_e & 127).astype(np.float32)
    key = (blk + 1) * TILES + tile_of  # blocks shifted by the self block

    idx1_all = np.zeros((NC, CAP), dtype=np.int16)
    idx2_all = np.zeros((NC, CAP), dtype=np.int16)
    dstl_all = np.zeros((NC, CAP), dtype=np.float32)
    norm_all = np.zeros((NC, CAP), dtype=np.float32)
    # permuted g_full row of every src node: r*NSH + pos (balance keeps
    # slot < caps[t], so pos < NSH and g_shard row == pos)
    sc = s_all // NSH  # src core
    sj = s_all - sc * NSH
    src_pos = pos_all[sc, sj]
    g_row = sc * NSH + src_pos

    capf = captb.reshape(-1)
    for r in range(NC):
        sel = np.nonzero(core == r)[0]
        k = key[sel]
        order = np.argsort(k, kind="stable")
        sel = sel[order]
        k = k[order]
        cr = np.bincount(k, minlength=nkey)
        grp_start = np.zeros(nkey, dtype=np.int64)
        np.cumsum(cr[:-1], out=grp_start[1:])
        rank_in_grp = np.arange(sel.shape[0], dtype=np.int64) - np.repeat(grp_start, cr)
        assert (rank_in_grp < capf[k]).all()
        slots = off_flat[k] + rank_in_grp
        idx1_all[r, slots] = (s_all[sel] - blk[sel] * BLK).astype(np.int16)
        idx2_all[r, slots] = (g_row[sel] - blk[sel] * BLK).astype(np.int16)
        dstl_all[r, slots] = dstl[sel]
        norm_all[r, slots] = w_all[sel]
        # self block: node at permuted position p sits at slot p (cells are
        # tile-aligned in tile order); idx1 = original j (into x_own),
        # idx2 = p (into g_shard), norm = dinv^2
        nval = min(NSH, N - r * NSH)
        jj = np.arange(nval, dtype=np.int64)
        p = pos_all[r, :nval]
        idx1_all[r, p] = jj.astype(np.int16)
        idx2_all[r, p] = p.astype(np.int16)
        dstl_all[r, p] = (p & 127).astype(np.float32)
        norm_all[r, p] = (dinv[r * NSH + jj] ** 2).astype(np.float32)

    def wrap_idx(a):
        return np.ascontiguousarray(
            np.tile(a.reshape(NC, CAP // 16, 16).transpose(0, 2, 1), (1, 8, 1))
        )

    idx1_sb = wrap_idx(idx1_all)
    idx2_sb = wrap_idx(idx2_all)
    # per-matmul meta [128, 2M] bf16: lanes outside [lo, hi) zeroed; dstl
    # gets -1 on dead lanes so is_equal never matches (norm is 0 anyway)
    md = np.full((NC, M, 128), -1.0, dtype=np.float32)
    mn = np.zeros((NC, M, 128), dtype=np.float32)
    for i in range(M):
        b, t, c, lo, hi = mm[i]
        base = int(c) * 128
        md[:, i, lo - base : hi - base] = dstl_all[:, lo:hi]
        mn[:, i, lo - base : hi - base] = norm_all[:, lo:hi]
    meta = np.empty((NC, 128, 2 * M), dtype=np.float32)
    meta[:, :, 0::2] = md.transpose(0, 2, 1)
    meta[:, :, 1::2] = mn.transpose(0, 2, 1)

    return dict(
        N=N, E=E, NSH=NSH, TILES=TILES, NBLK=NBLK, BLK=BLK, C=C, M=M,
        Kb=Kb, mm=mm, idx_sb=idx1_sb, idx2_sb=idx2_sb, meta=meta,
        pos_all=pos_all,
    )


def _build_bass(sp, for_timing=False):
    import concourse.bass as bass
    import concourse.bacc as bacc
    import concourse.mybir as mybir
    import concourse.tile as tile

    f32 = mybir.dt.float32
    bf16 = mybir.dt.bfloat16
    i16 = mybir.dt.int16
    N, NSH, TILES, NBLK, BLK, C, M = (
        sp["N"], sp["NSH"], sp["TILES"], sp["NBLK"], sp["BLK"], sp["C"], sp["M"]
    )
    Kb = sp["Kb"]
    mm = sp["mm"]  # [(b, t, chunk, lo, hi)] in consumption order
    NPAD = TILES * 128

    # per-block gather call lists: calls partition each block's (global)
    # chunk run into <= CALL_CHUNKS pieces
    NBLK1 = len(Kb)
    calls = []  # calls[b] = [(chunk_lo, nchunks), ...]
    chunk0 = 0
    for b in range(NBLK1):
        kb = int(Kb[b])
        lst = []
        s0, left = chunk0, kb
        while left > 0:
            cs = min(CALL_CHUNKS, left)
            lst.append((s0, cs))
            s0 += cs
            left -= cs
        calls.append(lst)
        chunk0 += kb
    MW = 128  # meta window (matmuls per meta tile)

    nc = bacc.Bacc("TRN2", target_bir_lowering=False, debug=False, num_devices=NC)
    x_in = nc.dram_tensor("x", [N, CIN], bf16, kind="ExternalInput")
    x_own_in = nc.dram_tensor("x_own", [NSH, CIN], bf16, kind="ExternalInput")
    idx_in = nc.dram_tensor("idx", [128, C * 8], i16, kind="ExternalInput")
    idx2_in = nc.dram_tensor("idx2", [128, C * 8], i16, kind="ExternalInput")
    meta_in = nc.dram_tensor("meta", [128, 2 * M], f32, kind="ExternalInput")
    wb_in = nc.dram_tensor("wb", [128, WBW], f32, kind="ExternalInput")
    outT = nc.dram_tensor("outT", [COUT, NPAD], f32, kind="ExternalOutput")

    with tile.TileContext(nc) as tc:
        with (
            tc.tile_pool(name="const", bufs=1) as constp,
            tc.tile_pool(name="stream", bufs=3) as streamp,
            tc.tile_pool(name="msgs", bufs=3) as msgsp,
            tc.tile_pool(name="work", bufs=12) as workp,
            tc.tile_pool(name="pacc", bufs=PSUM_ACC_BUFS, space="PSUM") as paccp,
            tc.tile_pool(name="pproj", bufs=2, space="PSUM") as pprojp,
            tc.tile_pool(name="dram", bufs=1, space="DRAM") as dramp,
        ):
            wb = constp.tile([128, WBW], f32)
            nc.sync.dma_start(wb[:], wb_in[:])
            W1 = wb[:, 0:128]
            W2p = wb[:, 128:256]
            b1 = wb[:, 384:385]
            b2 = wb[:64, 385:386]
            iota_bf = constp.tile([128, 128], bf16)
            nc.vector.tensor_copy(iota_bf[:], wb[:, 256:384])

            g_shard = dramp.tile([NSH, 128], bf16)
            g_full = dramp.tile(
                [NC * NSH, 128], bf16,
                addr_space="Local" if for_timing else "Shared",
            )

            def aggregate(self_table, table, elem, layer, idx_dram, epilogue):
                """Tile-major consumption: each tile's cells (all blocks)
                accumulate in one PSUM bank; 5 block-major gather streams
                feed the matmuls; `epilogue(t, pt)` consumes the full tile."""
                nrow = 128 if layer == 1 else COUT
                # per-stream state
                s_ci = [0] * NBLK1  # next call index
                s_cur = [(-1, 0)] * NBLK1  # (chunk_lo, nchunks) of current
                s_mg = [None] * NBLK1
                mt = None
                cur_w = -1
                pt = None
                cur_t = -1
                for i in range(M):
                    b, t, c, lo, hi = (int(v) for v in mm[i])
                    cur0, curk = s_cur[b]
                    if s_mg[b] is None or c >= cur0 + curk:
                        cur0, curk = calls[b][s_ci[b]]
                        s_ci[b] += 1
                        s_cur[b] = (cur0, curk)
                        assert cur0 <= c < cur0 + curk
                        idxt = streamp.tile(
                            [128, CALL_CHUNKS * 8], i16, tag=f"idx{b}"
                        )
                        nc.sync.dma_start(
                            idxt[:, : curk * 8],
                            idx_dram[:, cur0 * 8 : (cur0 + curk) * 8],
                        )
                        mg = msgsp.tile(
                            [128, CALL_CHUNKS, elem], bf16, tag=f"msgs{b}"
                        )
                        s_mg[b] = mg
                        if b == 0:
                            src_ap = self_table[:, :]
                        else:
                            base = (b - 1) * BLK
                            src_ap = table[base : base + min(BLK, N - base), :]
                        nc.gpsimd.dma_gather(
                            mg[:, :curk, :],
                            src_ap,
                            idxt[:, : curk * 8],
                            num_idxs=curk * 128,
                            num_idxs_reg=curk * 128,
                            elem_size=elem,
                            single_packet=False,
                        )
                    if i // MW != cur_w:
                        cur_w = i // MW
                        nmw = min(MW, M - cur_w * MW)
                        mt = streamp.tile([128, 2 * MW], f32, tag="meta")
                        nc.sync.dma_start(
                            mt[:, : 2 * nmw],
                            meta_in[:, 2 * cur_w * MW : 2 * (cur_w * MW + nmw)],
                        )
                    if t != cur_t:
                        if pt is not None:
                            epilogue(cur_t, pt)
                        pt = paccp.tile([128, 128], f32, tag="pacc")
                        cur_t = t
                        first = True
                    else:
                        first = False
                    last = (i == M - 1) or (int(mm[i + 1][1]) != t)
                    cl = c - cur0
                    mloc = i - cur_w * MW
                    oh = workp.tile([128, 128], bf16, tag="oh")
                    nc.vector.tensor_scalar(
                        oh[:],
                        iota_bf[:],
                        mt[:, 2 * mloc : 2 * mloc + 1],
                        mt[:, 2 * mloc + 1 : 2 * mloc + 2],
                        mybir.AluOpType.is_equal,
                        mybir.AluOpType.mult,
                    )
                    nc.tensor.matmul(
                        pt[:nrow, :],
                        s_mg[b][:, cl, :nrow],
                        oh[:],
                        start=first,
                        stop=last,
                    )
                epilogue(cur_t, pt)

            # ---------------- layer 1 ----------------
            # per tile: aggT -> hT = relu(W1^T aggT + b1) -> g = hT^T W2pad
            def epilogue1(t, pt):
                r0 = t * 128
                rows = min(128, NSH - r0)
                aggT = workp.tile([128, 128], f32, tag="aggT")
                nc.scalar.activation(
                    aggT[:], pt[:], mybir.ActivationFunctionType.Copy
                )
                hp = pprojp.tile([128, 128], f32, tag="proj")
                nc.tensor.matmul(hp[:], W1, aggT[:], start=True, stop=True)
                hs = workp.tile([128, 128], f32, tag="hs")
                nc.scalar.activation(
                    hs[:], hp[:], mybir.ActivationFunctionType.Relu,
                    bias=b1, scale=1.0,
                )
                gp = pprojp.tile([128, 128], f32, tag="proj")
                nc.tensor.matmul(gp[:], hs[:], W2p, start=True, stop=True)
                gs = workp.tile([128, 128], bf16, tag="gs")
                nc.scalar.activation(
                    gs[:], gp[:], mybir.ActivationFunctionType.Copy
                )
                nc.sync.dma_start(g_shard[r0 : r0 + rows, :], gs[:rows, :])

            aggregate(x_own_in, x_in, CIN, 1, idx_in, epilogue1)

            # ---------------- exchange ----------------
            if for_timing:
                nc.gpsimd.dma_start(g_full[:NSH, :], g_shard[:])
            else:
                nc.gpsimd.collective_compute(
                    "AllGather",
                    mybir.AluOpType.bypass,
                    replica_groups=[list(range(NC))],
                    ins=[g_shard[:]],
                    outs=[g_full[:]],
                )

            # ---------------- layer 2 ----------------
            def epilogue2(t, pt):
                r0 = t * 128
                cols = min(128, NSH - r0)
                ob = workp.tile([64, 128], f32, tag="ob")
                nc.scalar.activation(
                    ob[:],
                    pt[:COUT, :],
                    mybir.ActivationFunctionType.Identity,
                    bias=b2,
                    scale=1.0,
                )
                nc.sync.dma_start(outT[:, r0 : r0 + cols], ob[:, :cols])

            aggregate(g_shard, g_full, 128, 2, idx2_in, epilogue2)

    nc.compile()
    return nc


_CACHE = {}


def _get_program(sp):
    key = (sp["N"], sp["C"], sp["mm"].tobytes())
    if key not in _CACHE:
        _CACHE[key] = _build_bass(sp)
    return _CACHE[key]


def _make_wb(W1, b1, W2, b2):
    wb = np.zeros((128, WBW), dtype=np.float32)
    wb[:, 0:128] = np.asarray(W1, dtype=np.float32)
    wb[:, 128 : 128 + COUT] = np.asarray(W2, dtype=np.float32)
    wb[:, 256:384] = np.arange(128, dtype=np.float32)[None, :]
    wb[:, 384] = np.asarray(b1, dtype=np.float32)
    wb[:64, 385] = np.asarray(b2, dtype=np.float32)
    return wb


def make_in_maps(sp, x, W1, b1, W2, b2):
    xb = np.ascontiguousarray(np.asarray(x, dtype=np.float32).astype(BF16))
    wb = _make_wb(W1, b1, W2, b2)
    NSH = sp["NSH"]
    xown = np.zeros((NC, NSH, CIN), dtype=BF16)
    for r in range(NC):
        hi = min(sp["N"], (r + 1) * NSH)
        xown[r, : hi - r * NSH] = xb[r * NSH : hi]
    return [
        {
            "x": xb,
            "x_own": xown[r],
            "idx": sp["idx_sb"][r],
            "idx2": sp["idx2_sb"][r],
            "meta": sp["meta"][r],
            "wb": wb,
        }
        for r in range(NC)
    ]


def kernel(x, edge_index, W1, b1, W2, b2, _trace=False):
    from concourse.bass_utils import run_bass_kernel_spmd

    x = np.asarray(x, dtype=np.float32)
    N = x.shape[0]
    sp = _schedule(np.asarray(edge_index), N)
    nc = _get_program(sp)
    in_maps = make_in_maps(sp, x, W1, b1, W2, b2)
    res = run_bass_kernel_spmd(nc, in_maps, list(range(NC)), trace=_trace)

    NSH = sp["NSH"]
    out = np.empty((N, COUT), dtype=np.float32)
    for r in range(NC):
        lo = r * NSH
        hi = min(N, lo + NSH)
        out[lo:hi] = res.results[r]["outT"][:, sp["pos_all"][r, : hi - lo]].T
    if _trace:
        kernel.last_result = res
    return out



# revision 6
# speedup vs baseline: 1.1625x; 1.1625x over previous
"""2-layer GCN on 8 Trainium2 NeuronCores (Bass/Tile).

Math: gcn_conv(x, W, b) = D^-1/2 (A+I) D^-1/2 (x W) + b.  Propagation
commutes with the weight matmul, so layer 1 aggregates raw x
(h = relu((A_hat x) W1 + b1)) and layer 2 projects first
(out = A_hat (h W2) + b2), minimizing per-edge gather bytes.

Distribution: destination nodes sharded 8 ways (12500/core); each core
gathers source features for its own edges from a full local feature table
(x is an input; g = h W2 is assembled with one 8-rank AllGather).

g is stored at its true width (64 cols bf16 = 128 B rows) so the
AllGather moves half the bytes of a 128-padded layout.  dma_gather
requires 256 B-aligned elements, so layer-2 gathers fetch a 256 B *unit*
(two consecutive g rows) per edge: streams come in (unit-range, parity)
flavors whose table APs are offset by 64 elements so the wanted row
always lands in msgs cols 0:64.

Per-core aggregation, per layer:
- block-major dma_gather streams (int16 idx limit => <=25000-row/unit
  tables). L1: 4 source-range blocks + a SELF stream reading x_own.
  L2: 4 (unit-range x parity) blocks reading g_full + 2 parity SELF
  streams reading g_shard.
- Edges are laid out in fractional per-(tile, block) cells balanced by a
  per-core node permutation (greedy vector bin packing), ~2% padding;
  boundary chunks are consumed by two matmuls with foreign lanes zeroed.
- Consumption is tile-major: bf16 one-hot scatter matrices
  (tensor_scalar is_equal*norm) feed PE matmuls msgs^T @ onehot
  (bf16 -> fp32 PSUM); each destination tile accumulates all its cells
  in one PSUM bank, then a single ACT-engine escape feeds the inline
  projection (W1 -> relu -> W2 -> bf16 g row) or the bias+output write.
"""
import sys

sys.path.insert(0, "/opt/trn_rl_repo")
import numpy as np
import ml_dtypes

BF16 = ml_dtypes.bfloat16
NC = 8
CIN, CH, COUT = 128, 128, 64
CALL_CHUNKS = 36  # gather-call granularity (chunks of 128 edges)
PSUM_ACC_BUFS = 6  # concurrent per-tile accumulation banks
WBW = 386  # wb cols: W1[0:128] W2[128:192] iota[256:384] b1[384] b2[385]


def _balance(nv, caps, k_tb):
    """Best-fit-decreasing node->tile assignment for one core.

    nv: [NSH, NB] per-node block in-degree vectors (both layers' blocks).
    caps: [TILES] node slots per tile.  k_tb: [TILES, NB] cell targets.
    Returns pos[NSH] (node -> global slot = tile*128 + slot_in_tile) or None
    if infeasible.
    """
    NSH, NB = nv.shape
    TILES = caps.shape[0]
    capv = k_tb.astype(np.float64)
    np.maximum(capv, 1e-9, out=capv)
    usedv = np.zeros((TILES, NB), dtype=np.float64)
    usect = np.zeros(TILES, dtype=np.int64)
    pos = np.empty(NSH, dtype=np.int64)
    order = np.argsort(-nv.sum(1), kind="stable")
    for n in order:
        v = nv[n].astype(np.float64)
        post = (usedv + v[None, :]) / capv
        feas = (post <= 1.0).all(1) & (usect < caps)
        if not feas.any():
            return None
        score = np.maximum(post.max(1), (usect + 1) / caps)
        score[~feas] = np.inf
        t = int(np.argmin(score))
        usedv[t] += v
        pos[n] = t * 128 + usect[t]
        usect[t] += 1
    return pos


def _mk_mm(captb, TILES):
    """Tile-major matmul table for a cell-capacity matrix [NB, TILES].

    Returns (mm [(b,t,c,lo,hi)], off_flat, Kb, C).  Chunk numbering is
    global across blocks (block b's chunks follow block b-1's).
    """
    NB = captb.shape[0]
    nkey = NB * TILES
    off_flat = np.zeros(nkey + 1, dtype=np.int64)
    np.cumsum(captb.reshape(-1), out=off_flat[1:])
    CAP = int(off_flat[-1])
    assert CAP % 128 == 0
    # block starts must be chunk-aligned so gather streams stay block-major
    blk_cap = captb.sum(axis=1)
    assert (blk_cap % 128 == 0).all(), blk_cap
    Kb = blk_cap // 128
    C = int(Kb.sum())
    mm = []
    for t in range(TILES):
        for b in range(NB):
            o = int(off_flat[b * TILES + t])
            cap = int(captb[b, t])
            if cap == 0:
                continue
            c0, c1 = o >> 7, (o + cap - 1) >> 7
            for c in range(c0, c1 + 1):
                lo = max(o, c * 128)
                hi = min(o + cap, (c + 1) * 128)
                mm.append((b, t, c, lo, hi))
    return np.array(mm, dtype=np.int64), off_flat, Kb, C


def _fill_slots(sel_by_core, key_all, val_all, dstl_all, norm_all, off_flat,
                capf, CAP):
    """Scatter per-edge records into their cell slots for all cores.

    Returns (idx [NC, CAP] int16, dstl [NC, CAP] f32, norm [NC, CAP] f32).
    """
    idx = np.zeros((NC, CAP), dtype=np.int16)
    dst = np.full((NC, CAP), -1.0, dtype=np.float32)
    nrm = np.zeros((NC, CAP), dtype=np.float32)
    for r in range(NC):
        sel = sel_by_core[r]
        k = key_all[sel]
        order = np.argsort(k, kind="stable")
        sel = sel[order]
        k = k[order]
        nkey = capf.shape[0]
        cr = np.bincount(k, minlength=nkey)
        grp_start = np.zeros(nkey, dtype=np.int64)
        np.cumsum(cr[:-1], out=grp_start[1:])
        rank = np.arange(sel.shape[0], dtype=np.int64) - np.repeat(grp_start, cr)
        assert (rank < capf[k]).all(), "cell overflow"
        slots = off_flat[k] + rank
        idx[r, slots] = val_all[sel].astype(np.int16)
        dst[r, slots] = dstl_all[sel]
        nrm[r, slots] = norm_all[sel]
    return idx, dst, nrm


def _wrap_idx(a):
    NCc, CAP = a.shape
    return np.ascontiguousarray(
        np.tile(a.reshape(NCc, CAP // 16, 16).transpose(0, 2, 1), (1, 8, 1))
    )


def _mk_meta(mm, dstl_all, norm_all):
    M = mm.shape[0]
    md = np.full((NC, M, 128), -1.0, dtype=np.float32)
    mn = np.zeros((NC, M, 128), dtype=np.float32)
    for i in range(M):
        b, t, c, lo, hi = mm[i]
        base = int(c) * 128
        md[:, i, lo - base : hi - base] = dstl_all[:, lo:hi]
        mn[:, i, lo - base : hi - base] = norm_all[:, lo:hi]
    meta = np.empty((NC, 128, 2 * M), dtype=np.float32)
    meta[:, :, 0::2] = md.transpose(0, 2, 1)
    meta[:, :, 1::2] = mn.transpose(0, 2, 1)
    return meta


def _schedule(edge_index, n_nodes):
    """Static SPMD schedule + per-core edge arrays for both layers."""
    N = n_nodes
    NSH = (N + NC - 1) // NC
    TILES = (NSH + 127) // 128
    BLK = 25000  # L1 x-table rows per block
    NB1 = -(-N // BLK)  # L1 source-range blocks
    UBLK = 25000  # L2 units per block (unit = 2 nodes)
    NU = (N + 1) // 2
    NB2u = -(-NU // UBLK)  # L2 unit-range blocks
    assert NSH % 2 == 0

    src = np.asarray(edge_index[0], dtype=np.int64)
    dst = np.asarray(edge_index[1], dtype=np.int64)
    E = src.shape[0]
    deg = np.bincount(dst, minlength=N).astype(np.float64) + 1.0
    dinv = 1.0 / np.sqrt(deg)
    w_all = (dinv[src] * dinv[dst]).astype(np.float32)

    core = dst // NSH
    j_all = dst - core * NSH
    blk1 = src // BLK

    # per-core per-node block in-degree vectors for balance:
    # L1 blocks (src range, self excluded) ++ L2 blocks (unit range x parity,
    # self excluded -- self parity depends on pos, handled by SELF streams)
    sc = src // NSH  # src core
    ub = sc // 4  # L2 unit-range block (aligned with shard pairs)
    nv1 = np.bincount(
        (core * NSH + j_all) * NB1 + blk1, minlength=NC * NSH * NB1
    ).reshape(NC, NSH, NB1)
    B1 = nv1.sum(axis=1)

    caps = np.full(TILES, 128, dtype=np.int64)
    caps[TILES - 1] = NSH - 128 * (TILES - 1)

    # The balance sees L1 source-range blocks and L2 unit-range blocks
    # (parity-summed); L2 parity cells are then sized exactly after an
    # explicit parity assignment pass below.
    nv2u = np.bincount(
        (core * NSH + j_all) * NB2u + ub, minlength=NC * NSH * NB2u
    ).reshape(NC, NSH, NB2u)
    B2u = nv2u.sum(axis=1)  # [NC, NB2u]

    def spread(tot):
        tgt = tot * caps / NSH
        base = np.floor(tgt).astype(np.int64)
        rem = int(tot - base.sum())
        order = np.argsort(-(tgt - base))
        base[order[:rem]] += 1
        return base

    slack = 256
    for _attempt in range(8):
        TOT1 = ((B1.max(axis=0) + slack + 127) // 128) * 128  # [NB1]
        TOT2u = B2u.max(axis=0) + 2 * slack  # [NB2u] (both parities)
        captb1 = np.stack([spread(t) for t in TOT1])  # [NB1, TILES]
        captb2u = np.stack([spread(t) for t in TOT2u])  # [NB2u, TILES]
        k_tb = np.concatenate([captb1.T, captb2u.T], axis=1)
        nv = np.concatenate([nv1, nv2u], axis=2)
        pos_all = np.empty((NC, NSH), dtype=np.int64)
        ok = True
        for r in range(NC):
            pos = _balance(nv[r], caps, k_tb)
            if pos is None:
                ok = False
                break
            pos_all[r] = pos
        if ok:
            break
        slack *= 2
    assert ok, "balance failed"

    # ---- explicit parity assignment (within-tile slot reshuffle) ----
    # A src node's position parity decides which L2 parity stream its
    # out-edges ride at every consumer.  Within each producer tile the
    # even/odd slot counts are fixed; which node takes which parity is
    # free.  Greedy discrepancy minimization over consumer (core, tile)
    # cells keeps every parity cell near half its unit-range cell.
    tile_of_e = pos_all[core, j_all] >> 7  # dst tile per edge (stable)
    cell_of_e = core * TILES + tile_of_e
    order_e = np.argsort(src, kind="stable")
    e_sorted = cell_of_e[order_e]
    src_sorted = src[order_e]
    starts = np.searchsorted(src_sorted, np.arange(N + 1))
    parity = np.zeros(N, dtype=np.int64)
    for g in range(NB2u):
        D = np.zeros(NC * TILES, dtype=np.int64)
        # nodes of producer cores 4g..4g+3, grouped by (core, tile)
        for r in range(4 * g, min(4 * g + 4, NC)):
            nval = min(NSH, N - r * NSH)
            p = pos_all[r, :nval]
            t = p >> 7
            for tt in range(TILES):
                nodes = np.nonzero(t == tt)[0] + r * NSH
                if nodes.size == 0:
                    continue
                degs = starts[nodes + 1] - starts[nodes]
                nodes = nodes[np.argsort(-degs, kind="stable")]
                n_even = (nodes.size + 1) // 2
                ev_left, od_left = n_even, nodes.size - n_even
                for n in nodes:
                    cells = e_sorted[starts[n] : starts[n + 1]]
                    if ev_left == 0:
                        p_n = 1
                    elif od_left == 0:
                        p_n = 0
                    else:
                        p_n = 0 if D[cells].sum() <= 0 else 1
                    parity[n] = p_n
                    if p_n == 0:
                        ev_left -= 1
                        np.add.at(D, cells, 1)
                    else:
                        od_left -= 1
                        np.add.at(D, cells, -1)
    # reassign within-tile slots by parity class
    for r in range(NC):
        nval = min(NSH, N - r * NSH)
        p = pos_all[r, :nval]
        t = p >> 7
        par_n = parity[r * NSH : r * NSH + nval]
        newpos = np.empty(nval, dtype=np.int64)
        for tt in range(TILES):
            nodes = np.nonzero(t == tt)[0]
            if nodes.size == 0:
                continue
            ev = nodes[par_n[nodes] == 0]
            od = nodes[par_n[nodes] == 1]
            newpos[ev] = tt * 128 + 2 * np.arange(ev.size)
            newpos[od] = tt * 128 + 2 * np.arange(od.size) + 1
        pos_all[r, :nval] = newpos

    pos_e = pos_all[core, j_all]
    tile_of = pos_e >> 7
    dstl = (pos_e & 127).astype(np.float32)

    # permuted g row of every src node
    src_pos = pos_all[sc, src - sc * NSH]
    g_row = sc * NSH + src_pos
    par = (g_row & 1).astype(np.int64)

    # ---------------- layer 1 cells ----------------
    # block 0 = SELF (tile-aligned, cap 128/tile, reads x_own), 1..NB1 = src
    captb1f = np.concatenate(
        [np.full((1, TILES), 128, dtype=np.int64), captb1], axis=0
    )
    mm1, off1, Kb1, C1 = _mk_mm(captb1f, TILES)
    CAP1 = C1 * 128
    key1 = (blk1 + 1) * TILES + tile_of
    val1 = src - blk1 * BLK
    sel_by_core = [np.nonzero(core == r)[0] for r in range(NC)]
    idx1, dstl1, norm1 = _fill_slots(
        sel_by_core, key1, val1, dstl, w_all, off1, captb1f.reshape(-1), CAP1
    )
    # L1 self: node at permuted position p sits at slot p; idx = original j
    for r in range(NC):
        nval = min(NSH, N - r * NSH)
        jj = np.arange(nval, dtype=np.int64)
        p = pos_all[r, :nval]
        idx1[r, p] = jj.astype(np.int16)
        dstl1[r, p] = (p & 127).astype(np.float32)
        norm1[r, p] = (dinv[r * NSH + jj] ** 2).astype(np.float32)
    meta1 = _mk_meta(mm1, dstl1, norm1)

    # ---------------- layer 2 cells ----------------
    # blocks 0..2*NB2u-1 = (unit range x parity) reading g_full;
    # blocks -2,-1 = SELF even/odd (tile-aligned, cap 64/tile, read g_shard)
    # Parity cells sized exactly: max over cores + chunk-align block sums.
    NBm = 2 * NB2u
    cnt2 = np.bincount(
        (core * TILES + tile_of) * NBm + (ub * 2 + par),
        minlength=NC * TILES * NBm,
    ).reshape(NC, TILES, NBm)
    captb2m = cnt2.max(axis=0).T.copy()  # [NBm, TILES]
    for b in range(NBm):
        rem = int(captb2m[b].sum()) % 128
        if rem:
            captb2m[b, TILES - 1] += 128 - rem
    half = np.zeros((2, TILES), dtype=np.int64)
    for t in range(TILES):
        cap_t = int(caps[t])
        half[0, t] = (cap_t + 1) // 2  # even positions in tile
        half[1, t] = cap_t // 2
    # pad self blocks to chunk multiples (dead lanes at the very end)
    selfpad = np.zeros((2, TILES), dtype=np.int64)
    for q in range(2):
        rem = int(half[q].sum()) % 128
        if rem:
            selfpad[q, TILES - 1] = 128 - rem
    captb2f = np.concatenate([captb2m, half + selfpad], axis=0)
    mm2, off2, Kb2, C2 = _mk_mm(captb2f, TILES)
    CAP2 = C2 * 128
    key2 = (ub * 2 + par) * TILES + tile_of
    val2 = (g_row >> 1) - ub * UBLK
    idx2, dstl2, norm2 = _fill_slots(
        sel_by_core, key2, val2, dstl, w_all, off2, captb2f.reshape(-1), CAP2
    )
    # L2 self: node at pos p -> self block (parity of p), slot = rank of p
    # among same-parity positions of its tile (tile-major order)
    for r in range(NC):
        nval = min(NSH, N - r * NSH)
        jj = np.arange(nval, dtype=np.int64)
        p = pos_all[r, :nval]
        t = p >> 7
        q = p & 1
        # slot within (self block q): offset of tile + rank among parity
        # positions of the tile, ordered by p
        order = np.argsort(p, kind="stable")
        slots = np.empty(nval, dtype=np.int64)
        for qq in range(2):
            selq = order[q[order] == qq]
            tq = t[selq]
            # rank within tile
            start = np.zeros(TILES, dtype=np.int64)
            cnts = np.bincount(tq, minlength=TILES)
            np.cumsum(cnts[:-1], out=start[1:])
            rank = np.arange(selq.shape[0], dtype=np.int64) - np.repeat(
                start, cnts
            )
            cellbase = off2[(NBm + qq) * TILES + tq]
            slots[selq] = cellbase + rank
        idx2[r, slots] = (p >> 1).astype(np.int16)
        dstl2[r, slots] = (p & 127).astype(np.float32)
        norm2[r, slots] = (dinv[r * NSH + jj] ** 2).astype(np.float32)
    meta2 = _mk_meta(mm2, dstl2, norm2)

    return dict(
        N=N, E=E, NSH=NSH, TILES=TILES, NB1=NB1, BLK=BLK, NB2u=NB2u,
        UBLK=UBLK,
        C1=C1, M1=mm1.shape[0], mm1=mm1, Kb1=Kb1,
        C2=C2, M2=mm2.shape[0], mm2=mm2, Kb2=Kb2,
        idx1_sb=_wrap_idx(idx1), idx2_sb=_wrap_idx(idx2),
        meta1=meta1, meta2=meta2,
        pos_all=pos_all,
    )


def _build_bass(sp, for_timing=False):
    import concourse.bass as bass
    import concourse.bacc as bacc
    import concourse.mybir as mybir
    import concourse.tile as tile

    f32 = mybir.dt.float32
    bf16 = mybir.dt.bfloat16
    i16 = mybir.dt.int16
    N, NSH, TILES = sp["N"], sp["NSH"], sp["TILES"]
    BLK, NB1 = sp["BLK"], sp["NB1"]
    UBLK, NB2u = sp["UBLK"], sp["NB2u"]
    NU = (N + 1) // 2
    NPAD = TILES * 128

    def mk_calls(Kb):
        calls = []
        chunk0 = 0
        for kb in (int(k) for k in Kb):
            lst = []
            s0, left = chunk0, kb
            while left > 0:
                cs = min(CALL_CHUNKS, left)
                lst.append((s0, cs))
                s0 += cs
                left -= cs
            calls.append(lst)
            chunk0 += kb
        return calls

    calls1 = mk_calls(sp["Kb1"])
    calls2 = mk_calls(sp["Kb2"])
    MW = 128  # meta window (matmuls per meta tile)

    nc = bacc.Bacc("TRN2", target_bir_lowering=False, debug=False, num_devices=NC)
    x_in = nc.dram_tensor("x", [N, CIN], bf16, kind="ExternalInput")
    x_own_in = nc.dram_tensor("x_own", [NSH, CIN], bf16, kind="ExternalInput")
    idx1_in = nc.dram_tensor("idx1", [128, sp["C1"] * 8], i16, kind="ExternalInput")
    idx2_in = nc.dram_tensor("idx2", [128, sp["C2"] * 8], i16, kind="ExternalInput")
    meta1_in = nc.dram_tensor("meta1", [128, 2 * sp["M1"]], f32, kind="ExternalInput")
    meta2_in = nc.dram_tensor("meta2", [128, 2 * sp["M2"]], f32, kind="ExternalInput")
    wb_in = nc.dram_tensor("wb", [128, WBW], f32, kind="ExternalInput")
    outT = nc.dram_tensor("outT", [COUT, NPAD], f32, kind="ExternalOutput")

    with tile.TileContext(nc) as tc:
        with (
            tc.tile_pool(name="const", bufs=1) as constp,
            tc.tile_pool(name="stream", bufs=3) as streamp,
            tc.tile_pool(name="msgs", bufs=3) as msgsp,
            tc.tile_pool(name="work", bufs=12) as workp,
            tc.tile_pool(name="pacc", bufs=PSUM_ACC_BUFS, space="PSUM") as paccp,
            tc.tile_pool(name="pproj", bufs=2, space="PSUM") as pprojp,
            tc.tile_pool(name="dram", bufs=1, space="DRAM") as dramp,
        ):
            wb = constp.tile([128, WBW], f32)
            nc.sync.dma_start(wb[:], wb_in[:])
            W1 = wb[:, 0:128]
            W2 = wb[:, 128 : 128 + COUT]
            b1 = wb[:, 384:385]
            b2 = wb[:64, 385:386]
            iota_bf = constp.tile([128, 128], bf16)
            nc.vector.tensor_copy(iota_bf[:], wb[:, 256:384])

            g_shard = dramp.tile([NSH + 2, COUT], bf16)
            g_full = dramp.tile(
                [NU + 2, 2 * COUT], bf16,
                addr_space="Local" if for_timing else "Shared",
            )
            gsu = g_shard[:].flatten()
            gfu = g_full[:].flatten()

            def gf_view(off, rows):
                return gfu[off : off + rows * 128].rearrange(
                    "(r c) -> r c", c=128
                )

            def gs_view(off, rows):
                return gsu[off : off + rows * 128].rearrange(
                    "(r c) -> r c", c=128
                )

            # layer-2 stream tables: (unit-range x parity) + self even/odd
            l2_tables = []
            for u0 in range(NB2u):
                for q in range(2):
                    rows = min(UBLK, NU - u0 * UBLK)
                    l2_tables.append(gf_view(u0 * UBLK * 128 + q * 64, rows))
            l2_tables.append(gs_view(0, NSH // 2))
            l2_tables.append(gs_view(64, NSH // 2))

            l1_tables = [x_own_in[:, :]]
            for b in range(NB1):
                base = b * BLK
                l1_tables.append(x_in[base : base + min(BLK, N - base), :])

            def aggregate(tables, mm, calls, meta_in, idx_dram, nrow, epilogue):
                """Tile-major consumption: each tile's cells (all blocks)
                accumulate in one PSUM bank; block-major gather streams
                feed the matmuls; `epilogue(t, pt)` consumes the full tile."""
                NBt = len(tables)
                s_ci = [0] * NBt
                s_cur = [(-1, 0)] * NBt
                s_mg = [None] * NBt
                mt = None
                cur_w = -1
                pt = None
                cur_t = -1
                M = mm.shape[0]
                for i in range(M):
                    b, t, c, lo, hi = (int(v) for v in mm[i])
                    cur0, curk = s_cur[b]
                    if s_mg[b] is None or c >= cur0 + curk:
                        cur0, curk = calls[b][s_ci[b]]
                        s_ci[b] += 1
                        s_cur[b] = (cur0, curk)
                        assert cur0 <= c < cur0 + curk
                        idxt = streamp.tile(
                            [128, CALL_CHUNKS * 8], i16, tag=f"idx{b}"
                        )
                        nc.sync.dma_start(
                            idxt[:, : curk * 8],
                            idx_dram[:, cur0 * 8 : (cur0 + curk) * 8],
                        )
                        mg = msgsp.tile(
                            [128, CALL_CHUNKS, 128], bf16, tag=f"msgs{b}"
                        )
                        s_mg[b] = mg
                        nc.gpsimd.dma_gather(
                            mg[:, :curk, :],
                            tables[b],
                            idxt[:, : curk * 8],
                            num_idxs=curk * 128,
                            num_idxs_reg=curk * 128,
                            elem_size=128,
                            single_packet=False,
                        )
                    if i // MW != cur_w:
                        cur_w = i // MW
                        nmw = min(MW, M - cur_w * MW)
                        mt = streamp.tile([128, 2 * MW], f32, tag="meta")
                        nc.sync.dma_start(
                            mt[:, : 2 * nmw],
                            meta_in[:, 2 * cur_w * MW : 2 * (cur_w * MW + nmw)],
                        )
                    if t != cur_t:
                        if pt is not None:
                            epilogue(cur_t, pt)
                        pt = paccp.tile([128, 128], f32, tag="pacc")
                        cur_t = t
                        first = True
                    else:
                        first = False
                    last = (i == M - 1) or (int(mm[i + 1][1]) != t)
                    cl = c - cur0
                    mloc = i - cur_w * MW
                    oh = workp.tile([128, 128], bf16, tag="oh")
                    nc.vector.tensor_scalar(
                        oh[:],
                        iota_bf[:],
                        mt[:, 2 * mloc : 2 * mloc + 1],
                        mt[:, 2 * mloc + 1 : 2 * mloc + 2],
                        mybir.AluOpType.is_equal,
                        mybir.AluOpType.mult,
                    )
                    nc.tensor.matmul(
                        pt[:nrow, :],
                        s_mg[b][:, cl, :nrow],
                        oh[:],
                        start=first,
                        stop=last,
                    )
                epilogue(cur_t, pt)

            # ---------------- layer 1 ----------------
            def epilogue1(t, pt):
                r0 = t * 128
                rows = min(128, NSH - r0)
                aggT = workp.tile([128, 128], f32, tag="aggT")
                nc.scalar.activation(
                    aggT[:], pt[:], mybir.ActivationFunctionType.Copy
                )
                hp = pprojp.tile([128, 128], f32, tag="proj")
                nc.tensor.matmul(hp[:], W1, aggT[:], start=True, stop=True)
                hs = workp.tile([128, 128], f32, tag="hs")
                nc.scalar.activation(
                    hs[:], hp[:], mybir.ActivationFunctionType.Relu,
                    bias=b1, scale=1.0,
                )
                gp = pprojp.tile([128, 128], f32, tag="proj")
                nc.tensor.matmul(gp[:, :COUT], hs[:], W2, start=True, stop=True)
                gs = workp.tile([128, COUT], bf16, tag="gs")
                nc.scalar.activation(
                    gs[:], gp[:, :COUT], mybir.ActivationFunctionType.Copy
                )
                nc.sync.dma_start(g_shard[r0 : r0 + rows, :], gs[:rows, :])

            aggregate(l1_tables, sp["mm1"], calls1, meta1_in, idx1_in, 128,
                      epilogue1)

            # ---------------- exchange ----------------
            if for_timing:
                nc.gpsimd.dma_start(
                    gf_view(0, NSH // 2), gs_view(0, NSH // 2)
                )
            else:
                nc.gpsimd.collective_compute(
                    "AllGather",
                    mybir.AluOpType.bypass,
                    replica_groups=[list(range(NC))],
                    ins=[g_shard[0:NSH, :]],
                    outs=[g_full[:].flatten()[0 : N * COUT].rearrange(
                        "(r c) -> r c", c=128
                    )],
                )

            # ---------------- layer 2 ----------------
            def epilogue2(t, pt):
                r0 = t * 128
                cols = min(128, NSH - r0)
                ob = workp.tile([64, 128], f32, tag="ob")
                nc.scalar.activation(
                    ob[:],
                    pt[:COUT, :],
                    mybir.ActivationFunctionType.Identity,
                    bias=b2,
                    scale=1.0,
                )
                nc.sync.dma_start(outT[:, r0 : r0 + cols], ob[:, :cols])

            aggregate(l2_tables, sp["mm2"], calls2, meta2_in, idx2_in, COUT,
                      epilogue2)

    nc.compile()
    return nc


_CACHE = {}


def _get_program(sp):
    key = (sp["N"], sp["C1"], sp["C2"], sp["mm1"].tobytes(), sp["mm2"].tobytes())
    if key not in _CACHE:
        _CACHE[key] = _build_bass(sp)
    return _CACHE[key]


def _make_wb(W1, b1, W2, b2):
    wb = np.zeros((128, WBW), dtype=np.float32)
    wb[:, 0:128] = np.asarray(W1, dtype=np.float32)
    wb[:, 128 : 128 + COUT] = np.asarray(W2, dtype=np.float32)
    wb[:, 256:384] = np.arange(128, dtype=np.float32)[None, :]
    wb[:, 384] = np.asarray(b1, dtype=np.float32)
    wb[:64, 385] = np.asarray(b2, dtype=np.float32)
    return wb


def make_in_maps(sp, x, W1, b1, W2, b2):
    xb = np.ascontiguousarray(np.asarray(x, dtype=np.float32).astype(BF16))
    wb = _make_wb(W1, b1, W2, b2)
    NSH = sp["NSH"]
    xown = np.zeros((NC, NSH, CIN), dtype=BF16)
    for r in range(NC):
        hi = min(sp["N"], (r + 1) * NSH)
        xown[r, : hi - r * NSH] = xb[r * NSH : hi]
    return [
        {
            "x": xb,
            "x_own": xown[r],
            "idx1": sp["idx1_sb"][r],
            "idx2": sp["idx2_sb"][r],
            "meta1": sp["meta1"][r],
            "meta2": sp["meta2"][r],
            "wb": wb,
        }
        for r in range(NC)
    ]


def kernel(x, edge_index, W1, b1, W2, b2, _trace=False):
    from concourse.bass_utils import run_bass_kernel_spmd

    x = np.asarray(x, dtype=np.float32)
    N = x.shape[0]
    sp = _schedule(np.asarray(edge_index), N)
    nc = _get_program(sp)
    in_maps = make_in_maps(sp, x, W1, b1, W2, b2)
    res = run_bass_kernel_spmd(nc, in_maps, list(range(NC)), trace=_trace)

    NSH = sp["NSH"]
    out = np.empty((N, COUT), dtype=np.float32)
    for r in range(NC):
        lo = r * NSH
        hi = min(N, lo + NSH)
        out[lo:hi] = res.results[r]["outT"][:, sp["pos_all"][r, : hi - lo]].T
    if _trace:
        kernel.last_result = res
    return out


# revision 21
# speedup vs baseline: 1.2165x; 1.0465x over previous
"""2-layer GCN on 8 Trainium2 NeuronCores (Bass/Tile).

Math: gcn_conv(x, W, b) = D^-1/2 (A+I) D^-1/2 (x W) + b.  Propagation
commutes with the weight matmul, so layer 1 aggregates raw x
(h = relu((A_hat x) W1 + b1)) and layer 2 projects first
(out = A_hat (h W2) + b2), minimizing per-edge gather bytes.

Distribution: destination nodes sharded 8 ways (12500/core); each core
gathers source features for its own edges from a full local feature table
(x is an input; g = h W2 is assembled with one 8-rank AllGather).

g is stored at its true width (64 cols bf16 = 128 B rows) so the
AllGather moves half the bytes of a 128-padded layout.  dma_gather
requires 256 B-aligned elements, so layer-2 gathers fetch a 256 B *unit*
(two consecutive g rows) per edge: streams come in (unit-range, parity)
flavors whose table APs are offset by 64 elements so the wanted row
always lands in msgs cols 0:64.

Per-core aggregation, per layer:
- block-major dma_gather streams (int16 idx limit => <=25000-row/unit
  tables). L1: 4 source-range blocks + a SELF stream reading x_own.
  L2: 4 (unit-range x parity) blocks reading g_full + 2 parity SELF
  streams reading g_shard.
- Edges are laid out in fractional per-(tile, block) cells balanced by a
  per-core node permutation (greedy vector bin packing), ~2% padding;
  boundary chunks are consumed by two matmuls with foreign lanes zeroed.
- Consumption is tile-major: bf16 one-hot scatter matrices
  (tensor_scalar is_equal*norm) feed PE matmuls msgs^T @ onehot
  (bf16 -> fp32 PSUM); each destination tile accumulates all its cells
  in one PSUM bank, then a single ACT-engine escape feeds the inline
  projection (W1 -> relu -> W2 -> bf16 g row) or the bias+output write.
"""
import sys

sys.path.insert(0, "/opt/trn_rl_repo")
import numpy as np
import ml_dtypes

BF16 = ml_dtypes.bfloat16
NC = 8
CIN, CH, COUT = 128, 128, 64
CALL_CHUNKS = 36  # gather-call granularity (chunks of 128 edges)
PSUM_ACC_BUFS = 6  # concurrent per-tile accumulation banks
WBW = 386  # wb cols: W1[0:128] W2[128:192] iota[256:384] b1[384] b2[385]


def _balance(nv, caps, k_tb):
    """Best-fit-decreasing node->tile assignment for one core.

    nv: [NSH, NB] per-node block in-degree vectors (both layers' blocks).
    caps: [TILES] node slots per tile.  k_tb: [TILES, NB] cell targets.
    Returns pos[NSH] (node -> global slot = tile*128 + slot_in_tile) or None
    if infeasible.
    """
    NSH, NB = nv.shape
    TILES = caps.shape[0]
    capv = k_tb.astype(np.float64)
    np.maximum(capv, 1e-9, out=capv)
    usedv = np.zeros((TILES, NB), dtype=np.float64)
    usect = np.zeros(TILES, dtype=np.int64)
    pos = np.empty(NSH, dtype=np.int64)
    order = np.argsort(-nv.sum(1), kind="stable")
    for n in order:
        v = nv[n].astype(np.float64)
        post = (usedv + v[None, :]) / capv
        feas = (post <= 1.0).all(1) & (usect < caps)
        if not feas.any():
            return None
        score = np.maximum(post.max(1), (usect + 1) / caps)
        score[~feas] = np.inf
        t = int(np.argmin(score))
        usedv[t] += v
        pos[n] = t * 128 + usect[t]
        usect[t] += 1
    return pos


def _mk_mm(captb, TILES):
    """Tile-major matmul table for a cell-capacity matrix [NB, TILES].

    Returns (mm [(b,t,c,lo,hi)], off_flat, Kb, C).  Chunk numbering is
    global across blocks (block b's chunks follow block b-1's).
    """
    NB = captb.shape[0]
    nkey = NB * TILES
    off_flat = np.zeros(nkey + 1, dtype=np.int64)
    np.cumsum(captb.reshape(-1), out=off_flat[1:])
    CAP = int(off_flat[-1])
    assert CAP % 128 == 0
    # block starts must be chunk-aligned so gather streams stay block-major
    blk_cap = captb.sum(axis=1)
    assert (blk_cap % 128 == 0).all(), blk_cap
    Kb = blk_cap // 128
    C = int(Kb.sum())
    mm = []
    for t in range(TILES):
        for b in range(NB):
            o = int(off_flat[b * TILES + t])
            cap = int(captb[b, t])
            if cap == 0:
                continue
            c0, c1 = o >> 7, (o + cap - 1) >> 7
            for c in range(c0, c1 + 1):
                lo = max(o, c * 128)
                hi = min(o + cap, (c + 1) * 128)
                mm.append((b, t, c, lo, hi))
    return np.array(mm, dtype=np.int64), off_flat, Kb, C


def _fill_slots(sel_by_core, key_all, val_all, dstl_all, norm_all, off_flat,
                capf, CAP):
    """Scatter per-edge records into their cell slots for all cores.

    Returns (idx [NC, CAP] int16, dstl [NC, CAP] f32, norm [NC, CAP] f32).
    """
    idx = np.zeros((NC, CAP), dtype=np.int16)
    dst = np.full((NC, CAP), -1.0, dtype=np.float32)
    nrm = np.zeros((NC, CAP), dtype=np.float32)
    for r in range(NC):
        sel = sel_by_core[r]
        k = key_all[sel]
        order = np.argsort(k, kind="stable")
        sel = sel[order]
        k = k[order]
        nkey = capf.shape[0]
        cr = np.bincount(k, minlength=nkey)
        grp_start = np.zeros(nkey, dtype=np.int64)
        np.cumsum(cr[:-1], out=grp_start[1:])
        rank = np.arange(sel.shape[0], dtype=np.int64) - np.repeat(grp_start, cr)
        assert (rank < capf[k]).all(), "cell overflow"
        slots = off_flat[k] + rank
        idx[r, slots] = val_all[sel].astype(np.int16)
        dst[r, slots] = dstl_all[sel]
        nrm[r, slots] = norm_all[sel]
    return idx, dst, nrm


def _wrap_idx(a):
    NCc, CAP = a.shape
    return np.ascontiguousarray(
        np.tile(a.reshape(NCc, CAP // 16, 16).transpose(0, 2, 1), (1, 8, 1))
    )


def _l1_pair_core(i_src, i_tile, i_dstl, i_norm, TILES, NW, WCAP):
    """Pair one core's L1 edge instances and lay out its x2 trail table.

    Any two same-tile instances may pair (one 512 B descriptor).  Pairs
    form a multigraph over src nodes; Eulerian trails lay it out so each
    pair occupies one table row [x[u] | x[v]].  Returns per-descriptor
    records plus per-window row node sequences.
    """
    order = np.lexsort((i_src, i_tile))
    ts = i_tile[order]
    cnt = np.bincount(ts, minlength=TILES)
    starts = np.zeros(TILES + 1, dtype=np.int64)
    np.cumsum(cnt, out=starts[1:])
    pA_l, pB_l, sing_l = [], [], []
    for t in range(TILES):
        s, e = int(starts[t]), int(starts[t + 1])
        k = (e - s) // 2
        seg = order[s:e]
        pA_l.append(seg[0 : 2 * k : 2])
        pB_l.append(seg[1 : 2 * k : 2])
        if (e - s) % 2:
            sing_l.append(int(seg[-1]))
    pA = np.concatenate(pA_l)
    pB = np.concatenate(pB_l)
    P = pA.shape[0]

    verts, inv = np.unique(
        np.concatenate([i_src[pA], i_src[pB]]), return_inverse=True
    )
    a = inv[:P].astype(np.int64)
    b = inv[P:].astype(np.int64)
    V = verts.shape[0]

    # union-find for components
    parent = np.arange(V, dtype=np.int64)

    def find(x):
        root = x
        while parent[root] != root:
            root = parent[root]
        while parent[x] != root:
            parent[x], x = root, parent[x]
        return root

    for i in range(P):
        ra, rb = find(a[i]), find(b[i])
        if ra != rb:
            parent[ra] = rb
    comp = np.fromiter((find(i) for i in range(V)), np.int64, V)

    # pair odd-degree vertices within components with virtual edges
    deg = np.bincount(a, minlength=V) + np.bincount(b, minlength=V)
    odd = np.nonzero(deg % 2 == 1)[0]
    oorder = odd[np.argsort(comp[odd], kind="stable")]
    va, vb = oorder[0::2], oorder[1::2]
    NE = P + va.shape[0]
    ea = np.concatenate([a, va])
    eb = np.concatenate([b, vb])

    # CSR half-edge adjacency + iterative Hierholzer
    he_v = np.concatenate([ea, eb])
    hstart = np.zeros(V + 1, dtype=np.int64)
    np.cumsum(np.bincount(he_v, minlength=V), out=hstart[1:])
    hlist = np.argsort(he_v, kind="stable")
    ptr = hstart[:-1].copy()
    used = np.zeros(NE, dtype=bool)
    trails = []  # (nodes [k+1], lo_inst [k], hi_inst [k])
    for v0 in range(V):
        while True:
            while ptr[v0] < hstart[v0 + 1] and used[hlist[ptr[v0]] % NE]:
                ptr[v0] += 1
            if ptr[v0] >= hstart[v0 + 1]:
                break
            stack_v = [v0]
            stack_e = []
            circ = []  # edge ids, circuit order
            while stack_v:
                x = stack_v[-1]
                while ptr[x] < hstart[x + 1] and used[hlist[ptr[x]] % NE]:
                    ptr[x] += 1
                if ptr[x] < hstart[x + 1]:
                    e = hlist[ptr[x]] % NE
                    used[e] = True
                    stack_v.append(ea[e] + eb[e] - x)
                    stack_e.append(e)
                else:
                    stack_v.pop()
                    if stack_e:
                        circ.append(stack_e.pop())
            circ.reverse()
            # walk the circuit, splitting at virtual edges
            v = v0
            nodes, lo_i, hi_i = [v], [], []
            for e in circ:
                if e >= P:  # virtual: close current trail
                    if lo_i:
                        trails.append((nodes, lo_i, hi_i))
                    v = ea[e] + eb[e] - v
                    nodes, lo_i, hi_i = [v], [], []
                else:
                    nv_ = ea[e] + eb[e] - v
                    lo = pA[e] if a[e] == v else pB[e]
                    hi = pA[e] + pB[e] - lo
                    if a[e] == b[e]:  # self-loop: either orientation
                        lo, hi = pA[e], pB[e]
                    nodes.append(nv_)
                    lo_i.append(lo)
                    hi_i.append(hi)
                    v = nv_
            if lo_i:
                trails.append((nodes, lo_i, hi_i))

    singles = list(sing_l)  # instance ids; ride an occurrence row
    rdeg = deg.copy()  # remaining real-edge degree
    total_rows = sum(len(n) for n, _, _ in trails)
    budget = NW * WCAP - 256  # headroom for orphan rows below
    # spill: pop trail-end pairs into singles until rows fit the windows
    guard = 0
    while total_rows > budget:
        progress = False
        for tr in trails:
            if total_rows <= budget:
                break
            nodes, lo_i, hi_i = tr
            if not lo_i:
                continue
            u_end = nodes[-1]
            u_prev = nodes[-2]
            d_end = rdeg[u_end] - (2 if u_end == u_prev else 1)
            if d_end < 1:
                continue
            if len(lo_i) == 1 and rdeg[u_prev] - 1 < 1:
                continue
            singles.append(lo_i.pop())
            singles.append(hi_i.pop())
            nodes.pop()
            rdeg[u_end] -= 1
            rdeg[u_prev] -= 1
            total_rows -= 1
            progress = True
        guard += 1
        assert progress and guard < 50, "L1 spill failed"
    trails = [t for t in trails if t[1]]

    # orphan singles: src with no remaining occurrence gets a 1-row trail
    occ_ok = np.zeros(V, dtype=bool)
    occ_ok[np.nonzero(rdeg > 0)[0]] = True
    vmap = {int(v): i for i, v in enumerate(verts)}
    fixed = []
    for s in singles:
        u = int(i_src[s])
        ui = vmap.get(u, -1)
        if ui >= 0 and occ_ok[ui]:
            fixed.append((s, ui))
        else:
            if ui < 0:
                vmap[u] = ui = V
                verts = np.append(verts, u)
                occ_ok = np.append(occ_ok, False)
                V += 1
            trails.append(([ui], [], []))  # 1-row trail, ridden by single
            occ_ok[ui] = True
            total_rows += 1
            fixed.append((s, ui))
    assert total_rows <= NW * WCAP, "L1 rows over budget after orphans"

    # window assignment: greedy vector packing on tile histograms
    tilecnt = np.bincount(i_tile, minlength=TILES).astype(np.float64)
    target = np.maximum(tilecnt / (2 * NW), 1.0)
    cellcnt = np.zeros((NW, TILES), dtype=np.int64)
    rows_used = np.zeros(NW, dtype=np.int64)
    tr_rows = np.array([len(n) for n, _, _ in trails])
    tr_order = np.argsort(-tr_rows, kind="stable")
    win_of = np.empty(len(trails), dtype=np.int64)
    for ti in tr_order:
        nodes, lo_i, hi_i = trails[ti]
        h = np.bincount(i_tile[lo_i], minlength=TILES) if lo_i else 0
        best_w, best_s = -1, None
        for w in range(NW):
            if rows_used[w] + len(nodes) > WCAP:
                continue
            s = ((cellcnt[w] + h) / target).max() if lo_i else (
                rows_used[w] / WCAP
            )
            if best_w < 0 or s < best_s:
                best_w, best_s = w, s
        assert best_w >= 0, "window overflow"
        win_of[ti] = best_w
        rows_used[best_w] += len(nodes)
        if lo_i:
            cellcnt[best_w] += h
    # build per-window row sequences + per-step descriptor records
    rows_w = [[] for _ in range(NW)]
    roff = np.zeros(NW, dtype=np.int64)
    d_key, d_idx = [], []
    d_dl = [[], []]
    d_nm = [[], []]
    first_occ = {}
    for ti, (nodes, lo_i, hi_i) in enumerate(trails):
        w = win_of[ti]
        base = roff[w]
        rows_w[w].append(np.asarray(nodes, dtype=np.int64))
        for k_ in range(len(nodes)):
            u = nodes[k_]
            if u not in first_occ:
                first_occ[u] = (w, base + k_)
        for k_, (lo, hi) in enumerate(zip(lo_i, hi_i)):
            d_key.append(w * TILES + int(i_tile[lo]))
            d_idx.append(base + k_)
            d_dl[0].append(i_dstl[lo])
            d_nm[0].append(i_norm[lo])
            d_dl[1].append(i_dstl[hi])
            d_nm[1].append(i_norm[hi])
        roff[w] += len(nodes)
    for s, ui in fixed:
        w, row = first_occ[ui]
        d_key.append(w * TILES + int(i_tile[s]))
        d_idx.append(row)
        d_dl[0].append(i_dstl[s])
        d_nm[0].append(i_norm[s])
        d_dl[1].append(-1.0)
        d_nm[1].append(0.0)
    key = np.asarray(d_key, dtype=np.int64)
    cell_final = np.bincount(key, minlength=NW * TILES).reshape(NW, TILES)
    return dict(
        cellcnt=cell_final,
        key=key,
        idxv=np.asarray(d_idx, dtype=np.int64),
        dstl=[np.asarray(d_dl[0], np.float32), np.asarray(d_dl[1], np.float32)],
        norm=[np.asarray(d_nm[0], np.float32), np.asarray(d_nm[1], np.float32)],
        rows_w=[
            verts[np.concatenate(rw)] if rw else np.zeros(0, dtype=np.int64)
            for rw in rows_w
        ],
    )


def _mk_meta_h(mmx, dstl_h, norm_h):
    """Meta for 6-column mm rows (b, t, c, lo, hi, half)."""
    M = mmx.shape[0]
    md = np.full((NC, M, 128), -1.0, dtype=np.float32)
    mn = np.zeros((NC, M, 128), dtype=np.float32)
    for i in range(M):
        b, t, c, lo, hi, h = mmx[i]
        base = int(c) * 128
        md[:, i, lo - base : hi - base] = dstl_h[h][:, lo:hi]
        mn[:, i, lo - base : hi - base] = norm_h[h][:, lo:hi]
    meta = np.empty((NC, 128, 2 * M), dtype=np.float32)
    meta[:, :, 0::2] = md.transpose(0, 2, 1)
    meta[:, :, 1::2] = mn.transpose(0, 2, 1)
    return meta


def _mk_meta(mm, dstl_all, norm_all):
    M = mm.shape[0]
    md = np.full((NC, M, 128), -1.0, dtype=np.float32)
    mn = np.zeros((NC, M, 128), dtype=np.float32)
    for i in range(M):
        b, t, c, lo, hi = mm[i]
        base = int(c) * 128
        md[:, i, lo - base : hi - base] = dstl_all[:, lo:hi]
        mn[:, i, lo - base : hi - base] = norm_all[:, lo:hi]
    meta = np.empty((NC, 128, 2 * M), dtype=np.float32)
    meta[:, :, 0::2] = md.transpose(0, 2, 1)
    meta[:, :, 1::2] = mn.transpose(0, 2, 1)
    return meta


def _schedule(edge_index, n_nodes):
    """Static SPMD schedule + per-core edge arrays for both layers."""
    N = n_nodes
    NSH = (N + NC - 1) // NC
    TILES = (NSH + 127) // 128
    BLK = 25000  # L1 x-table rows per block
    NB1 = -(-N // BLK)  # L1 source-range blocks
    UBLK = 25000  # L2 units per block (unit = 2 nodes)
    NU = (N + 1) // 2
    NB2u = -(-NU // UBLK)  # L2 unit-range blocks
    assert NSH % 2 == 0

    src = np.asarray(edge_index[0], dtype=np.int64)
    dst = np.asarray(edge_index[1], dtype=np.int64)
    E = src.shape[0]
    deg = np.bincount(dst, minlength=N).astype(np.float64) + 1.0
    dinv = 1.0 / np.sqrt(deg)
    w_all = (dinv[src] * dinv[dst]).astype(np.float32)

    core = dst // NSH
    j_all = dst - core * NSH
    blk1 = src // BLK

    # per-core per-node block in-degree vectors for balance:
    # L1 blocks (src range, self excluded) ++ L2 blocks (unit range x parity,
    # self excluded -- self parity depends on pos, handled by SELF streams)
    sc = src // NSH  # src core
    ub = sc // 4  # L2 unit-range block (aligned with shard pairs)
    nv1 = np.bincount(
        (core * NSH + j_all) * NB1 + blk1, minlength=NC * NSH * NB1
    ).reshape(NC, NSH, NB1)
    B1 = nv1.sum(axis=1)

    caps = np.full(TILES, 128, dtype=np.int64)
    caps[TILES - 1] = NSH - 128 * (TILES - 1)

    # The balance sees L1 source-range blocks and L2 unit-range blocks
    # (parity-summed); L2 parity cells are then sized exactly after an
    # explicit parity assignment pass below.
    nv2u = np.bincount(
        (core * NSH + j_all) * NB2u + ub, minlength=NC * NSH * NB2u
    ).reshape(NC, NSH, NB2u)
    B2u = nv2u.sum(axis=1)  # [NC, NB2u]

    def spread(tot):
        tgt = tot * caps / NSH
        base = np.floor(tgt).astype(np.int64)
        rem = int(tot - base.sum())
        order = np.argsort(-(tgt - base))
        base[order[:rem]] += 1
        return base

    slack = 256
    for _attempt in range(8):
        TOT1 = ((B1.max(axis=0) + slack + 127) // 128) * 128  # [NB1]
        TOT2u = B2u.max(axis=0) + 2 * slack  # [NB2u] (both parities)
        captb1 = np.stack([spread(t) for t in TOT1])  # [NB1, TILES]
        captb2u = np.stack([spread(t) for t in TOT2u])  # [NB2u, TILES]
        k_tb = np.concatenate([captb1.T, captb2u.T], axis=1)
        nv = np.concatenate([nv1, nv2u], axis=2)
        pos_all = np.empty((NC, NSH), dtype=np.int64)
        ok = True
        for r in range(NC):
            pos = _balance(nv[r], caps, k_tb)
            if pos is None:
                ok = False
                break
            pos_all[r] = pos
        if ok:
            break
        slack *= 2
    assert ok, "balance failed"

    # ---- explicit parity assignment (within-tile slot reshuffle) ----
    # A src node's position parity decides which L2 parity stream its
    # out-edges ride at every consumer.  Within each producer tile the
    # even/odd slot counts are fixed; which node takes which parity is
    # free.  Greedy discrepancy minimization over consumer (core, tile)
    # cells keeps every parity cell near half its unit-range cell.
    tile_of_e = pos_all[core, j_all] >> 7  # dst tile per edge (stable)
    cell_of_e = core * TILES + tile_of_e
    order_e = np.argsort(src, kind="stable")
    e_sorted = cell_of_e[order_e]
    src_sorted = src[order_e]
    starts = np.searchsorted(src_sorted, np.arange(N + 1))
    parity = np.zeros(N, dtype=np.int64)
    for g in range(NB2u):
        D = np.zeros(NC * TILES, dtype=np.int64)
        # nodes of producer cores 4g..4g+3, grouped by (core, tile)
        for r in range(4 * g, min(4 * g + 4, NC)):
            nval = min(NSH, N - r * NSH)
            p = pos_all[r, :nval]
            t = p >> 7
            for tt in range(TILES):
                nodes = np.nonzero(t == tt)[0] + r * NSH
                if nodes.size == 0:
                    continue
                degs = starts[nodes + 1] - starts[nodes]
                nodes = nodes[np.argsort(-degs, kind="stable")]
                n_even = (nodes.size + 1) // 2
                ev_left, od_left = n_even, nodes.size - n_even
                for n in nodes:
                    cells = e_sorted[starts[n] : starts[n + 1]]
                    if ev_left == 0:
                        p_n = 1
                    elif od_left == 0:
                        p_n = 0
                    else:
                        p_n = 0 if D[cells].sum() <= 0 else 1
                    parity[n] = p_n
                    if p_n == 0:
                        ev_left -= 1
                        np.add.at(D, cells, 1)
                    else:
                        od_left -= 1
                        np.add.at(D, cells, -1)
    # reassign within-tile slots by parity class
    for r in range(NC):
        nval = min(NSH, N - r * NSH)
        p = pos_all[r, :nval]
        t = p >> 7
        par_n = parity[r * NSH : r * NSH + nval]
        newpos = np.empty(nval, dtype=np.int64)
        for tt in range(TILES):
            nodes = np.nonzero(t == tt)[0]
            if nodes.size == 0:
                continue
            ev = nodes[par_n[nodes] == 0]
            od = nodes[par_n[nodes] == 1]
            newpos[ev] = tt * 128 + 2 * np.arange(ev.size)
            newpos[od] = tt * 128 + 2 * np.arange(od.size) + 1
        pos_all[r, :nval] = newpos

    pos_e = pos_all[core, j_all]
    tile_of = pos_e >> 7
    dstl = (pos_e & 127).astype(np.float32)

    # permuted g row of every src node
    src_pos = pos_all[sc, src - sc * NSH]
    g_row = sc * NSH + src_pos
    par = (g_row & 1).astype(np.int64)

    # ---------------- layer 1: paired descriptors ----------------
    # Each 512 B descriptor serves TWO same-tile edges: the per-core x2
    # table row i holds [x[a_i] | x[a_{i+1}]] along an Eulerian trail of
    # the chosen pair multigraph, so paired srcs sit in one row.  Blocks
    # are fixed 32760-row table windows (int16 idx).
    sel_by_core = [np.nonzero(core == r)[0] for r in range(NC)]
    WCAP = 32760
    NW = 4
    cores_l1 = []
    for r in range(NC):
        sel = sel_by_core[r]
        nval = min(NSH, N - r * NSH)
        jj = np.arange(nval, dtype=np.int64)
        p_self = pos_all[r, :nval]
        i_src = np.concatenate([src[sel], r * NSH + jj])
        i_tile = np.concatenate([tile_of[sel], p_self >> 7])
        i_dstl = np.concatenate(
            [dstl[sel], (p_self & 127).astype(np.float32)]
        )
        i_norm = np.concatenate(
            [w_all[sel], (dinv[r * NSH + jj] ** 2).astype(np.float32)]
        )
        cores_l1.append(
            _l1_pair_core(i_src, i_tile, i_dstl, i_norm, TILES, NW, WCAP)
        )
    # shared cell caps: max over cores, block sums chunk-aligned
    captb1 = np.zeros((NW, TILES), dtype=np.int64)
    for cd in cores_l1:
        np.maximum(captb1, cd["cellcnt"], out=captb1)
    for w in range(NW):
        rem = int(captb1[w].sum()) % 128
        if rem:
            captb1[w, TILES - 1] += 128 - rem
    mm1, off1, Kb1, C1 = _mk_mm(captb1, TILES)
    CAP1 = C1 * 128
    capf1 = captb1.reshape(-1)
    idx1 = np.zeros((NC, CAP1), dtype=np.int16)
    dstl1h = np.full((2, NC, CAP1), -1.0, dtype=np.float32)
    norm1h = np.zeros((2, NC, CAP1), dtype=np.float32)
    for r in range(NC):
        cd = cores_l1[r]
        k = cd["key"]
        order = np.argsort(k, kind="stable")
        k = k[order]
        nkey = capf1.shape[0]
        cr = np.bincount(k, minlength=nkey)
        grp_start = np.zeros(nkey, dtype=np.int64)
        np.cumsum(cr[:-1], out=grp_start[1:])
        rank = np.arange(k.shape[0], dtype=np.int64) - np.repeat(grp_start, cr)
        assert (rank < capf1[k]).all(), "L1 cell overflow"
        slots = off1[k] + rank
        idx1[r, slots] = cd["idxv"][order].astype(np.int16)
        for h in range(2):
            dstl1h[h, r, slots] = cd["dstl"][h][order]
            norm1h[h, r, slots] = cd["norm"][h][order]
    # expand mm rows: one matmul per (desc-chunk, half)
    mm1x = np.empty((mm1.shape[0] * 2, 6), dtype=np.int64)
    mm1x[0::2, :5] = mm1
    mm1x[1::2, :5] = mm1
    mm1x[0::2, 5] = 0
    mm1x[1::2, 5] = 1
    meta1 = _mk_meta_h(mm1x, dstl1h, norm1h)

    # ---------------- layer 2 cells ----------------
    # blocks 0..2*NB2u-1 = (unit range x parity) reading g_full;
    # blocks -2,-1 = SELF even/odd (tile-aligned, cap 64/tile, read g_shard)
    # Parity cells sized exactly: max over cores + chunk-align block sums.
    NBm = 2 * NB2u
    cnt2 = np.bincount(
        (core * TILES + tile_of) * NBm + (ub * 2 + par),
        minlength=NC * TILES * NBm,
    ).reshape(NC, TILES, NBm)
    captb2m = cnt2.max(axis=0).T.copy()  # [NBm, TILES]
    for b in range(NBm):
        rem = int(captb2m[b].sum()) % 128
        if rem:
            captb2m[b, TILES - 1] += 128 - rem
    half = np.zeros((2, TILES), dtype=np.int64)
    for t in range(TILES):
        cap_t = int(caps[t])
        half[0, t] = (cap_t + 1) // 2  # even positions in tile
        half[1, t] = cap_t // 2
    # pad self blocks to chunk multiples (dead lanes at the very end)
    selfpad = np.zeros((2, TILES), dtype=np.int64)
    for q in range(2):
        rem = int(half[q].sum()) % 128
        if rem:
            selfpad[q, TILES - 1] = 128 - rem
    captb2f = np.concatenate([captb2m, half + selfpad], axis=0)
    mm2, off2, Kb2, C2 = _mk_mm(captb2f, TILES)
    CAP2 = C2 * 128
    key2 = (ub * 2 + par) * TILES + tile_of
    val2 = (g_row >> 1) - ub * UBLK
    idx2, dstl2, norm2 = _fill_slots(
        sel_by_core, key2, val2, dstl, w_all, off2, captb2f.reshape(-1), CAP2
    )
    # L2 self: node at pos p -> self block (parity of p), slot = rank of p
    # among same-parity positions of its tile (tile-major order)
    for r in range(NC):
        nval = min(NSH, N - r * NSH)
        jj = np.arange(nval, dtype=np.int64)
        p = pos_all[r, :nval]
        t = p >> 7
        q = p & 1
        # slot within (self block q): offset of tile + rank among parity
        # positions of the tile, ordered by p
        order = np.argsort(p, kind="stable")
        slots = np.empty(nval, dtype=np.int64)
        for qq in range(2):
            selq = order[q[order] == qq]
            tq = t[selq]
            # rank within tile
            start = np.zeros(TILES, dtype=np.int64)
            cnts = np.bincount(tq, minlength=TILES)
            np.cumsum(cnts[:-1], out=start[1:])
            rank = np.arange(selq.shape[0], dtype=np.int64) - np.repeat(
                start, cnts
            )
            cellbase = off2[(NBm + qq) * TILES + tq]
            slots[selq] = cellbase + rank
        idx2[r, slots] = (p >> 1).astype(np.int16)
        dstl2[r, slots] = (p & 127).astype(np.float32)
        norm2[r, slots] = (dinv[r * NSH + jj] ** 2).astype(np.float32)
    meta2 = _mk_meta(mm2, dstl2, norm2)
    mm2x = np.concatenate(
        [mm2, np.zeros((mm2.shape[0], 1), dtype=np.int64)], axis=1
    )

    l1_rows = [cores_l1[r]["rows_w"] for r in range(NC)]
    return dict(
        N=N, E=E, NSH=NSH, TILES=TILES, NB2u=NB2u, UBLK=UBLK,
        NW=NW, WCAP=WCAP, l1_rows=l1_rows,
        C1=C1, M1=mm1x.shape[0], mm1=mm1x, Kb1=Kb1,
        C2=C2, M2=mm2x.shape[0], mm2=mm2x, Kb2=Kb2,
        idx1_sb=_wrap_idx(idx1), idx2_sb=_wrap_idx(idx2),
        meta1=meta1, meta2=meta2,
        pos_all=pos_all,
    )


def _build_bass(sp, for_timing=False):
    import concourse.bass as bass
    import concourse.bacc as bacc
    import concourse.mybir as mybir
    import concourse.tile as tile

    f32 = mybir.dt.float32
    bf16 = mybir.dt.bfloat16
    i16 = mybir.dt.int16
    N, NSH, TILES = sp["N"], sp["NSH"], sp["TILES"]
    NW, WCAP = sp["NW"], sp["WCAP"]
    UBLK, NB2u = sp["UBLK"], sp["NB2u"]
    NU = (N + 1) // 2
    NPAD = TILES * 128

    def mk_calls(Kb, cc):
        calls = []
        chunk0 = 0
        for kb in (int(k) for k in Kb):
            lst = []
            s0, left = chunk0, kb
            while left > 0:
                cs = min(cc, left)
                lst.append((s0, cs))
                s0 += cs
                left -= cs
            calls.append(lst)
            chunk0 += kb
        return calls

    CC1, CC2 = 16, 24
    calls1 = mk_calls(sp["Kb1"], CC1)
    calls2 = mk_calls(sp["Kb2"], CC2)
    MW = 128  # meta window (matmuls per meta tile)

    nc = bacc.Bacc("TRN2", target_bir_lowering=False, debug=False, num_devices=NC)
    x2_in = nc.dram_tensor("x2", [NW * WCAP + 2, 256], bf16, kind="ExternalInput")
    idx1_in = nc.dram_tensor("idx1", [128, sp["C1"] * 8], i16, kind="ExternalInput")
    idx2_in = nc.dram_tensor("idx2", [128, sp["C2"] * 8], i16, kind="ExternalInput")
    meta1_in = nc.dram_tensor("meta1", [128, 2 * sp["M1"]], f32, kind="ExternalInput")
    meta2_in = nc.dram_tensor("meta2", [128, 2 * sp["M2"]], f32, kind="ExternalInput")
    wb_in = nc.dram_tensor("wb", [128, WBW], f32, kind="ExternalInput")
    outT = nc.dram_tensor("outT", [COUT, NPAD], f32, kind="ExternalOutput")

    with tile.TileContext(nc) as tc:
        with (
            tc.tile_pool(name="const", bufs=1) as constp,
            tc.tile_pool(name="stream", bufs=3) as streamp,
            tc.tile_pool(name="msgs", bufs=2) as msgsp,
            tc.tile_pool(name="work", bufs=12) as workp,
            tc.tile_pool(name="pacc", bufs=PSUM_ACC_BUFS, space="PSUM") as paccp,
            tc.tile_pool(name="pproj", bufs=2, space="PSUM") as pprojp,
            tc.tile_pool(name="dram", bufs=1, space="DRAM") as dramp,
        ):
            wb = constp.tile([128, WBW], f32)
            nc.sync.dma_start(wb[:], wb_in[:])
            W1 = wb[:, 0:128]
            W2 = wb[:, 128 : 128 + COUT]
            b1 = wb[:, 384:385]
            b2 = wb[:64, 385:386]
            iota_bf = constp.tile([128, 128], bf16)
            nc.vector.tensor_copy(iota_bf[:], wb[:, 256:384])

            g_shard = dramp.tile([NSH + 2, COUT], bf16)
            g_full = dramp.tile(
                [NU + 2, 2 * COUT], bf16,
                addr_space="Local" if for_timing else "Shared",
            )
            gsu = g_shard[:].flatten()
            gfu = g_full[:].flatten()

            def gf_view(off, rows):
                return gfu[off : off + rows * 128].rearrange(
                    "(r c) -> r c", c=128
                )

            def gs_view(off, rows):
                return gsu[off : off + rows * 128].rearrange(
                    "(r c) -> r c", c=128
                )

            # layer-2 stream tables: (unit-range x parity) + self even/odd
            l2_tables = []
            for u0 in range(NB2u):
                for q in range(2):
                    rows = min(UBLK, NU - u0 * UBLK)
                    l2_tables.append(gf_view(u0 * UBLK * 128 + q * 64, rows))
            l2_tables.append(gs_view(0, NSH // 2))
            l2_tables.append(gs_view(64, NSH // 2))

            l1_tables = [
                x2_in[w * WCAP : w * WCAP + WCAP + 1, :] for w in range(NW)
            ]

            def aggregate(tables, mm, calls, meta_in, idx_dram, nrow, elem,
                          cc, epilogue):
                """Tile-major consumption: each tile's cells (all blocks)
                accumulate in one PSUM bank; block-major gather streams
                feed the matmuls; `epilogue(t, pt)` consumes the full tile."""
                NBt = len(tables)
                s_ci = [0] * NBt
                s_cur = [(-1, 0)] * NBt
                s_mg = [None] * NBt
                mt = None
                cur_w = -1
                pt = None
                cur_t = -1
                M = mm.shape[0]
                for i in range(M):
                    b, t, c, lo, hi, half = (int(v) for v in mm[i])
                    cur0, curk = s_cur[b]
                    if s_mg[b] is None or c >= cur0 + curk:
                        cur0, curk = calls[b][s_ci[b]]
                        s_ci[b] += 1
                        s_cur[b] = (cur0, curk)
                        assert cur0 <= c < cur0 + curk
                        idxt = streamp.tile(
                            [128, cc * 8], i16, tag=f"idx{elem}_{b}"
                        )
                        nc.sync.dma_start(
                            idxt[:, : curk * 8],
                            idx_dram[:, cur0 * 8 : (cur0 + curk) * 8],
                        )
                        mg = msgsp.tile(
                            [128, cc, elem], bf16, tag=f"msgs{elem}_{b}"
                        )
                        s_mg[b] = mg
                        nc.gpsimd.dma_gather(
                            mg[:, :curk, :],
                            tables[b],
                            idxt[:, : curk * 8],
                            num_idxs=curk * 128,
                            num_idxs_reg=curk * 128,
                            elem_size=elem,
                            single_packet=False,
                        )
                    if i // MW != cur_w:
                        cur_w = i // MW
                        nmw = min(MW, M - cur_w * MW)
                        mt = streamp.tile([128, 2 * MW], f32, tag="meta")
                        nc.sync.dma_start(
                            mt[:, : 2 * nmw],
                            meta_in[:, 2 * cur_w * MW : 2 * (cur_w * MW + nmw)],
                        )
                    if t != cur_t:
                        if pt is not None:
                            epilogue(cur_t, pt)
                        pt = paccp.tile([128, 128], f32, tag="pacc")
                        cur_t = t
                        first = True
                    else:
                        first = False
                    last = (i == M - 1) or (int(mm[i + 1][1]) != t)
                    cl = c - cur0
                    mloc = i - cur_w * MW
                    oh = workp.tile([128, 128], bf16, tag="oh")
                    nc.vector.tensor_scalar(
                        oh[:],
                        iota_bf[:],
                        mt[:, 2 * mloc : 2 * mloc + 1],
                        mt[:, 2 * mloc + 1 : 2 * mloc + 2],
                        mybir.AluOpType.is_equal,
                        mybir.AluOpType.mult,
                    )
                    co = half * 128
                    nc.tensor.matmul(
                        pt[:nrow, :],
                        s_mg[b][:, cl, co : co + nrow],
                        oh[:],
                        start=first,
                        stop=last,
                    )
                epilogue(cur_t, pt)

            # ---------------- layer 1 ----------------
            def epilogue1(t, pt):
                r0 = t * 128
                rows = min(128, NSH - r0)
                aggT = workp.tile([128, 128], f32, tag="aggT")
                nc.scalar.activation(
                    aggT[:], pt[:], mybir.ActivationFunctionType.Copy
                )
                hp = pprojp.tile([128, 128], f32, tag="proj")
                nc.tensor.matmul(hp[:], W1, aggT[:], start=True, stop=True)
                hs = workp.tile([128, 128], f32, tag="hs")
                nc.scalar.activation(
                    hs[:], hp[:], mybir.ActivationFunctionType.Relu,
                    bias=b1, scale=1.0,
                )
                gp = pprojp.tile([128, 128], f32, tag="proj")
                nc.tensor.matmul(gp[:, :COUT], hs[:], W2, start=True, stop=True)
                gs = workp.tile([128, COUT], bf16, tag="gs")
                nc.scalar.activation(
                    gs[:], gp[:, :COUT], mybir.ActivationFunctionType.Copy
                )
                nc.sync.dma_start(g_shard[r0 : r0 + rows, :], gs[:rows, :])

            aggregate(l1_tables, sp["mm1"], calls1, meta1_in, idx1_in, 128,
                      256, CC1, epilogue1)

            # ---------------- exchange ----------------
            if for_timing:
                nc.gpsimd.dma_start(
                    gf_view(0, NSH // 2), gs_view(0, NSH // 2)
                )
            else:
                nc.gpsimd.collective_compute(
                    "AllGather",
                    mybir.AluOpType.bypass,
                    replica_groups=[list(range(NC))],
                    ins=[g_shard[0:NSH, :]],
                    outs=[g_full[:].flatten()[0 : N * COUT].rearrange(
                        "(r c) -> r c", c=128
                    )],
                )

            # ---------------- layer 2 ----------------
            def epilogue2(t, pt):
                r0 = t * 128
                cols = min(128, NSH - r0)
                ob = workp.tile([64, 128], f32, tag="ob")
                nc.scalar.activation(
                    ob[:],
                    pt[:COUT, :],
                    mybir.ActivationFunctionType.Identity,
                    bias=b2,
                    scale=1.0,
                )
                nc.sync.dma_start(outT[:, r0 : r0 + cols], ob[:, :cols])

            aggregate(l2_tables, sp["mm2"], calls2, meta2_in, idx2_in, COUT,
                      128, CC2, epilogue2)

    nc.compile()
    return nc


_CACHE = {}


def _get_program(sp):
    key = (sp["N"], sp["C1"], sp["C2"], sp["mm1"].tobytes(), sp["mm2"].tobytes())
    if key not in _CACHE:
        _CACHE[key] = _build_bass(sp)
    return _CACHE[key]


def _make_wb(W1, b1, W2, b2):
    wb = np.zeros((128, WBW), dtype=np.float32)
    wb[:, 0:128] = np.asarray(W1, dtype=np.float32)
    wb[:, 128 : 128 + COUT] = np.asarray(W2, dtype=np.float32)
    wb[:, 256:384] = np.arange(128, dtype=np.float32)[None, :]
    wb[:, 384] = np.asarray(b1, dtype=np.float32)
    wb[:64, 385] = np.asarray(b2, dtype=np.float32)
    return wb


def make_in_maps(sp, x, W1, b1, W2, b2):
    xb = np.ascontiguousarray(np.asarray(x, dtype=np.float32).astype(BF16))
    wb = _make_wb(W1, b1, W2, b2)
    NW, WCAP = sp["NW"], sp["WCAP"]
    maps = []
    for r in range(NC):
        x2 = np.zeros((NW * WCAP + 2, 256), dtype=BF16)
        for w, nodes in enumerate(sp["l1_rows"][r]):
            R = nodes.shape[0]
            base = w * WCAP
            x2[base : base + R, 0:128] = xb[nodes]
            if R > 1:
                x2[base : base + R - 1, 128:256] = xb[nodes[1:]]
        maps.append(
            {
                "x2": x2,
                "idx1": sp["idx1_sb"][r],
                "idx2": sp["idx2_sb"][r],
                "meta1": sp["meta1"][r],
                "meta2": sp["meta2"][r],
                "wb": wb,
            }
        )
    return maps


def kernel(x, edge_index, W1, b1, W2, b2, _trace=False):
    from concourse.bass_utils import run_bass_kernel_spmd

    x = np.asarray(x, dtype=np.float32)
    N = x.shape[0]
    sp = _schedule(np.asarray(edge_index), N)
    nc = _get_program(sp)
    in_maps = make_in_maps(sp, x, W1, b1, W2, b2)
    res = run_bass_kernel_spmd(nc, in_maps, list(range(NC)), trace=_trace)

    NSH = sp["NSH"]
    out = np.empty((N, COUT), dtype=np.float32)
    for r in range(NC):
        lo = r * NSH
        hi = min(N, lo + NSH)
        out[lo:hi] = res.results[r]["outT"][:, sp["pos_all"][r, : hi - lo]].T
    if _trace:
        kernel.last_result = res
    return out


# revision 31
# speedup vs baseline: 1.4056x; 1.1554x over previous
"""2-layer GCN on 8 Trainium2 NeuronCores (Bass/Tile).

Math: gcn_conv(x, W, b) = D^-1/2 (A+I) D^-1/2 (x W) + b.  Propagation
commutes with the weight matmul, so layer 1 aggregates raw x
(h = relu((A_hat x) W1 + b1)) and layer 2 projects first
(out = A_hat (h W2) + b2), minimizing per-edge gather bytes.

Distribution: destination nodes sharded 8 ways (12500/core); each core
gathers source features for its own edges from a full local feature table
(x is an input; g = h W2 is assembled with one 8-rank AllGather).

g is stored at its true width (64 cols bf16 = 128 B rows) so the
AllGather moves half the bytes of a 128-padded layout.  dma_gather
requires 256 B-aligned elements, so layer-2 gathers fetch a 256 B *unit*
(two consecutive g rows) per edge: streams come in (unit-range, parity)
flavors whose table APs are offset by 64 elements so the wanted row
always lands in msgs cols 0:64.

Per-core aggregation, per layer:
- block-major dma_gather streams (int16 idx limit => <=25000-row/unit
  tables). L1: 4 source-range blocks + a SELF stream reading x_own.
  L2: 4 (unit-range x parity) blocks reading g_full + 2 parity SELF
  streams reading g_shard.
- Edges are laid out in fractional per-(tile, block) cells balanced by a
  per-core node permutation (greedy vector bin packing), ~2% padding;
  boundary chunks are consumed by two matmuls with foreign lanes zeroed.
- Consumption is tile-major: bf16 one-hot scatter matrices
  (tensor_scalar is_equal*norm) feed PE matmuls msgs^T @ onehot
  (bf16 -> fp32 PSUM); each destination tile accumulates all its cells
  in one PSUM bank, then a single ACT-engine escape feeds the inline
  projection (W1 -> relu -> W2 -> bf16 g row) or the bias+output write.
"""
import sys

sys.path.insert(0, "/opt/trn_rl_repo")
import numpy as np
import ml_dtypes

BF16 = ml_dtypes.bfloat16
NC = 8
CIN, CH, COUT = 128, 128, 64
CALL_CHUNKS = 36  # gather-call granularity (chunks of 128 edges)
PSUM_ACC_BUFS = 6  # concurrent per-tile accumulation banks
WBW = 386  # wb cols: W1[0:128] W2[128:192] iota[256:384] b1[384] b2[385]


def _balance(nv, caps, k_tb):
    """Best-fit-decreasing node->tile assignment for one core.

    nv: [NSH, NB] per-node block in-degree vectors (both layers' blocks).
    caps: [TILES] node slots per tile.  k_tb: [TILES, NB] cell targets.
    Returns pos[NSH] (node -> global slot = tile*128 + slot_in_tile) or None
    if infeasible.
    """
    NSH, NB = nv.shape
    TILES = caps.shape[0]
    capv = k_tb.astype(np.float64)
    np.maximum(capv, 1e-9, out=capv)
    usedv = np.zeros((TILES, NB), dtype=np.float64)
    usect = np.zeros(TILES, dtype=np.int64)
    pos = np.empty(NSH, dtype=np.int64)
    order = np.argsort(-nv.sum(1), kind="stable")
    for n in order:
        v = nv[n].astype(np.float64)
        post = (usedv + v[None, :]) / capv
        feas = (post <= 1.0).all(1) & (usect < caps)
        if not feas.any():
            return None
        score = np.maximum(post.max(1), (usect + 1) / caps)
        score[~feas] = np.inf
        t = int(np.argmin(score))
        usedv[t] += v
        pos[n] = t * 128 + usect[t]
        usect[t] += 1
    return pos


def _mk_mm(captb, TILES):
    """Tile-major matmul table for a cell-capacity matrix [NB, TILES].

    Returns (mm [(b,t,c,lo,hi)], off_flat, Kb, C).  Chunk numbering is
    global across blocks (block b's chunks follow block b-1's).
    """
    NB = captb.shape[0]
    nkey = NB * TILES
    off_flat = np.zeros(nkey + 1, dtype=np.int64)
    np.cumsum(captb.reshape(-1), out=off_flat[1:])
    CAP = int(off_flat[-1])
    assert CAP % 128 == 0
    # block starts must be chunk-aligned so gather streams stay block-major
    blk_cap = captb.sum(axis=1)
    assert (blk_cap % 128 == 0).all(), blk_cap
    Kb = blk_cap // 128
    C = int(Kb.sum())
    mm = []
    for t in range(TILES):
        for b in range(NB):
            o = int(off_flat[b * TILES + t])
            cap = int(captb[b, t])
            if cap == 0:
                continue
            c0, c1 = o >> 7, (o + cap - 1) >> 7
            for c in range(c0, c1 + 1):
                lo = max(o, c * 128)
                hi = min(o + cap, (c + 1) * 128)
                mm.append((b, t, c, lo, hi))
    return np.array(mm, dtype=np.int64), off_flat, Kb, C


def _fill_slots(sel_by_core, key_all, val_all, dstl_all, norm_all, off_flat,
                capf, CAP):
    """Scatter per-edge records into their cell slots for all cores.

    Returns (idx [NC, CAP] int16, dstl [NC, CAP] f32, norm [NC, CAP] f32).
    """
    idx = np.zeros((NC, CAP), dtype=np.int16)
    dst = np.full((NC, CAP), -1.0, dtype=np.float32)
    nrm = np.zeros((NC, CAP), dtype=np.float32)
    for r in range(NC):
        sel = sel_by_core[r]
        k = key_all[sel]
        order = np.argsort(k, kind="stable")
        sel = sel[order]
        k = k[order]
        nkey = capf.shape[0]
        cr = np.bincount(k, minlength=nkey)
        grp_start = np.zeros(nkey, dtype=np.int64)
        np.cumsum(cr[:-1], out=grp_start[1:])
        rank = np.arange(sel.shape[0], dtype=np.int64) - np.repeat(grp_start, cr)
        assert (rank < capf[k]).all(), "cell overflow"
        slots = off_flat[k] + rank
        idx[r, slots] = val_all[sel].astype(np.int16)
        dst[r, slots] = dstl_all[sel]
        nrm[r, slots] = norm_all[sel]
    return idx, dst, nrm


def _wrap_idx(a):
    NCc, CAP = a.shape
    return np.ascontiguousarray(
        np.tile(a.reshape(NCc, CAP // 16, 16).transpose(0, 2, 1), (1, 8, 1))
    )


def _l1_pair_core(i_src, i_tile, i_dstl, i_norm, TILES, NW, WCAP):
    """Pair one core's L1 edge instances and lay out its x2 trail table.

    Any two same-tile instances may pair (one 512 B descriptor).  Pairs
    form a multigraph over src nodes; Eulerian trails lay it out so each
    pair occupies one table row [x[u] | x[v]].  Returns per-descriptor
    records plus per-window row node sequences.
    """
    order = np.lexsort((i_src, i_tile))
    ts = i_tile[order]
    cnt = np.bincount(ts, minlength=TILES)
    starts = np.zeros(TILES + 1, dtype=np.int64)
    np.cumsum(cnt, out=starts[1:])
    pA_l, pB_l, sing_l = [], [], []
    for t in range(TILES):
        s, e = int(starts[t]), int(starts[t + 1])
        k = (e - s) // 2
        seg = order[s:e]
        pA_l.append(seg[0 : 2 * k : 2])
        pB_l.append(seg[1 : 2 * k : 2])
        if (e - s) % 2:
            sing_l.append(int(seg[-1]))
    pA = np.concatenate(pA_l)
    pB = np.concatenate(pB_l)
    P = pA.shape[0]

    verts, inv = np.unique(
        np.concatenate([i_src[pA], i_src[pB]]), return_inverse=True
    )
    a = inv[:P].astype(np.int64)
    b = inv[P:].astype(np.int64)
    V = verts.shape[0]

    # union-find for components
    parent = np.arange(V, dtype=np.int64)

    def find(x):
        root = x
        while parent[root] != root:
            root = parent[root]
        while parent[x] != root:
            parent[x], x = root, parent[x]
        return root

    for i in range(P):
        ra, rb = find(a[i]), find(b[i])
        if ra != rb:
            parent[ra] = rb
    comp = np.fromiter((find(i) for i in range(V)), np.int64, V)

    # pair odd-degree vertices within components with virtual edges
    deg = np.bincount(a, minlength=V) + np.bincount(b, minlength=V)
    odd = np.nonzero(deg % 2 == 1)[0]
    oorder = odd[np.argsort(comp[odd], kind="stable")]
    va, vb = oorder[0::2], oorder[1::2]
    NE = P + va.shape[0]
    ea = np.concatenate([a, va])
    eb = np.concatenate([b, vb])

    # CSR half-edge adjacency + iterative Hierholzer
    he_v = np.concatenate([ea, eb])
    hstart = np.zeros(V + 1, dtype=np.int64)
    np.cumsum(np.bincount(he_v, minlength=V), out=hstart[1:])
    hlist = np.argsort(he_v, kind="stable")
    ptr = hstart[:-1].copy()
    used = np.zeros(NE, dtype=bool)
    trails = []  # (nodes [k+1], lo_inst [k], hi_inst [k])
    for v0 in range(V):
        while True:
            while ptr[v0] < hstart[v0 + 1] and used[hlist[ptr[v0]] % NE]:
                ptr[v0] += 1
            if ptr[v0] >= hstart[v0 + 1]:
                break
            stack_v = [v0]
            stack_e = []
            circ = []  # edge ids, circuit order
            while stack_v:
                x = stack_v[-1]
                while ptr[x] < hstart[x + 1] and used[hlist[ptr[x]] % NE]:
                    ptr[x] += 1
                if ptr[x] < hstart[x + 1]:
                    e = hlist[ptr[x]] % NE
                    used[e] = True
                    stack_v.append(ea[e] + eb[e] - x)
                    stack_e.append(e)
                else:
                    stack_v.pop()
                    if stack_e:
                        circ.append(stack_e.pop())
            circ.reverse()
            # walk the circuit, splitting at virtual edges
            v = v0
            nodes, lo_i, hi_i = [v], [], []
            for e in circ:
                if e >= P:  # virtual: close current trail
                    if lo_i:
                        trails.append((nodes, lo_i, hi_i))
                    v = ea[e] + eb[e] - v
                    nodes, lo_i, hi_i = [v], [], []
                else:
                    nv_ = ea[e] + eb[e] - v
                    lo = pA[e] if a[e] == v else pB[e]
                    hi = pA[e] + pB[e] - lo
                    if a[e] == b[e]:  # self-loop: either orientation
                        lo, hi = pA[e], pB[e]
                    nodes.append(nv_)
                    lo_i.append(lo)
                    hi_i.append(hi)
                    v = nv_
            if lo_i:
                trails.append((nodes, lo_i, hi_i))

    singles = list(sing_l)  # instance ids; ride an occurrence row
    rdeg = deg.copy()  # remaining real-edge degree
    total_rows = sum(len(n) for n, _, _ in trails)
    budget = NW * WCAP - 256  # headroom for orphan rows below
    # spill: pop trail-end pairs into singles until rows fit the windows
    guard = 0
    while total_rows > budget:
        progress = False
        for tr in trails:
            if total_rows <= budget:
                break
            nodes, lo_i, hi_i = tr
            if not lo_i:
                continue
            u_end = nodes[-1]
            u_prev = nodes[-2]
            d_end = rdeg[u_end] - (2 if u_end == u_prev else 1)
            if d_end < 1:
                continue
            if len(lo_i) == 1 and rdeg[u_prev] - 1 < 1:
                continue
            singles.append(lo_i.pop())
            singles.append(hi_i.pop())
            nodes.pop()
            rdeg[u_end] -= 1
            rdeg[u_prev] -= 1
            total_rows -= 1
            progress = True
        guard += 1
        assert progress and guard < 50, "L1 spill failed"
    trails = [t for t in trails if t[1]]

    # orphan singles: src with no remaining occurrence gets a 1-row trail
    occ_ok = np.zeros(V, dtype=bool)
    occ_ok[np.nonzero(rdeg > 0)[0]] = True
    vmap = {int(v): i for i, v in enumerate(verts)}
    fixed = []
    for s in singles:
        u = int(i_src[s])
        ui = vmap.get(u, -1)
        if ui >= 0 and occ_ok[ui]:
            fixed.append((s, ui))
        else:
            if ui < 0:
                vmap[u] = ui = V
                verts = np.append(verts, u)
                occ_ok = np.append(occ_ok, False)
                V += 1
            trails.append(([ui], [], []))  # 1-row trail, ridden by single
            occ_ok[ui] = True
            total_rows += 1
            fixed.append((s, ui))
    assert total_rows <= NW * WCAP, "L1 rows over budget after orphans"

    # window assignment: greedy vector packing on tile histograms
    tilecnt = np.bincount(i_tile, minlength=TILES).astype(np.float64)
    target = np.maximum(tilecnt / (2 * NW), 1.0)
    cellcnt = np.zeros((NW, TILES), dtype=np.int64)
    rows_used = np.zeros(NW, dtype=np.int64)
    tr_rows = np.array([len(n) for n, _, _ in trails])
    tr_order = np.argsort(-tr_rows, kind="stable")
    win_of = np.empty(len(trails), dtype=np.int64)
    for ti in tr_order:
        nodes, lo_i, hi_i = trails[ti]
        h = np.bincount(i_tile[lo_i], minlength=TILES) if lo_i else 0
        best_w, best_s = -1, None
        for w in range(NW):
            if rows_used[w] + len(nodes) > WCAP:
                continue
            s = ((cellcnt[w] + h) / target).max() if lo_i else (
                rows_used[w] / WCAP
            )
            if best_w < 0 or s < best_s:
                best_w, best_s = w, s
        assert best_w >= 0, "window overflow"
        win_of[ti] = best_w
        rows_used[best_w] += len(nodes)
        if lo_i:
            cellcnt[best_w] += h
    # build per-window row sequences + per-step descriptor records
    rows_w = [[] for _ in range(NW)]
    roff = np.zeros(NW, dtype=np.int64)
    d_key, d_idx = [], []
    d_dl = [[], []]
    d_nm = [[], []]
    first_occ = {}
    for ti, (nodes, lo_i, hi_i) in enumerate(trails):
        w = win_of[ti]
        base = roff[w]
        rows_w[w].append(np.asarray(nodes, dtype=np.int64))
        for k_ in range(len(nodes)):
            u = nodes[k_]
            if u not in first_occ:
                first_occ[u] = (w, base + k_)
        for k_, (lo, hi) in enumerate(zip(lo_i, hi_i)):
            d_key.append(w * TILES + int(i_tile[lo]))
            d_idx.append(base + k_)
            d_dl[0].append(i_dstl[lo])
            d_nm[0].append(i_norm[lo])
            d_dl[1].append(i_dstl[hi])
            d_nm[1].append(i_norm[hi])
        roff[w] += len(nodes)
    for s, ui in fixed:
        w, row = first_occ[ui]
        d_key.append(w * TILES + int(i_tile[s]))
        d_idx.append(row)
        d_dl[0].append(i_dstl[s])
        d_nm[0].append(i_norm[s])
        d_dl[1].append(-1.0)
        d_nm[1].append(0.0)
    key = np.asarray(d_key, dtype=np.int64)
    cell_final = np.bincount(key, minlength=NW * TILES).reshape(NW, TILES)
    return dict(
        cellcnt=cell_final,
        key=key,
        idxv=np.asarray(d_idx, dtype=np.int64),
        dstl=[np.asarray(d_dl[0], np.float32), np.asarray(d_dl[1], np.float32)],
        norm=[np.asarray(d_nm[0], np.float32), np.asarray(d_nm[1], np.float32)],
        rows_w=[
            verts[np.concatenate(rw)] if rw else np.zeros(0, dtype=np.int64)
            for rw in rows_w
        ],
    )


def _mk_meta_h(mmx, dstl_h, norm_h):
    """Meta for 6-column mm rows (b, t, c, lo, hi, half)."""
    M = mmx.shape[0]
    md = np.full((NC, M, 128), -1.0, dtype=np.float32)
    mn = np.zeros((NC, M, 128), dtype=np.float32)
    for i in range(M):
        b, t, c, lo, hi, h = mmx[i]
        base = int(c) * 128
        md[:, i, lo - base : hi - base] = dstl_h[h][:, lo:hi]
        mn[:, i, lo - base : hi - base] = norm_h[h][:, lo:hi]
    meta = np.empty((NC, 128, 2 * M), dtype=np.float32)
    meta[:, :, 0::2] = md.transpose(0, 2, 1)
    meta[:, :, 1::2] = mn.transpose(0, 2, 1)
    return meta


def _mk_meta(mm, dstl_all, norm_all):
    M = mm.shape[0]
    md = np.full((NC, M, 128), -1.0, dtype=np.float32)
    mn = np.zeros((NC, M, 128), dtype=np.float32)
    for i in range(M):
        b, t, c, lo, hi = mm[i]
        base = int(c) * 128
        md[:, i, lo - base : hi - base] = dstl_all[:, lo:hi]
        mn[:, i, lo - base : hi - base] = norm_all[:, lo:hi]
    meta = np.empty((NC, 128, 2 * M), dtype=np.float32)
    meta[:, :, 0::2] = md.transpose(0, 2, 1)
    meta[:, :, 1::2] = mn.transpose(0, 2, 1)
    return meta


def _schedule(edge_index, n_nodes):
    """Static SPMD schedule + per-core edge arrays for both layers."""
    N = n_nodes
    NSH = (N + NC - 1) // NC
    TILES = (NSH + 127) // 128
    BLK = 25000  # L1 x-table rows per block
    NB1 = -(-N // BLK)  # L1 source-range blocks
    UBLK = 25000  # L2 units per block (unit = 2 nodes)
    NU = (N + 1) // 2
    NB2u = -(-NU // UBLK)  # L2 unit-range blocks
    assert NSH % 2 == 0

    src = np.asarray(edge_index[0], dtype=np.int64)
    dst = np.asarray(edge_index[1], dtype=np.int64)
    E = src.shape[0]
    deg = np.bincount(dst, minlength=N).astype(np.float64) + 1.0
    dinv = 1.0 / np.sqrt(deg)
    w_all = (dinv[src] * dinv[dst]).astype(np.float32)

    core = dst // NSH
    j_all = dst - core * NSH
    blk1 = src // BLK

    # per-core per-node block in-degree vectors for balance:
    # L1 blocks (src range, self excluded) ++ L2 blocks (unit range x parity,
    # self excluded -- self parity depends on pos, handled by SELF streams)
    sc = src // NSH  # src core
    ub = sc // 4  # L2 unit-range block (aligned with shard pairs)
    nv1 = np.bincount(
        (core * NSH + j_all) * NB1 + blk1, minlength=NC * NSH * NB1
    ).reshape(NC, NSH, NB1)
    B1 = nv1.sum(axis=1)

    caps = np.full(TILES, 128, dtype=np.int64)
    caps[TILES - 1] = NSH - 128 * (TILES - 1)

    # The balance sees L1 source-range blocks and L2 unit-range blocks
    # (parity-summed); L2 parity cells are then sized exactly after an
    # explicit parity assignment pass below.
    nv2u = np.bincount(
        (core * NSH + j_all) * NB2u + ub, minlength=NC * NSH * NB2u
    ).reshape(NC, NSH, NB2u)
    B2u = nv2u.sum(axis=1)  # [NC, NB2u]

    def spread(tot):
        tgt = tot * caps / NSH
        base = np.floor(tgt).astype(np.int64)
        rem = int(tot - base.sum())
        order = np.argsort(-(tgt - base))
        base[order[:rem]] += 1
        return base

    slack = 256
    for _attempt in range(8):
        TOT1 = ((B1.max(axis=0) + slack + 127) // 128) * 128  # [NB1]
        TOT2u = B2u.max(axis=0) + 2 * slack  # [NB2u] (both parities)
        captb1 = np.stack([spread(t) for t in TOT1])  # [NB1, TILES]
        captb2u = np.stack([spread(t) for t in TOT2u])  # [NB2u, TILES]
        k_tb = np.concatenate([captb1.T, captb2u.T], axis=1)
        nv = np.concatenate([nv1, nv2u], axis=2)
        pos_all = np.empty((NC, NSH), dtype=np.int64)
        ok = True
        for r in range(NC):
            pos = _balance(nv[r], caps, k_tb)
            if pos is None:
                ok = False
                break
            pos_all[r] = pos
        if ok:
            break
        slack *= 2
    assert ok, "balance failed"

    # ---- explicit parity assignment (within-tile slot reshuffle) ----
    # A src node's position parity decides which L2 parity stream its
    # out-edges ride at every consumer.  Within each producer tile the
    # even/odd slot counts are fixed; which node takes which parity is
    # free.  Greedy discrepancy minimization over consumer (core, tile)
    # cells keeps every parity cell near half its unit-range cell.
    tile_of_e = pos_all[core, j_all] >> 7  # dst tile per edge (stable)
    cell_of_e = core * TILES + tile_of_e
    order_e = np.argsort(src, kind="stable")
    e_sorted = cell_of_e[order_e]
    src_sorted = src[order_e]
    starts = np.searchsorted(src_sorted, np.arange(N + 1))
    parity = np.zeros(N, dtype=np.int64)
    for g in range(NB2u):
        D = np.zeros(NC * TILES, dtype=np.int64)
        # nodes of producer cores 4g..4g+3, grouped by (core, tile)
        for r in range(4 * g, min(4 * g + 4, NC)):
            nval = min(NSH, N - r * NSH)
            p = pos_all[r, :nval]
            t = p >> 7
            for tt in range(TILES):
                nodes = np.nonzero(t == tt)[0] + r * NSH
                if nodes.size == 0:
                    continue
                degs = starts[nodes + 1] - starts[nodes]
                nodes = nodes[np.argsort(-degs, kind="stable")]
                n_even = (nodes.size + 1) // 2
                ev_left, od_left = n_even, nodes.size - n_even
                for n in nodes:
                    cells = e_sorted[starts[n] : starts[n + 1]]
                    if ev_left == 0:
                        p_n = 1
                    elif od_left == 0:
                        p_n = 0
                    else:
                        p_n = 0 if D[cells].sum() <= 0 else 1
                    parity[n] = p_n
                    if p_n == 0:
                        ev_left -= 1
                        np.add.at(D, cells, 1)
                    else:
                        od_left -= 1
                        np.add.at(D, cells, -1)
    # reassign within-tile slots by parity class
    for r in range(NC):
        nval = min(NSH, N - r * NSH)
        p = pos_all[r, :nval]
        t = p >> 7
        par_n = parity[r * NSH : r * NSH + nval]
        newpos = np.empty(nval, dtype=np.int64)
        for tt in range(TILES):
            nodes = np.nonzero(t == tt)[0]
            if nodes.size == 0:
                continue
            ev = nodes[par_n[nodes] == 0]
            od = nodes[par_n[nodes] == 1]
            newpos[ev] = tt * 128 + 2 * np.arange(ev.size)
            newpos[od] = tt * 128 + 2 * np.arange(od.size) + 1
        pos_all[r, :nval] = newpos

    pos_e = pos_all[core, j_all]
    tile_of = pos_e >> 7
    dstl = (pos_e & 127).astype(np.float32)

    # permuted g row of every src node
    src_pos = pos_all[sc, src - sc * NSH]
    g_row = sc * NSH + src_pos
    par = (g_row & 1).astype(np.int64)

    # ---------------- layer 1: paired descriptors ----------------
    # Each 512 B descriptor serves TWO same-tile edges: the per-core x2
    # table row i holds [x[a_i] | x[a_{i+1}]] along an Eulerian trail of
    # the chosen pair multigraph, so paired srcs sit in one row.  Blocks
    # are fixed 32760-row table windows (int16 idx).
    sel_by_core = [np.nonzero(core == r)[0] for r in range(NC)]
    WCAP = 32760
    NW = 4
    cores_l1 = []
    for r in range(NC):
        sel = sel_by_core[r]
        nval = min(NSH, N - r * NSH)
        jj = np.arange(nval, dtype=np.int64)
        p_self = pos_all[r, :nval]
        i_src = np.concatenate([src[sel], r * NSH + jj])
        i_tile = np.concatenate([tile_of[sel], p_self >> 7])
        i_dstl = np.concatenate(
            [dstl[sel], (p_self & 127).astype(np.float32)]
        )
        i_norm = np.concatenate(
            [w_all[sel], (dinv[r * NSH + jj] ** 2).astype(np.float32)]
        )
        cores_l1.append(
            _l1_pair_core(i_src, i_tile, i_dstl, i_norm, TILES, NW, WCAP)
        )
    # shared cell caps: max over cores, block sums chunk-aligned
    captb1 = np.zeros((NW, TILES), dtype=np.int64)
    for cd in cores_l1:
        np.maximum(captb1, cd["cellcnt"], out=captb1)
    for w in range(NW):
        rem = int(captb1[w].sum()) % 128
        if rem:
            captb1[w, TILES - 1] += 128 - rem
    mm1, off1, Kb1, C1 = _mk_mm(captb1, TILES)
    CAP1 = C1 * 128
    capf1 = captb1.reshape(-1)
    idx1 = np.zeros((NC, CAP1), dtype=np.int16)
    dstl1h = np.full((2, NC, CAP1), -1.0, dtype=np.float32)
    norm1h = np.zeros((2, NC, CAP1), dtype=np.float32)
    for r in range(NC):
        cd = cores_l1[r]
        k = cd["key"]
        order = np.argsort(k, kind="stable")
        k = k[order]
        nkey = capf1.shape[0]
        cr = np.bincount(k, minlength=nkey)
        grp_start = np.zeros(nkey, dtype=np.int64)
        np.cumsum(cr[:-1], out=grp_start[1:])
        rank = np.arange(k.shape[0], dtype=np.int64) - np.repeat(grp_start, cr)
        assert (rank < capf1[k]).all(), "L1 cell overflow"
        slots = off1[k] + rank
        idx1[r, slots] = cd["idxv"][order].astype(np.int16)
        for h in range(2):
            dstl1h[h, r, slots] = cd["dstl"][h][order]
            norm1h[h, r, slots] = cd["norm"][h][order]
    # expand mm rows: one matmul per (desc-chunk, half)
    mm1x = np.empty((mm1.shape[0] * 2, 6), dtype=np.int64)
    mm1x[0::2, :5] = mm1
    mm1x[1::2, :5] = mm1
    mm1x[0::2, 5] = 0
    mm1x[1::2, 5] = 1
    meta1 = _mk_meta_h(mm1x, dstl1h, norm1h)

    # ---------------- layer 2 cells ----------------
    # blocks 0..2*NB2u-1 = (unit range x parity) reading g_full;
    # blocks -2,-1 = SELF even/odd (tile-aligned, cap 64/tile, read g_shard)
    # Parity cells sized exactly: max over cores + chunk-align block sums.
    NBm = 2 * NB2u
    cnt2 = np.bincount(
        (core * TILES + tile_of) * NBm + (ub * 2 + par),
        minlength=NC * TILES * NBm,
    ).reshape(NC, TILES, NBm)
    captb2m = cnt2.max(axis=0).T.copy()  # [NBm, TILES]
    for b in range(NBm):
        rem = int(captb2m[b].sum()) % 128
        if rem:
            captb2m[b, TILES - 1] += 128 - rem
    half = np.zeros((2, TILES), dtype=np.int64)
    for t in range(TILES):
        cap_t = int(caps[t])
        half[0, t] = (cap_t + 1) // 2  # even positions in tile
        half[1, t] = cap_t // 2
    # pad self blocks to chunk multiples (dead lanes at the very end)
    selfpad = np.zeros((2, TILES), dtype=np.int64)
    for q in range(2):
        rem = int(half[q].sum()) % 128
        if rem:
            selfpad[q, TILES - 1] = 128 - rem
    captb2f = np.concatenate([captb2m, half + selfpad], axis=0)
    mm2, off2, Kb2, C2 = _mk_mm(captb2f, TILES)
    CAP2 = C2 * 128
    key2 = (ub * 2 + par) * TILES + tile_of
    val2 = (g_row >> 1) - ub * UBLK
    idx2, dstl2, norm2 = _fill_slots(
        sel_by_core, key2, val2, dstl, w_all, off2, captb2f.reshape(-1), CAP2
    )
    # L2 self: node at pos p -> self block (parity of p), slot = rank of p
    # among same-parity positions of its tile (tile-major order)
    for r in range(NC):
        nval = min(NSH, N - r * NSH)
        jj = np.arange(nval, dtype=np.int64)
        p = pos_all[r, :nval]
        t = p >> 7
        q = p & 1
        # slot within (self block q): offset of tile + rank among parity
        # positions of the tile, ordered by p
        order = np.argsort(p, kind="stable")
        slots = np.empty(nval, dtype=np.int64)
        for qq in range(2):
            selq = order[q[order] == qq]
            tq = t[selq]
            # rank within tile
            start = np.zeros(TILES, dtype=np.int64)
            cnts = np.bincount(tq, minlength=TILES)
            np.cumsum(cnts[:-1], out=start[1:])
            rank = np.arange(selq.shape[0], dtype=np.int64) - np.repeat(
                start, cnts
            )
            cellbase = off2[(NBm + qq) * TILES + tq]
            slots[selq] = cellbase + rank
        idx2[r, slots] = (p >> 1).astype(np.int16)
        dstl2[r, slots] = (p & 127).astype(np.float32)
        norm2[r, slots] = (dinv[r * NSH + jj] ** 2).astype(np.float32)
    meta2 = _mk_meta(mm2, dstl2, norm2)
    mm2x = np.concatenate(
        [mm2, np.zeros((mm2.shape[0], 1), dtype=np.int64)], axis=1
    )

    l1_rows = [cores_l1[r]["rows_w"] for r in range(NC)]
    return dict(
        N=N, E=E, NSH=NSH, TILES=TILES, NB2u=NB2u, UBLK=UBLK,
        NW=NW, WCAP=WCAP, l1_rows=l1_rows,
        C1=C1, M1=mm1x.shape[0], mm1=mm1x, Kb1=Kb1,
        C2=C2, M2=mm2x.shape[0], mm2=mm2x, Kb2=Kb2,
        idx1_sb=_wrap_idx(idx1), idx2_sb=_wrap_idx(idx2),
        meta1=meta1, meta2=meta2,
        pos_all=pos_all,
    )


def _build_bass(sp, for_timing=False):
    import concourse.bass as bass
    import concourse.bacc as bacc
    import concourse.mybir as mybir
    import concourse.tile as tile

    f32 = mybir.dt.float32
    bf16 = mybir.dt.bfloat16
    i16 = mybir.dt.int16
    N, NSH, TILES = sp["N"], sp["NSH"], sp["TILES"]
    NW, WCAP = sp["NW"], sp["WCAP"]
    UBLK, NB2u = sp["UBLK"], sp["NB2u"]
    NU = (N + 1) // 2
    NPAD = TILES * 128

    def mk_calls(Kb, cc):
        calls = []
        chunk0 = 0
        for kb in (int(k) for k in Kb):
            lst = []
            s0, left = chunk0, kb
            while left > 0:
                cs = min(cc, left)
                lst.append((s0, cs))
                s0 += cs
                left -= cs
            calls.append(lst)
            chunk0 += kb
        return calls

    CC1, CC2 = 14, 18
    IDXB = 4  # gather calls per idx-tile load (>=512 B lines, 2x fewer DMAs)
    calls1 = mk_calls(sp["Kb1"], CC1)
    calls2 = mk_calls(sp["Kb2"], CC2)
    MW = 128  # meta window (matmuls per meta tile)

    nc = bacc.Bacc("TRN2", target_bir_lowering=False, debug=False, num_devices=NC)
    x2_in = nc.dram_tensor("x2", [NW * WCAP + 2, 256], bf16, kind="ExternalInput")
    idx1_in = nc.dram_tensor("idx1", [128, sp["C1"] * 8], i16, kind="ExternalInput")
    idx2_in = nc.dram_tensor("idx2", [128, sp["C2"] * 8], i16, kind="ExternalInput")
    meta1_in = nc.dram_tensor("meta1", [128, 2 * sp["M1"]], f32, kind="ExternalInput")
    meta2_in = nc.dram_tensor("meta2", [128, 2 * sp["M2"]], f32, kind="ExternalInput")
    wb_in = nc.dram_tensor("wb", [128, WBW], f32, kind="ExternalInput")
    outT = nc.dram_tensor("outT", [COUT, NPAD], f32, kind="ExternalOutput")

    with tile.TileContext(nc) as tc:
        with (
            tc.tile_pool(name="const", bufs=1) as constp,
            tc.tile_pool(name="stream", bufs=3) as streamp,
            tc.tile_pool(name="msgs1", bufs=3) as msgs1p,
            tc.tile_pool(name="msgs2", bufs=2) as msgs2p,
            tc.tile_pool(name="oh", bufs=3) as ohp,
            tc.tile_pool(name="work", bufs=8) as workp,
            tc.tile_pool(name="pacc", bufs=PSUM_ACC_BUFS, space="PSUM") as paccp,
            tc.tile_pool(name="pproj", bufs=2, space="PSUM") as pprojp,
            tc.tile_pool(name="dram", bufs=1, space="DRAM") as dramp,
        ):
            wb = constp.tile([128, WBW], f32)
            nc.sync.dma_start(wb[:], wb_in[:])
            W1 = wb[:, 0:128]
            W2 = wb[:, 128 : 128 + COUT]
            b1 = wb[:, 384:385]
            b2 = wb[:64, 385:386]
            iota_bf = constp.tile([128, 128], bf16)
            nc.vector.tensor_copy(iota_bf[:], wb[:, 256:384])

            g_shard = dramp.tile([NSH + 2, COUT], bf16)
            g_full = dramp.tile(
                [NU + 2, 2 * COUT], bf16,
                addr_space="Local" if for_timing else "Shared",
            )
            gsu = g_shard[:].flatten()
            gfu = g_full[:].flatten()

            def gf_view(off, rows):
                return gfu[off : off + rows * 128].rearrange(
                    "(r c) -> r c", c=128
                )

            def gs_view(off, rows):
                return gsu[off : off + rows * 128].rearrange(
                    "(r c) -> r c", c=128
                )

            # layer-2 stream tables: (unit-range x parity) + self even/odd
            l2_tables = []
            for u0 in range(NB2u):
                for q in range(2):
                    rows = min(UBLK, NU - u0 * UBLK)
                    l2_tables.append(gf_view(u0 * UBLK * 128 + q * 64, rows))
            l2_tables.append(gs_view(0, NSH // 2))
            l2_tables.append(gs_view(64, NSH // 2))

            l1_tables = [
                x2_in[w * WCAP : w * WCAP + WCAP + 1, :] for w in range(NW)
            ]

            def aggregate(tables, mm, calls, meta_in, idx_dram, nrow, elem,
                          cc, msgsp, epilogue):
                """Tile-major consumption: each tile's cells (all blocks)
                accumulate in one PSUM bank; block-major gather streams
                feed the matmuls; `epilogue(t, pt)` consumes the full tile."""
                NBt = len(tables)
                s_ci = [0] * NBt
                s_cur = [(-1, 0)] * NBt
                s_mg = [None] * NBt
                s_it = [(None, -1, 0)] * NBt  # (idx tile, chunk_lo, nchunks)
                mt = None
                cur_w = -1
                pt = None
                cur_t = -1
                oht = None  # grouped one-hot tile: 8 slots per pool alloc
                M = mm.shape[0]
                for i in range(M):
                    b, t, c, lo, hi, half = (int(v) for v in mm[i])
                    cur0, curk = s_cur[b]
                    if s_mg[b] is None or c >= cur0 + curk:
                        ci = s_ci[b]
                        cur0, curk = calls[b][ci]
                        s_ci[b] += 1
                        s_cur[b] = (cur0, curk)
                        assert cur0 <= c < cur0 + curk
                        idxt, i0, ik = s_it[b]
                        if idxt is None or cur0 + curk > i0 + ik:
                            # load idx rows for the next IDXB calls at once
                            i0 = cur0
                            ik = sum(
                                k for _, k in calls[b][ci : ci + IDXB]
                            )
                            idxt = streamp.tile(
                                [128, IDXB * cc * 8], i16, tag=f"idx{elem}_{b}"
                            )
                            nc.sync.dma_start(
                                idxt[:, : ik * 8],
                                idx_dram[:, i0 * 8 : (i0 + ik) * 8],
                            )
                            s_it[b] = (idxt, i0, ik)
                        mg = msgsp.tile(
                            [128, cc, elem], bf16, tag=f"msgs{elem}_{b}"
                        )
                        s_mg[b] = mg
                        o8 = (cur0 - i0) * 8
                        nc.gpsimd.dma_gather(
                            mg[:, :curk, :],
                            tables[b],
                            idxt[:, o8 : o8 + curk * 8],
                            num_idxs=curk * 128,
                            num_idxs_reg=curk * 128,
                            elem_size=elem,
                            single_packet=False,
                        )
                    if i // MW != cur_w:
                        cur_w = i // MW
                        nmw = min(MW, M - cur_w * MW)
                        mt = streamp.tile([128, 2 * MW], f32, tag="meta")
                        nc.sync.dma_start(
                            mt[:, : 2 * nmw],
                            meta_in[:, 2 * cur_w * MW : 2 * (cur_w * MW + nmw)],
                        )
                    if t != cur_t:
                        if pt is not None:
                            epilogue(cur_t, pt)
                        pt = paccp.tile([128, 128], f32, tag="pacc")
                        cur_t = t
                        first = True
                    else:
                        first = False
                    last = (i == M - 1) or (int(mm[i + 1][1]) != t)
                    cl = c - cur0
                    mloc = i - cur_w * MW
                    osl = i % 8
                    if osl == 0 or oht is None:
                        oht = ohp.tile([128, 8 * 128], bf16, tag="oh")
                    oh = oht[:, osl * 128 : (osl + 1) * 128]
                    nc.vector.tensor_scalar(
                        oh,
                        iota_bf[:],
                        mt[:, 2 * mloc : 2 * mloc + 1],
                        mt[:, 2 * mloc + 1 : 2 * mloc + 2],
                        mybir.AluOpType.is_equal,
                        mybir.AluOpType.mult,
                    )
                    co = half * 128
                    nc.tensor.matmul(
                        pt[:nrow, :],
                        s_mg[b][:, cl, co : co + nrow],
                        oh,
                        start=first,
                        stop=last,
                    )
                epilogue(cur_t, pt)

            # ---------------- layer 1 ----------------
            def epilogue1(t, pt):
                r0 = t * 128
                rows = min(128, NSH - r0)
                aggT = workp.tile([128, 128], f32, tag="aggT")
                nc.scalar.activation(
                    aggT[:], pt[:], mybir.ActivationFunctionType.Copy
                )
                hp = pprojp.tile([128, 128], f32, tag="proj")
                nc.tensor.matmul(hp[:], W1, aggT[:], start=True, stop=True)
                hs = workp.tile([128, 128], f32, tag="hs")
                nc.scalar.activation(
                    hs[:], hp[:], mybir.ActivationFunctionType.Relu,
                    bias=b1, scale=1.0,
                )
                gp = pprojp.tile([128, 128], f32, tag="proj")
                nc.tensor.matmul(gp[:, :COUT], hs[:], W2, start=True, stop=True)
                gs = workp.tile([128, COUT], bf16, tag="gs")
                nc.scalar.activation(
                    gs[:], gp[:, :COUT], mybir.ActivationFunctionType.Copy
                )
                nc.sync.dma_start(g_shard[r0 : r0 + rows, :], gs[:rows, :])

            aggregate(l1_tables, sp["mm1"], calls1, meta1_in, idx1_in, 128,
                      256, CC1, msgs1p, epilogue1)

            # ---------------- exchange ----------------
            if for_timing:
                nc.gpsimd.dma_start(
                    gf_view(0, NSH // 2), gs_view(0, NSH // 2)
                )
            else:
                nc.gpsimd.collective_compute(
                    "AllGather",
                    mybir.AluOpType.bypass,
                    replica_groups=[list(range(NC))],
                    ins=[g_shard[0:NSH, :]],
                    outs=[g_full[:].flatten()[0 : N * COUT].rearrange(
                        "(r c) -> r c", c=128
                    )],
                )

            # ---------------- layer 2 ----------------
            def epilogue2(t, pt):
                r0 = t * 128
                cols = min(128, NSH - r0)
                ob = workp.tile([64, 128], f32, tag="ob")
                nc.scalar.activation(
                    ob[:],
                    pt[:COUT, :],
                    mybir.ActivationFunctionType.Identity,
                    bias=b2,
                    scale=1.0,
                )
                nc.sync.dma_start(outT[:, r0 : r0 + cols], ob[:, :cols])

            aggregate(l2_tables, sp["mm2"], calls2, meta2_in, idx2_in, COUT,
                      128, CC2, msgs2p, epilogue2)

    nc.compile()
    return nc


_CACHE = {}


def _get_program(sp):
    key = (sp["N"], sp["C1"], sp["C2"], sp["mm1"].tobytes(), sp["mm2"].tobytes())
    if key not in _CACHE:
        _CACHE[key] = _build_bass(sp)
    return _CACHE[key]


def _make_wb(W1, b1, W2, b2):
    wb = np.zeros((128, WBW), dtype=np.float32)
    wb[:, 0:128] = np.asarray(W1, dtype=np.float32)
    wb[:, 128 : 128 + COUT] = np.asarray(W2, dtype=np.float32)
    wb[:, 256:384] = np.arange(128, dtype=np.float32)[None, :]
    wb[:, 384] = np.asarray(b1, dtype=np.float32)
    wb[:64, 385] = np.asarray(b2, dtype=np.float32)
    return wb


def make_in_maps(sp, x, W1, b1, W2, b2):
    xb = np.ascontiguousarray(np.asarray(x, dtype=np.float32).astype(BF16))
    wb = _make_wb(W1, b1, W2, b2)
    NW, WCAP = sp["NW"], sp["WCAP"]
    maps = []
    for r in range(NC):
        x2 = np.zeros((NW * WCAP + 2, 256), dtype=BF16)
        for w, nodes in enumerate(sp["l1_rows"][r]):
            R = nodes.shape[0]
            base = w * WCAP
            x2[base : base + R, 0:128] = xb[nodes]
            if R > 1:
                x2[base : base + R - 1, 128:256] = xb[nodes[1:]]
        maps.append(
            {
                "x2": x2,
                "idx1": sp["idx1_sb"][r],
                "idx2": sp["idx2_sb"][r],
                "meta1": sp["meta1"][r],
                "meta2": sp["meta2"][r],
                "wb": wb,
            }
        )
    return maps


def kernel(x, edge_index, W1, b1, W2, b2, _trace=False):
    from concourse.bass_utils import run_bass_kernel_spmd

    x = np.asarray(x, dtype=np.float32)
    N = x.shape[0]
    sp = _schedule(np.asarray(edge_index), N)
    nc = _get_program(sp)
    in_maps = make_in_maps(sp, x, W1, b1, W2, b2)
    res = run_bass_kernel_spmd(nc, in_maps, list(range(NC)), trace=_trace)

    NSH = sp["NSH"]
    out = np.empty((N, COUT), dtype=np.float32)
    for r in range(NC):
        lo = r * NSH
        hi = min(N, lo + NSH)
        out[lo:hi] = res.results[r]["outT"][:, sp["pos_all"][r, : hi - lo]].T
    if _trace:
        kernel.last_result = res
    return out


# revision 75
# speedup vs baseline: 1.6815x; 1.1962x over previous
"""2-layer GCN on 8 Trainium2 NeuronCores (Bass/Tile).

Math: gcn_conv(x, W, b) = D^-1/2 (A+I) D^-1/2 (x W) + b.  Propagation
commutes with the weight matmul, so layer 1 aggregates raw x
(h = relu((A_hat x) W1 + b1)) and layer 2 projects first
(out = A_hat (h W2) + b2), minimizing per-edge gather bytes.

Distribution: destination nodes sharded 8 ways (12500/core); each core
gathers source features for its own edges from a full local feature table
(x is an input; g = h W2 is assembled with one 8-rank AllGather).

g is stored at its true width (64 cols bf16 = 128 B rows) so the
AllGather moves half the bytes of a 128-padded layout.  dma_gather
requires 256 B-aligned elements, so layer-2 gathers fetch a 256 B *unit*
(two consecutive g rows) per edge: streams come in (unit-range, parity)
flavors whose table APs are offset by 64 elements so the wanted row
always lands in msgs cols 0:64.

Per-core aggregation, per layer:
- block-major dma_gather streams (int16 idx limit => <=25000-row/unit
  tables). L1: 4 source-range blocks + a SELF stream reading x_own.
  L2: 4 (unit-range x parity) blocks reading g_full + 2 parity SELF
  streams reading g_shard.
- Edges are laid out in fractional per-(tile, block) cells balanced by a
  per-core node permutation (greedy vector bin packing), ~2% padding;
  boundary chunks are consumed by two matmuls with foreign lanes zeroed.
- Consumption is tile-major: bf16 one-hot scatter matrices
  (tensor_scalar is_equal*norm) feed PE matmuls msgs^T @ onehot
  (bf16 -> fp32 PSUM); each destination tile accumulates all its cells
  in one PSUM bank, then a single ACT-engine escape feeds the inline
  projection (W1 -> relu -> W2 -> bf16 g row) or the bias+output write.
"""
import sys

sys.path.insert(0, "/opt/trn_rl_repo")
import numpy as np
import ml_dtypes

BF16 = ml_dtypes.bfloat16
NC = 8
CIN, CH, COUT = 128, 128, 64
CALL_CHUNKS = 36  # gather-call granularity (chunks of 128 edges)
PSUM_ACC_BUFS = 6  # concurrent per-tile accumulation banks
WBW = 386  # wb cols: W1[0:128] W2[128:192] iota[256:384] b1[384] b2[385]


def _balance(nv, caps, k_tb):
    """Best-fit-decreasing node->tile assignment for one core.

    nv: [NSH, NB] per-node block in-degree vectors (both layers' blocks).
    caps: [TILES] node slots per tile.  k_tb: [TILES, NB] cell targets.
    Returns pos[NSH] (node -> global slot = tile*128 + slot_in_tile) or None
    if infeasible.
    """
    NSH, NB = nv.shape
    TILES = caps.shape[0]
    capv = k_tb.astype(np.float64)
    np.maximum(capv, 1e-9, out=capv)
    usedv = np.zeros((TILES, NB), dtype=np.float64)
    usect = np.zeros(TILES, dtype=np.int64)
    pos = np.empty(NSH, dtype=np.int64)
    order = np.argsort(-nv.sum(1), kind="stable")
    for n in order:
        v = nv[n].astype(np.float64)
        post = (usedv + v[None, :]) / capv
        feas = (post <= 1.0).all(1) & (usect < caps)
        if not feas.any():
            return None
        score = np.maximum(post.max(1), (usect + 1) / caps)
        score[~feas] = np.inf
        t = int(np.argmin(score))
        usedv[t] += v
        pos[n] = t * 128 + usect[t]
        usect[t] += 1
    return pos


def _mk_mm(captb, TILES):
    """Tile-major matmul table for a cell-capacity matrix [NB, TILES].

    Returns (mm [(b,t,c,lo,hi)], off_flat, Kb, C).  Chunk numbering is
    global across blocks (block b's chunks follow block b-1's).
    """
    NB = captb.shape[0]
    nkey = NB * TILES
    off_flat = np.zeros(nkey + 1, dtype=np.int64)
    np.cumsum(captb.reshape(-1), out=off_flat[1:])
    CAP = int(off_flat[-1])
    assert CAP % 128 == 0
    # block starts must be chunk-aligned so gather streams stay block-major
    blk_cap = captb.sum(axis=1)
    assert (blk_cap % 128 == 0).all(), blk_cap
    Kb = blk_cap // 128
    C = int(Kb.sum())
    mm = []
    for t in range(TILES):
        for b in range(NB):
            o = int(off_flat[b * TILES + t])
            cap = int(captb[b, t])
            if cap == 0:
                continue
            c0, c1 = o >> 7, (o + cap - 1) >> 7
            for c in range(c0, c1 + 1):
                lo = max(o, c * 128)
                hi = min(o + cap, (c + 1) * 128)
                mm.append((b, t, c, lo, hi))
    return np.array(mm, dtype=np.int64), off_flat, Kb, C


def _fill_slots(sel_by_core, key_all, val_all, dstl_all, norm_all, off_flat,
                capf, CAP):
    """Scatter per-edge records into their cell slots for all cores.

    Returns (idx [NC, CAP] int16, dstl [NC, CAP] f32, norm [NC, CAP] f32).
    """
    idx = np.zeros((NC, CAP), dtype=np.int16)
    dst = np.full((NC, CAP), -1.0, dtype=np.float32)
    nrm = np.zeros((NC, CAP), dtype=np.float32)
    for r in range(NC):
        sel = sel_by_core[r]
        k = key_all[sel]
        order = np.argsort(k, kind="stable")
        sel = sel[order]
        k = k[order]
        nkey = capf.shape[0]
        cr = np.bincount(k, minlength=nkey)
        grp_start = np.zeros(nkey, dtype=np.int64)
        np.cumsum(cr[:-1], out=grp_start[1:])
        rank = np.arange(sel.shape[0], dtype=np.int64) - np.repeat(grp_start, cr)
        assert (rank < capf[k]).all(), "cell overflow"
        slots = off_flat[k] + rank
        idx[r, slots] = val_all[sel].astype(np.int16)
        dst[r, slots] = dstl_all[sel]
        nrm[r, slots] = norm_all[sel]
    return idx, dst, nrm


def _wrap_idx(a):
    NCc, CAP = a.shape
    return np.ascontiguousarray(
        np.tile(a.reshape(NCc, CAP // 16, 16).transpose(0, 2, 1), (1, 8, 1))
    )


def _l1_pair_core(i_src, i_tile, i_dstl, i_norm, TILES, NW, WCAP):
    """Pair one core's L1 edge instances and lay out its x2 trail table.

    Any two same-tile instances may pair (one 512 B descriptor).  Pairs
    form a multigraph over src nodes; Eulerian trails lay it out so each
    pair occupies one table row [x[u] | x[v]].  Returns per-descriptor
    records plus per-window row node sequences.
    """
    order = np.lexsort((i_src, i_tile))
    ts = i_tile[order]
    cnt = np.bincount(ts, minlength=TILES)
    starts = np.zeros(TILES + 1, dtype=np.int64)
    np.cumsum(cnt, out=starts[1:])
    pA_l, pB_l, sing_l = [], [], []
    for t in range(TILES):
        s, e = int(starts[t]), int(starts[t + 1])
        k = (e - s) // 2
        seg = order[s:e]
        pA_l.append(seg[0 : 2 * k : 2])
        pB_l.append(seg[1 : 2 * k : 2])
        if (e - s) % 2:
            sing_l.append(int(seg[-1]))
    pA = np.concatenate(pA_l)
    pB = np.concatenate(pB_l)
    P = pA.shape[0]

    verts, inv = np.unique(
        np.concatenate([i_src[pA], i_src[pB]]), return_inverse=True
    )
    a = inv[:P].astype(np.int64)
    b = inv[P:].astype(np.int64)
    V = verts.shape[0]

    # union-find for components
    parent = np.arange(V, dtype=np.int64)

    def find(x):
        root = x
        while parent[root] != root:
            root = parent[root]
        while parent[x] != root:
            parent[x], x = root, parent[x]
        return root

    for i in range(P):
        ra, rb = find(a[i]), find(b[i])
        if ra != rb:
            parent[ra] = rb
    comp = np.fromiter((find(i) for i in range(V)), np.int64, V)

    # pair odd-degree vertices within components with virtual edges
    deg = np.bincount(a, minlength=V) + np.bincount(b, minlength=V)
    odd = np.nonzero(deg % 2 == 1)[0]
    oorder = odd[np.argsort(comp[odd], kind="stable")]
    va, vb = oorder[0::2], oorder[1::2]
    NE = P + va.shape[0]
    ea = np.concatenate([a, va])
    eb = np.concatenate([b, vb])

    # CSR half-edge adjacency + iterative Hierholzer
    he_v = np.concatenate([ea, eb])
    hstart = np.zeros(V + 1, dtype=np.int64)
    np.cumsum(np.bincount(he_v, minlength=V), out=hstart[1:])
    hlist = np.argsort(he_v, kind="stable")
    ptr = hstart[:-1].copy()
    used = np.zeros(NE, dtype=bool)
    trails = []  # (nodes [k+1], lo_inst [k], hi_inst [k])
    for v0 in range(V):
        while True:
            while ptr[v0] < hstart[v0 + 1] and used[hlist[ptr[v0]] % NE]:
                ptr[v0] += 1
            if ptr[v0] >= hstart[v0 + 1]:
                break
            stack_v = [v0]
            stack_e = []
            circ = []  # edge ids, circuit order
            while stack_v:
                x = stack_v[-1]
                while ptr[x] < hstart[x + 1] and used[hlist[ptr[x]] % NE]:
                    ptr[x] += 1
                if ptr[x] < hstart[x + 1]:
                    e = hlist[ptr[x]] % NE
                    used[e] = True
                    stack_v.append(ea[e] + eb[e] - x)
                    stack_e.append(e)
                else:
                    stack_v.pop()
                    if stack_e:
                        circ.append(stack_e.pop())
            circ.reverse()
            # walk the circuit, splitting at virtual edges
            v = v0
            nodes, lo_i, hi_i = [v], [], []
            for e in circ:
                if e >= P:  # virtual: close current trail
                    if lo_i:
                        trails.append((nodes, lo_i, hi_i))
                    v = ea[e] + eb[e] - v
                    nodes, lo_i, hi_i = [v], [], []
                else:
                    nv_ = ea[e] + eb[e] - v
                    lo = pA[e] if a[e] == v else pB[e]
                    hi = pA[e] + pB[e] - lo
                    if a[e] == b[e]:  # self-loop: either orientation
                        lo, hi = pA[e], pB[e]
                    nodes.append(nv_)
                    lo_i.append(lo)
                    hi_i.append(hi)
                    v = nv_
            if lo_i:
                trails.append((nodes, lo_i, hi_i))

    singles = list(sing_l)  # instance ids; ride an occurrence row
    rdeg = deg.copy()  # remaining real-edge degree
    total_rows = sum(len(n) for n, _, _ in trails)
    budget = NW * WCAP - 256  # headroom for orphan rows below
    # spill: pop trail-end pairs into singles until rows fit the windows
    guard = 0
    while total_rows > budget:
        progress = False
        for tr in trails:
            if total_rows <= budget:
                break
            nodes, lo_i, hi_i = tr
            if not lo_i:
                continue
            u_end = nodes[-1]
            u_prev = nodes[-2]
            d_end = rdeg[u_end] - (2 if u_end == u_prev else 1)
            if d_end < 1:
                continue
            if len(lo_i) == 1 and rdeg[u_prev] - 1 < 1:
                continue
            singles.append(lo_i.pop())
            singles.append(hi_i.pop())
            nodes.pop()
            rdeg[u_end] -= 1
            rdeg[u_prev] -= 1
            total_rows -= 1
            progress = True
        guard += 1
        assert progress and guard < 50, "L1 spill failed"
    trails = [t for t in trails if t[1]]

    # orphan singles: src with no remaining occurrence gets a 1-row trail
    occ_ok = np.zeros(V, dtype=bool)
    occ_ok[np.nonzero(rdeg > 0)[0]] = True
    vmap = {int(v): i for i, v in enumerate(verts)}
    fixed = []
    for s in singles:
        u = int(i_src[s])
        ui = vmap.get(u, -1)
        if ui >= 0 and occ_ok[ui]:
            fixed.append((s, ui))
        else:
            if ui < 0:
                vmap[u] = ui = V
                verts = np.append(verts, u)
                occ_ok = np.append(occ_ok, False)
                V += 1
            trails.append(([ui], [], []))  # 1-row trail, ridden by single
            occ_ok[ui] = True
            total_rows += 1
            fixed.append((s, ui))
    assert total_rows <= NW * WCAP, "L1 rows over budget after orphans"

    # window assignment: greedy vector packing on tile histograms
    tilecnt = np.bincount(i_tile, minlength=TILES).astype(np.float64)
    target = np.maximum(tilecnt / (2 * NW), 1.0)
    cellcnt = np.zeros((NW, TILES), dtype=np.int64)
    rows_used = np.zeros(NW, dtype=np.int64)
    tr_rows = np.array([len(n) for n, _, _ in trails])
    tr_order = np.argsort(-tr_rows, kind="stable")
    win_of = np.empty(len(trails), dtype=np.int64)
    for ti in tr_order:
        nodes, lo_i, hi_i = trails[ti]
        h = np.bincount(i_tile[lo_i], minlength=TILES) if lo_i else 0
        best_w, best_s = -1, None
        for w in range(NW):
            if rows_used[w] + len(nodes) > WCAP:
                continue
            s = ((cellcnt[w] + h) / target).max() if lo_i else (
                rows_used[w] / WCAP
            )
            if best_w < 0 or s < best_s:
                best_w, best_s = w, s
        assert best_w >= 0, "window overflow"
        win_of[ti] = best_w
        rows_used[best_w] += len(nodes)
        if lo_i:
            cellcnt[best_w] += h
    # build per-window row sequences + per-step descriptor records
    rows_w = [[] for _ in range(NW)]
    roff = np.zeros(NW, dtype=np.int64)
    d_key, d_idx = [], []
    d_dl = [[], []]
    d_nm = [[], []]
    first_occ = {}
    for ti, (nodes, lo_i, hi_i) in enumerate(trails):
        w = win_of[ti]
        base = roff[w]
        rows_w[w].append(np.asarray(nodes, dtype=np.int64))
        for k_ in range(len(nodes)):
            u = nodes[k_]
            if u not in first_occ:
                first_occ[u] = (w, base + k_)
        for k_, (lo, hi) in enumerate(zip(lo_i, hi_i)):
            d_key.append(w * TILES + int(i_tile[lo]))
            d_idx.append(base + k_)
            d_dl[0].append(i_dstl[lo])
            d_nm[0].append(i_norm[lo])
            d_dl[1].append(i_dstl[hi])
            d_nm[1].append(i_norm[hi])
        roff[w] += len(nodes)
    for s, ui in fixed:
        w, row = first_occ[ui]
        d_key.append(w * TILES + int(i_tile[s]))
        d_idx.append(row)
        d_dl[0].append(i_dstl[s])
        d_nm[0].append(i_norm[s])
        d_dl[1].append(-1.0)
        d_nm[1].append(0.0)
    key = np.asarray(d_key, dtype=np.int64)
    cell_final = np.bincount(key, minlength=NW * TILES).reshape(NW, TILES)
    return dict(
        cellcnt=cell_final,
        key=key,
        idxv=np.asarray(d_idx, dtype=np.int64),
        dstl=[np.asarray(d_dl[0], np.float32), np.asarray(d_dl[1], np.float32)],
        norm=[np.asarray(d_nm[0], np.float32), np.asarray(d_nm[1], np.float32)],
        rows_w=[
            verts[np.concatenate(rw)] if rw else np.zeros(0, dtype=np.int64)
            for rw in rows_w
        ],
    )


def _mk_meta_h(mmx, dstl_h, norm_h):
    """Meta for 6-column mm rows (b, t, c, lo, hi, half)."""
    M = mmx.shape[0]
    md = np.full((NC, M, 128), -1.0, dtype=np.float32)
    mn = np.zeros((NC, M, 128), dtype=np.float32)
    for i in range(M):
        b, t, c, lo, hi, h = mmx[i]
        base = int(c) * 128
        md[:, i, lo - base : hi - base] = dstl_h[h][:, lo:hi]
        mn[:, i, lo - base : hi - base] = norm_h[h][:, lo:hi]
    meta = np.empty((NC, 128, 2 * M), dtype=np.float32)
    meta[:, :, 0::2] = md.transpose(0, 2, 1)
    meta[:, :, 1::2] = mn.transpose(0, 2, 1)
    return meta


def _mk_meta(mm, dstl_all, norm_all):
    M = mm.shape[0]
    md = np.full((NC, M, 128), -1.0, dtype=np.float32)
    mn = np.zeros((NC, M, 128), dtype=np.float32)
    for i in range(M):
        b, t, c, lo, hi = mm[i]
        base = int(c) * 128
        md[:, i, lo - base : hi - base] = dstl_all[:, lo:hi]
        mn[:, i, lo - base : hi - base] = norm_all[:, lo:hi]
    meta = np.empty((NC, 128, 2 * M), dtype=np.float32)
    meta[:, :, 0::2] = md.transpose(0, 2, 1)
    meta[:, :, 1::2] = mn.transpose(0, 2, 1)
    return meta


def _schedule(edge_index, n_nodes):
    """Static SPMD schedule + per-core edge arrays for both layers."""
    N = n_nodes
    NSH = (N + NC - 1) // NC
    TILES = (NSH + 127) // 128
    BLK = 25000  # L1 x-table rows per block
    NB1 = -(-N // BLK)  # L1 source-range blocks
    UBLK = 25000  # L2 units per block (unit = 2 nodes)
    NU = (N + 1) // 2
    NB2u = -(-NU // UBLK)  # L2 unit-range blocks
    assert NSH % 2 == 0

    src = np.asarray(edge_index[0], dtype=np.int64)
    dst = np.asarray(edge_index[1], dtype=np.int64)
    E = src.shape[0]
    deg = np.bincount(dst, minlength=N).astype(np.float64) + 1.0
    dinv = 1.0 / np.sqrt(deg)
    w_all = (dinv[src] * dinv[dst]).astype(np.float32)

    core = dst // NSH
    j_all = dst - core * NSH
    blk1 = src // BLK

    # per-core per-node block in-degree vectors for balance:
    # L1 blocks (src range, self excluded) ++ L2 blocks (unit range x parity,
    # self excluded -- self parity depends on pos, handled by SELF streams)
    sc = src // NSH  # src core
    ub = sc // 4  # L2 unit-range block (aligned with shard pairs)
    nv1 = np.bincount(
        (core * NSH + j_all) * NB1 + blk1, minlength=NC * NSH * NB1
    ).reshape(NC, NSH, NB1)
    B1 = nv1.sum(axis=1)

    caps = np.full(TILES, 128, dtype=np.int64)
    caps[TILES - 1] = NSH - 128 * (TILES - 1)

    # exchange piece geometry (needed to label L2 blocks): tile groups
    PIECE_T = [0, 12, 24, 36, 48, 60, 72, 84, TILES]
    NP_ = len(PIECE_T) - 1
    pieces = []
    for p in range(NP_):
        t0, t1 = PIECE_T[p], PIECE_T[p + 1]
        r0 = int(caps[:t0].sum())
        r1 = int(caps[:t1].sum())
        assert (r1 - r0) % 2 == 0 and r0 % 2 == 0
        pieces.append([t0, t1, r0, r1 - r0, 0])
    prows = np.array([pc[3] for pc in pieces], dtype=np.int64)
    pstart = np.array([pc[2] for pc in pieces], dtype=np.int64)
    pbase = np.zeros(NP_, dtype=np.int64)
    pbase[1:] = np.cumsum(prows * NC)[:-1]
    for p in range(NP_):
        pieces[p][4] = int(pbase[p])
    piece_of_tile = np.zeros(TILES, dtype=np.int64)
    for p in range(NP_):
        piece_of_tile[PIECE_T[p] : PIECE_T[p + 1]] = p

    # The balance sees L1 source-range blocks; pass 2 adds L2 unit-range
    # columns labeled from pass-1 positions (the label of an edge depends
    # on where OTHER cores placed its src, so it needs a first pass).
    def spread(tot):
        tgt = tot * caps / NSH
        base = np.floor(tgt).astype(np.int64)
        rem = int(tot - base.sum())
        order = np.argsort(-(tgt - base))
        base[order[:rem]] += 1
        return base

    nv2u = None
    for pass_ in range(2):
        slack = 256
        for _attempt in range(8):
            TOT1 = ((B1.max(axis=0) + slack + 127) // 128) * 128  # [NB1]
            captb1 = np.stack([spread(t) for t in TOT1])  # [NB1, TILES]
            if nv2u is None:
                k_tb = captb1.T
                nv = nv1
            else:
                TOT2u = nv2u.sum(axis=1).max(axis=0) + 2 * slack
                captb2u = np.stack([spread(t) for t in TOT2u])
                k_tb = np.concatenate([captb1.T, captb2u.T], axis=1)
                nv = np.concatenate([nv1, nv2u], axis=2)
            pos_try = np.empty((NC, NSH), dtype=np.int64)
            ok = True
            for r in range(NC):
                pos = _balance(nv[r], caps, k_tb)
                if pos is None:
                    ok = False
                    break
                pos_try[r] = pos
            if ok:
                pos_all = pos_try
                break
            slack *= 2
        if not ok:
            assert pass_ == 1, "balance failed"
            break  # keep pass-1 positions
        if pass_ == 0:
            sp0 = pos_all[sc, src - sc * NSH]
            pp0 = piece_of_tile[sp0 >> 7]
            g0 = pbase[pp0] + sc * prows[pp0] + (sp0 - pstart[pp0])
            ub0 = (g0 >> 1) // UBLK
            nv2u = np.bincount(
                (core * NSH + j_all) * NB2u + ub0,
                minlength=NC * NSH * NB2u,
            ).reshape(NC, NSH, NB2u)

    # ---- explicit parity assignment (within-tile slot reshuffle) ----
    # A src node's position parity decides which L2 parity stream its
    # out-edges ride at every consumer.  Within each producer tile the
    # even/odd slot counts are fixed; which node takes which parity is
    # free.  Greedy discrepancy minimization over consumer (core, tile)
    # cells keeps every parity cell near half its unit-range cell.
    tile_of_e = pos_all[core, j_all] >> 7  # dst tile per edge (stable)
    cell_of_e = core * TILES + tile_of_e
    order_e = np.argsort(src, kind="stable")
    e_sorted = cell_of_e[order_e]
    src_sorted = src[order_e]
    starts = np.searchsorted(src_sorted, np.arange(N + 1))
    parity = np.zeros(N, dtype=np.int64)
    for g in range(NB2u):
        D = np.zeros(NC * TILES, dtype=np.int64)
        # nodes of producer cores 4g..4g+3, grouped by (core, tile)
        for r in range(4 * g, min(4 * g + 4, NC)):
            nval = min(NSH, N - r * NSH)
            p = pos_all[r, :nval]
            t = p >> 7
            for tt in range(TILES):
                nodes = np.nonzero(t == tt)[0] + r * NSH
                if nodes.size == 0:
                    continue
                degs = starts[nodes + 1] - starts[nodes]
                nodes = nodes[np.argsort(-degs, kind="stable")]
                n_even = (nodes.size + 1) // 2
                ev_left, od_left = n_even, nodes.size - n_even
                for n in nodes:
                    cells = e_sorted[starts[n] : starts[n + 1]]
                    if ev_left == 0:
                        p_n = 1
                    elif od_left == 0:
                        p_n = 0
                    else:
                        p_n = 0 if D[cells].sum() <= 0 else 1
                    parity[n] = p_n
                    if p_n == 0:
                        ev_left -= 1
                        np.add.at(D, cells, 1)
                    else:
                        od_left -= 1
                        np.add.at(D, cells, -1)
    # reassign within-tile slots by parity class
    for r in range(NC):
        nval = min(NSH, N - r * NSH)
        p = pos_all[r, :nval]
        t = p >> 7
        par_n = parity[r * NSH : r * NSH + nval]
        newpos = np.empty(nval, dtype=np.int64)
        for tt in range(TILES):
            nodes = np.nonzero(t == tt)[0]
            if nodes.size == 0:
                continue
            ev = nodes[par_n[nodes] == 0]
            od = nodes[par_n[nodes] == 1]
            newpos[ev] = tt * 128 + 2 * np.arange(ev.size)
            newpos[od] = tt * 128 + 2 * np.arange(od.size) + 1
        pos_all[r, :nval] = newpos

    pos_e = pos_all[core, j_all]
    tile_of = pos_e >> 7
    dstl = (pos_e & 127).astype(np.float32)

    # permuted g row of every src node -- PIECE-MAJOR g_full layout: the
    # AllGather is split into 4 tile-group pieces; piece p's region holds
    # all 8 ranks' rows for positions [pstart_p, pstart_p + prow_p), so
    # each piece collective writes one contiguous region.
    src_pos = pos_all[sc, src - sc * NSH]
    p_src = piece_of_tile[src_pos >> 7]
    g_row = pbase[p_src] + sc * prows[p_src] + (src_pos - pstart[p_src])
    par = (g_row & 1).astype(np.int64)
    ub = (g_row >> 1) // UBLK

    # ---------------- layer 1: paired descriptors ----------------
    # Each 512 B descriptor serves TWO same-tile edges: the per-core x2
    # table row i holds [x[a_i] | x[a_{i+1}]] along an Eulerian trail of
    # the chosen pair multigraph, so paired srcs sit in one row.  Blocks
    # are fixed 32760-row table windows (int16 idx).
    sel_by_core = [np.nonzero(core == r)[0] for r in range(NC)]
    WCAP = 32760
    NW = 4
    cores_l1 = []
    for r in range(NC):
        sel = sel_by_core[r]
        nval = min(NSH, N - r * NSH)
        jj = np.arange(nval, dtype=np.int64)
        p_self = pos_all[r, :nval]
        i_src = np.concatenate([src[sel], r * NSH + jj])
        i_tile = np.concatenate([tile_of[sel], p_self >> 7])
        i_dstl = np.concatenate(
            [dstl[sel], (p_self & 127).astype(np.float32)]
        )
        i_norm = np.concatenate(
            [w_all[sel], (dinv[r * NSH + jj] ** 2).astype(np.float32)]
        )
        cores_l1.append(
            _l1_pair_core(i_src, i_tile, i_dstl, i_norm, TILES, NW, WCAP)
        )
    # shared cell caps: max over cores, block sums chunk-aligned
    captb1 = np.zeros((NW, TILES), dtype=np.int64)
    for cd in cores_l1:
        np.maximum(captb1, cd["cellcnt"], out=captb1)
    for w in range(NW):
        rem = int(captb1[w].sum()) % 128
        if rem:
            captb1[w, TILES - 1] += 128 - rem
    mm1, off1, Kb1, C1 = _mk_mm(captb1, TILES)
    CAP1 = C1 * 128
    capf1 = captb1.reshape(-1)
    idx1 = np.zeros((NC, CAP1), dtype=np.int16)
    dstl1h = np.full((2, NC, CAP1), -1.0, dtype=np.float32)
    norm1h = np.zeros((2, NC, CAP1), dtype=np.float32)
    for r in range(NC):
        cd = cores_l1[r]
        k = cd["key"]
        order = np.argsort(k, kind="stable")
        k = k[order]
        nkey = capf1.shape[0]
        cr = np.bincount(k, minlength=nkey)
        grp_start = np.zeros(nkey, dtype=np.int64)
        np.cumsum(cr[:-1], out=grp_start[1:])
        rank = np.arange(k.shape[0], dtype=np.int64) - np.repeat(grp_start, cr)
        assert (rank < capf1[k]).all(), "L1 cell overflow"
        slots = off1[k] + rank
        idx1[r, slots] = cd["idxv"][order].astype(np.int16)
        for h in range(2):
            dstl1h[h, r, slots] = cd["dstl"][h][order]
            norm1h[h, r, slots] = cd["norm"][h][order]
    # expand mm rows: one matmul per (desc-chunk, half)
    mm1x = np.empty((mm1.shape[0] * 2, 6), dtype=np.int64)
    mm1x[0::2, :5] = mm1
    mm1x[1::2, :5] = mm1
    mm1x[0::2, 5] = 0
    mm1x[1::2, 5] = 1
    meta1 = _mk_meta_h(mm1x, dstl1h, norm1h)

    # ---------------- layer 2 cells ----------------
    # blocks 0..2*NB2u-1 = (unit range x parity) reading g_full; SELF is a
    # per-tile diag matmul against the SBUF-kept gs tile (block id NBm).
    # Parity cells sized exactly: max over cores + chunk-align block sums.
    NBm = 2 * NB2u
    cnt2 = np.bincount(
        (core * TILES + tile_of) * NBm + (ub * 2 + par),
        minlength=NC * TILES * NBm,
    ).reshape(NC, TILES, NBm)
    captb2m = cnt2.max(axis=0).T.copy()  # [NBm, TILES]
    for b in range(NBm):
        rem = int(captb2m[b].sum()) % 128
        if rem:
            captb2m[b, TILES - 1] += 128 - rem
    mm2, off2, Kb2, C2 = _mk_mm(captb2m, TILES)
    CAP2 = C2 * 128
    key2 = (ub * 2 + par) * TILES + tile_of
    val2 = (g_row >> 1) - ub * UBLK
    idx2, dstl2, norm2 = _fill_slots(
        sel_by_core, key2, val2, dstl, w_all, off2, captb2m.reshape(-1), CAP2
    )
    # self diag region: meta slots [CAP2 + t*128, +128), lane k = node at
    # pos t*128+k, value dinv^2 (dead lanes keep norm 0)
    dstl2s = np.tile(
        np.arange(128, dtype=np.float32)[None, :], (NC, TILES)
    ).reshape(NC, TILES * 128)
    norm2s = np.zeros((NC, TILES * 128), dtype=np.float32)
    for r in range(NC):
        nval = min(NSH, N - r * NSH)
        jj = np.arange(nval, dtype=np.int64)
        p = pos_all[r, :nval]
        norm2s[r, p] = (dinv[r * NSH + jj] ** 2).astype(np.float32)
    dstl2f = np.concatenate([dstl2, dstl2s], axis=1)
    norm2f = np.concatenate([norm2, norm2s], axis=1)
    # interleave: each tile's run starts with its self diag row
    mm2x_l = []
    cur_t = -1
    for row in mm2:
        t = int(row[1])
        if t != cur_t:
            mm2x_l.append(
                (NBm, t, C2 + t, CAP2 + t * 128, CAP2 + t * 128 + 128, 0)
            )
            cur_t = t
        mm2x_l.append((int(row[0]), t, int(row[2]), int(row[3]),
                       int(row[4]), 0))
    mm2x = np.array(mm2x_l, dtype=np.int64)
    meta2 = _mk_meta_h(
        mm2x,
        np.stack([dstl2f, dstl2f]),
        np.stack([norm2f, norm2f]),
    )

    l1_rows = [cores_l1[r]["rows_w"] for r in range(NC)]
    return dict(
        pieces=pieces,
        N=N, E=E, NSH=NSH, TILES=TILES, NB2u=NB2u, UBLK=UBLK,
        NW=NW, WCAP=WCAP, l1_rows=l1_rows,
        C1=C1, M1=mm1x.shape[0], mm1=mm1x, Kb1=Kb1,
        C2=C2, M2=mm2x.shape[0], mm2=mm2x, Kb2=Kb2,
        idx1_sb=_wrap_idx(idx1), idx2_sb=_wrap_idx(idx2),
        meta1=meta1, meta2=meta2,
        pos_all=pos_all,
    )


def _build_bass(sp, for_timing=False):
    import concourse.bass as bass
    import concourse.bacc as bacc
    import concourse.mybir as mybir
    import concourse.tile as tile

    f32 = mybir.dt.float32
    bf16 = mybir.dt.bfloat16
    i16 = mybir.dt.int16
    N, NSH, TILES = sp["N"], sp["NSH"], sp["TILES"]
    NW, WCAP = sp["NW"], sp["WCAP"]
    UBLK, NB2u = sp["UBLK"], sp["NB2u"]
    NU = (N + 1) // 2
    NPAD = TILES * 128

    def mk_calls(Kb, cc):
        calls = []
        chunk0 = 0
        for kb in (int(k) for k in Kb):
            lst = []
            s0, left = chunk0, kb
            while left > 0:
                cs = min(cc, left)
                lst.append((s0, cs))
                s0 += cs
                left -= cs
            calls.append(lst)
            chunk0 += kb
        return calls

    CC1, CC2 = 14, 20
    calls1 = mk_calls(sp["Kb1"], CC1)
    calls2 = mk_calls(sp["Kb2"], CC2)
    MW = 128  # meta window (matmuls per meta tile)

    nc = bacc.Bacc("TRN2", target_bir_lowering=False, debug=False, num_devices=NC)
    x2_in = nc.dram_tensor("x2", [NW * WCAP + 2, 256], bf16, kind="ExternalInput")
    idx1_in = nc.dram_tensor("idx1", [128, sp["C1"] * 8], i16, kind="ExternalInput")
    idx2_in = nc.dram_tensor("idx2", [128, sp["C2"] * 8], i16, kind="ExternalInput")
    meta1_in = nc.dram_tensor("meta1", [128, 2 * sp["M1"]], f32, kind="ExternalInput")
    meta2_in = nc.dram_tensor("meta2", [128, 2 * sp["M2"]], f32, kind="ExternalInput")
    wb_in = nc.dram_tensor("wb", [128, WBW], f32, kind="ExternalInput")
    outT = nc.dram_tensor("outT", [COUT, NPAD], f32, kind="ExternalOutput")

    with tile.TileContext(nc) as tc:
        with (
            tc.tile_pool(name="const", bufs=1) as constp,
            tc.tile_pool(name="stream", bufs=3) as streamp,
            tc.tile_pool(name="msgs1", bufs=3) as msgs1p,
            tc.tile_pool(name="msgs2", bufs=2) as msgs2p,
            tc.tile_pool(name="oh", bufs=3) as ohp,
            tc.tile_pool(name="gskeep", bufs=TILES) as gskeepp,
            tc.tile_pool(name="work", bufs=8) as workp,
            tc.tile_pool(name="pacc", bufs=PSUM_ACC_BUFS, space="PSUM") as paccp,
            tc.tile_pool(name="pproj", bufs=2, space="PSUM") as pprojp,
            tc.tile_pool(name="dram", bufs=1, space="DRAM") as dramp,
        ):
            wb = constp.tile([128, WBW], f32)
            nc.sync.dma_start(wb[:], wb_in[:])
            W1 = wb[:, 0:128]
            W2 = wb[:, 128 : 128 + COUT]
            b1 = wb[:, 384:385]
            b2 = wb[:64, 385:386]
            iota_bf = constp.tile([128, 128], bf16)
            nc.vector.tensor_copy(iota_bf[:], wb[:, 256:384])

            # idx tables live in SBUF for the whole run (one load each)
            idxt1 = constp.tile([128, sp["C1"] * 8], i16)
            nc.sync.dma_start(idxt1[:], idx1_in[:])
            idxt2 = constp.tile([128, sp["C2"] * 8], i16)
            nc.sync.dma_start(idxt2[:], idx2_in[:])

            g_shard = dramp.tile([NSH + 2, COUT], bf16)
            # piece-major g rows + pad (odd-parity streams overread 64
            # elements past the last unit)
            g_full = dramp.tile([N + 4, COUT], bf16)
            gsu = g_shard[:].flatten()
            gfu = g_full[:].flatten()

            def gf_view(off, rows):
                return gfu[off : off + rows * 128].rearrange(
                    "(r c) -> r c", c=128
                )

            def gs_view(off, rows):
                return gsu[off : off + rows * 128].rearrange(
                    "(r c) -> r c", c=128
                )

            # layer-2 stream tables: (unit-range x parity) + self even/odd
            l2_tables = []
            for u0 in range(NB2u):
                for q in range(2):
                    rows = min(UBLK, NU - u0 * UBLK)
                    l2_tables.append(gf_view(u0 * UBLK * 128 + q * 64, rows))
            l1_tables = [
                x2_in[w * WCAP : w * WCAP + WCAP + 1, :] for w in range(NW)
            ]

            def aggregate(tables, mm, calls, meta_in, idxt, nrow, elem,
                          cc, msgsp, epilogue, self_tiles=None,
                          pool_every=0, after_tile=None):
                """Tile-major consumption: each tile's cells (all blocks)
                accumulate in one PSUM bank; block-major gather streams
                feed the matmuls; `epilogue(t, pt)` consumes the full tile."""
                NBt = len(tables)
                s_ci = [0] * NBt
                s_cur = [(-1, 0)] * NBt
                s_mg = [None] * NBt
                mt = None
                cur_w = -1
                pt = None
                cur_t = -1
                oht = None  # grouped one-hot tile: 8 slots per pool alloc
                M = mm.shape[0]
                for i in range(M):
                    b, t, c, lo, hi, half = (int(v) for v in mm[i])
                    is_self = b >= NBt
                    if not is_self:
                        cur0, curk = s_cur[b]
                    if not is_self and (s_mg[b] is None or c >= cur0 + curk):
                        cur0, curk = calls[b][s_ci[b]]
                        s_ci[b] += 1
                        s_cur[b] = (cur0, curk)
                        assert cur0 <= c < cur0 + curk
                        mg = msgsp.tile(
                            [128, cc, elem], bf16, tag=f"msgs{elem}_{b}"
                        )
                        s_mg[b] = mg
                        nc.gpsimd.dma_gather(
                            mg[:, :curk, :],
                            tables[b],
                            idxt[:, cur0 * 8 : (cur0 + curk) * 8],
                            num_idxs=curk * 128,
                            num_idxs_reg=curk * 128,
                            elem_size=elem,
                            single_packet=False,
                        )
                    if i // MW != cur_w:
                        cur_w = i // MW
                        nmw = min(MW, M - cur_w * MW)
                        mt = streamp.tile([128, 2 * MW], f32, tag="meta")
                        nc.sync.dma_start(
                            mt[:, : 2 * nmw],
                            meta_in[:, 2 * cur_w * MW : 2 * (cur_w * MW + nmw)],
                        )
                    if t != cur_t:
                        if pt is not None:
                            epilogue(cur_t, pt)
                            if after_tile is not None:
                                after_tile(cur_t)
                        pt = paccp.tile([128, 128], f32, tag="pacc")
                        cur_t = t
                        first = True
                    else:
                        first = False
                    last = (i == M - 1) or (int(mm[i + 1][1]) != t)
                    mloc = i - cur_w * MW
                    osl = i % 8
                    if osl == 0 or oht is None:
                        oht = ohp.tile([128, 8 * 128], bf16, tag="oh")
                    oh = oht[:, osl * 128 : (osl + 1) * 128]
                    eng = (
                        nc.gpsimd
                        if pool_every and (i % pool_every == pool_every - 1)
                        else nc.vector
                    )
                    eng.tensor_scalar(
                        oh,
                        iota_bf[:],
                        mt[:, 2 * mloc : 2 * mloc + 1],
                        mt[:, 2 * mloc + 1 : 2 * mloc + 2],
                        mybir.AluOpType.is_equal,
                        mybir.AluOpType.mult,
                    )
                    if is_self:
                        lhsT = self_tiles[t][:, :nrow]
                    else:
                        cl = c - cur0
                        co = half * 128
                        lhsT = s_mg[b][:, cl, co : co + nrow]
                    nc.tensor.matmul(
                        pt[:nrow, :],
                        lhsT,
                        oh,
                        start=first,
                        stop=last,
                    )
                epilogue(cur_t, pt)
                if after_tile is not None:
                    after_tile(cur_t)

            # ---------------- layer 1 + piece-wise exchange ----------------
            # The AllGather is split into 4 tile-group pieces, each issued
            # right after its last epilogue: pieces 0-2 transfer during the
            # rest of layer 1 (their local SDMA send/recv work is charged in
            # the timing model via the ring-work copy); only the last
            # piece's link time is exposed (test.py adds it).
            pieces = sp["pieces"]
            if for_timing:
                ring_scratch = dramp.tile(
                    [NC * max(pc[3] for pc in pieces), COUT], bf16
                )

            def emit_piece(p):
                t0, t1, r0w, prow, base = pieces[p]
                if for_timing:
                    nc.gpsimd.dma_start(
                        g_full[base : base + prow, :],
                        g_shard[r0w : r0w + prow, :],
                    )
                    nc.gpsimd.dma_start(
                        ring_scratch[: NC * prow, :],
                        g_full[base : base + NC * prow, :],
                    )
                else:
                    nc.gpsimd.collective_compute(
                        "AllGather",
                        mybir.AluOpType.bypass,
                        replica_groups=[list(range(NC))],
                        ins=[g_shard[r0w : r0w + prow, :]],
                        outs=[g_full[base : base + NC * prow, :]],
                    )

            piece_end = {pc[1] - 1: p for p, pc in enumerate(pieces)}

            def after_tile1(t):
                if t in piece_end:
                    emit_piece(piece_end[t])

            gs_tiles = []  # SBUF-kept per-tile g rows for the L2 self diag

            def epilogue1(t, pt):
                r0 = t * 128
                rows = min(128, NSH - r0)
                aggT = workp.tile([128, 128], f32, tag="aggT")
                nc.scalar.activation(
                    aggT[:], pt[:], mybir.ActivationFunctionType.Copy
                )
                hp = pprojp.tile([128, 128], f32, tag="proj")
                nc.tensor.matmul(hp[:], W1, aggT[:], start=True, stop=True)
                hs = workp.tile([128, 128], f32, tag="hs")
                nc.scalar.activation(
                    hs[:], hp[:], mybir.ActivationFunctionType.Relu,
                    bias=b1, scale=1.0,
                )
                gp = pprojp.tile([128, 128], f32, tag="proj")
                nc.tensor.matmul(gp[:, :COUT], hs[:], W2, start=True, stop=True)
                gs = gskeepp.tile([128, COUT], bf16, tag="gs")
                nc.scalar.activation(
                    gs[:], gp[:, :COUT], mybir.ActivationFunctionType.Copy
                )
                gs_tiles.append(gs)
                nc.sync.dma_start(g_shard[r0 : r0 + rows, :], gs[:rows, :])

            aggregate(l1_tables, sp["mm1"], calls1, meta1_in, idxt1, 128,
                      256, CC1, msgs1p, epilogue1, pool_every=8,
                      after_tile=after_tile1)

            # ---------------- layer 2 ----------------
            def epilogue2(t, pt):
                r0 = t * 128
                cols = min(128, NSH - r0)
                ob = workp.tile([64, 128], f32, tag="ob")
                nc.scalar.activation(
                    ob[:],
                    pt[:COUT, :],
                    mybir.ActivationFunctionType.Identity,
                    bias=b2,
                    scale=1.0,
                )
                nc.sync.dma_start(outT[:, r0 : r0 + cols], ob[:, :cols])

            aggregate(l2_tables, sp["mm2"], calls2, meta2_in, idxt2, COUT,
                      128, CC2, msgs2p, epilogue2, self_tiles=gs_tiles)

    nc.compile()
    return nc


_CACHE = {}


def _get_program(sp):
    key = (sp["N"], sp["C1"], sp["C2"], sp["mm1"].tobytes(), sp["mm2"].tobytes())
    if key not in _CACHE:
        _CACHE[key] = _build_bass(sp)
    return _CACHE[key]


def _make_wb(W1, b1, W2, b2):
    wb = np.zeros((128, WBW), dtype=np.float32)
    wb[:, 0:128] = np.asarray(W1, dtype=np.float32)
    wb[:, 128 : 128 + COUT] = np.asarray(W2, dtype=np.float32)
    wb[:, 256:384] = np.arange(128, dtype=np.float32)[None, :]
    wb[:, 384] = np.asarray(b1, dtype=np.float32)
    wb[:64, 385] = np.asarray(b2, dtype=np.float32)
    return wb


def make_in_maps(sp, x, W1, b1, W2, b2):
    xb = np.ascontiguousarray(np.asarray(x, dtype=np.float32).astype(BF16))
    wb = _make_wb(W1, b1, W2, b2)
    NW, WCAP = sp["NW"], sp["WCAP"]
    maps = []
    for r in range(NC):
        x2 = np.zeros((NW * WCAP + 2, 256), dtype=BF16)
        for w, nodes in enumerate(sp["l1_rows"][r]):
            R = nodes.shape[0]
            base = w * WCAP
            x2[base : base + R, 0:128] = xb[nodes]
            if R > 1:
                x2[base : base + R - 1, 128:256] = xb[nodes[1:]]
        maps.append(
            {
                "x2": x2,
                "idx1": sp["idx1_sb"][r],
                "idx2": sp["idx2_sb"][r],
                "meta1": sp["meta1"][r],
                "meta2": sp["meta2"][r],
                "wb": wb,
            }
        )
    return maps


def kernel(x, edge_index, W1, b1, W2, b2, _trace=False):
    from concourse.bass_utils import run_bass_kernel_spmd

    x = np.asarray(x, dtype=np.float32)
    N = x.shape[0]
    sp = _schedule(np.asarray(edge_index), N)
    nc = _get_program(sp)
    in_maps = make_in_maps(sp, x, W1, b1, W2, b2)
    res = run_bass_kernel_spmd(nc, in_maps, list(range(NC)), trace=_trace)

    NSH = sp["NSH"]
    out = np.empty((N, COUT), dtype=np.float32)
    for r in range(NC):
        lo = r * NSH
        hi = min(N, lo + NSH)
        out[lo:hi] = res.results[r]["outT"][:, sp["pos_all"][r, : hi - lo]].T
    if _trace:
        kernel.last_result = res
    return out


# revision 81
# speedup vs baseline: 1.6887x; 1.0043x over previous
"""2-layer GCN on 8 Trainium2 NeuronCores (Bass/Tile).

Math: gcn_conv(x, W, b) = D^-1/2 (A+I) D^-1/2 (x W) + b.  Propagation
commutes with the weight matmul, so layer 1 aggregates raw x
(h = relu((A_hat x) W1 + b1)) and layer 2 projects first
(out = A_hat (h W2) + b2), minimizing per-edge gather bytes.

Distribution: destination nodes sharded 8 ways (12500/core); each core
gathers source features for its own edges from a full local feature table
(x is an input; g = h W2 is assembled with one 8-rank AllGather).

g is stored at its true width (64 cols bf16 = 128 B rows) so the
AllGather moves half the bytes of a 128-padded layout.  dma_gather
requires 256 B-aligned elements, so layer-2 gathers fetch a 256 B *unit*
(two consecutive g rows) per edge: streams come in (unit-range, parity)
flavors whose table APs are offset by 64 elements so the wanted row
always lands in msgs cols 0:64.

Per-core aggregation, per layer:
- block-major dma_gather streams (int16 idx limit => <=25000-row/unit
  tables). L1: 4 source-range blocks + a SELF stream reading x_own.
  L2: 4 (unit-range x parity) blocks reading g_full + 2 parity SELF
  streams reading g_shard.
- Edges are laid out in fractional per-(tile, block) cells balanced by a
  per-core node permutation (greedy vector bin packing), ~2% padding;
  boundary chunks are consumed by two matmuls with foreign lanes zeroed.
- Consumption is tile-major: bf16 one-hot scatter matrices
  (tensor_scalar is_equal*norm) feed PE matmuls msgs^T @ onehot
  (bf16 -> fp32 PSUM); each destination tile accumulates all its cells
  in one PSUM bank, then a single ACT-engine escape feeds the inline
  projection (W1 -> relu -> W2 -> bf16 g row) or the bias+output write.
"""
import sys

sys.path.insert(0, "/opt/trn_rl_repo")
import numpy as np
import ml_dtypes

BF16 = ml_dtypes.bfloat16
NC = 8
CIN, CH, COUT = 128, 128, 64
CALL_CHUNKS = 36  # gather-call granularity (chunks of 128 edges)
PSUM_ACC_BUFS = 6  # concurrent per-tile accumulation banks
WBW = 386  # wb cols: W1[0:128] W2[128:192] iota[256:384] b1[384] b2[385]


def _balance(nv, caps, k_tb):
    """Best-fit-decreasing node->tile assignment for one core.

    nv: [NSH, NB] per-node block in-degree vectors (both layers' blocks).
    caps: [TILES] node slots per tile.  k_tb: [TILES, NB] cell targets.
    Returns pos[NSH] (node -> global slot = tile*128 + slot_in_tile) or None
    if infeasible.
    """
    NSH, NB = nv.shape
    TILES = caps.shape[0]
    capv = k_tb.astype(np.float64)
    np.maximum(capv, 1e-9, out=capv)
    usedv = np.zeros((TILES, NB), dtype=np.float64)
    usect = np.zeros(TILES, dtype=np.int64)
    pos = np.empty(NSH, dtype=np.int64)
    order = np.argsort(-nv.sum(1), kind="stable")
    for n in order:
        v = nv[n].astype(np.float64)
        post = (usedv + v[None, :]) / capv
        feas = (post <= 1.0).all(1) & (usect < caps)
        if not feas.any():
            return None
        score = np.maximum(post.max(1), (usect + 1) / caps)
        score[~feas] = np.inf
        t = int(np.argmin(score))
        usedv[t] += v
        pos[n] = t * 128 + usect[t]
        usect[t] += 1
    return pos


def _mk_mm(captb, TILES):
    """Tile-major matmul table for a cell-capacity matrix [NB, TILES].

    Returns (mm [(b,t,c,lo,hi)], off_flat, Kb, C).  Chunk numbering is
    global across blocks (block b's chunks follow block b-1's).
    """
    NB = captb.shape[0]
    nkey = NB * TILES
    off_flat = np.zeros(nkey + 1, dtype=np.int64)
    np.cumsum(captb.reshape(-1), out=off_flat[1:])
    CAP = int(off_flat[-1])
    assert CAP % 128 == 0
    # block starts must be chunk-aligned so gather streams stay block-major
    blk_cap = captb.sum(axis=1)
    assert (blk_cap % 128 == 0).all(), blk_cap
    Kb = blk_cap // 128
    C = int(Kb.sum())
    mm = []
    for t in range(TILES):
        for b in range(NB):
            o = int(off_flat[b * TILES + t])
            cap = int(captb[b, t])
            if cap == 0:
                continue
            c0, c1 = o >> 7, (o + cap - 1) >> 7
            for c in range(c0, c1 + 1):
                lo = max(o, c * 128)
                hi = min(o + cap, (c + 1) * 128)
                mm.append((b, t, c, lo, hi))
    return np.array(mm, dtype=np.int64), off_flat, Kb, C


def _fill_slots(sel_by_core, key_all, val_all, dstl_all, norm_all, off_flat,
                capf, CAP):
    """Scatter per-edge records into their cell slots for all cores.

    Returns (idx [NC, CAP] int16, dstl [NC, CAP] f32, norm [NC, CAP] f32).
    """
    idx = np.zeros((NC, CAP), dtype=np.int16)
    dst = np.full((NC, CAP), -1.0, dtype=np.float32)
    nrm = np.zeros((NC, CAP), dtype=np.float32)
    for r in range(NC):
        sel = sel_by_core[r]
        k = key_all[sel]
        order = np.argsort(k, kind="stable")
        sel = sel[order]
        k = k[order]
        nkey = capf.shape[0]
        cr = np.bincount(k, minlength=nkey)
        grp_start = np.zeros(nkey, dtype=np.int64)
        np.cumsum(cr[:-1], out=grp_start[1:])
        rank = np.arange(sel.shape[0], dtype=np.int64) - np.repeat(grp_start, cr)
        assert (rank < capf[k]).all(), "cell overflow"
        slots = off_flat[k] + rank
        idx[r, slots] = val_all[sel].astype(np.int16)
        dst[r, slots] = dstl_all[sel]
        nrm[r, slots] = norm_all[sel]
    return idx, dst, nrm


def _wrap_idx(a):
    NCc, CAP = a.shape
    return np.ascontiguousarray(
        np.tile(a.reshape(NCc, CAP // 16, 16).transpose(0, 2, 1), (1, 8, 1))
    )


def _l1_pair_core(i_src, i_tile, i_dstl, i_norm, TILES, NW, WCAP):
    """Pair one core's L1 edge instances and lay out its x2 trail table.

    Any two same-tile instances may pair (one 512 B descriptor).  Pairs
    form a multigraph over src nodes; Eulerian trails lay it out so each
    pair occupies one table row [x[u] | x[v]].  Returns per-descriptor
    records plus per-window row node sequences.
    """
    order = np.lexsort((i_src, i_tile))
    ts = i_tile[order]
    cnt = np.bincount(ts, minlength=TILES)
    starts = np.zeros(TILES + 1, dtype=np.int64)
    np.cumsum(cnt, out=starts[1:])
    pA_l, pB_l, sing_l = [], [], []
    for t in range(TILES):
        s, e = int(starts[t]), int(starts[t + 1])
        k = (e - s) // 2
        seg = order[s:e]
        pA_l.append(seg[0 : 2 * k : 2])
        pB_l.append(seg[1 : 2 * k : 2])
        if (e - s) % 2:
            sing_l.append(int(seg[-1]))
    pA = np.concatenate(pA_l)
    pB = np.concatenate(pB_l)
    P = pA.shape[0]

    verts, inv = np.unique(
        np.concatenate([i_src[pA], i_src[pB]]), return_inverse=True
    )
    a = inv[:P].astype(np.int64)
    b = inv[P:].astype(np.int64)
    V = verts.shape[0]

    # union-find for components
    parent = np.arange(V, dtype=np.int64)

    def find(x):
        root = x
        while parent[root] != root:
            root = parent[root]
        while parent[x] != root:
            parent[x], x = root, parent[x]
        return root

    for i in range(P):
        ra, rb = find(a[i]), find(b[i])
        if ra != rb:
            parent[ra] = rb
    comp = np.fromiter((find(i) for i in range(V)), np.int64, V)

    # pair odd-degree vertices within components with virtual edges
    deg = np.bincount(a, minlength=V) + np.bincount(b, minlength=V)
    odd = np.nonzero(deg % 2 == 1)[0]
    oorder = odd[np.argsort(comp[odd], kind="stable")]
    va, vb = oorder[0::2], oorder[1::2]
    NE = P + va.shape[0]
    ea = np.concatenate([a, va])
    eb = np.concatenate([b, vb])

    # CSR half-edge adjacency + iterative Hierholzer
    he_v = np.concatenate([ea, eb])
    hstart = np.zeros(V + 1, dtype=np.int64)
    np.cumsum(np.bincount(he_v, minlength=V), out=hstart[1:])
    hlist = np.argsort(he_v, kind="stable")
    ptr = hstart[:-1].copy()
    used = np.zeros(NE, dtype=bool)
    trails = []  # (nodes [k+1], lo_inst [k], hi_inst [k])
    for v0 in range(V):
        while True:
            while ptr[v0] < hstart[v0 + 1] and used[hlist[ptr[v0]] % NE]:
                ptr[v0] += 1
            if ptr[v0] >= hstart[v0 + 1]:
                break
            stack_v = [v0]
            stack_e = []
            circ = []  # edge ids, circuit order
            while stack_v:
                x = stack_v[-1]
                while ptr[x] < hstart[x + 1] and used[hlist[ptr[x]] % NE]:
                    ptr[x] += 1
                if ptr[x] < hstart[x + 1]:
                    e = hlist[ptr[x]] % NE
                    used[e] = True
                    stack_v.append(ea[e] + eb[e] - x)
                    stack_e.append(e)
                else:
                    stack_v.pop()
                    if stack_e:
                        circ.append(stack_e.pop())
            circ.reverse()
            # walk the circuit, splitting at virtual edges
            v = v0
            nodes, lo_i, hi_i = [v], [], []
            for e in circ:
                if e >= P:  # virtual: close current trail
                    if lo_i:
                        trails.append((nodes, lo_i, hi_i))
                    v = ea[e] + eb[e] - v
                    nodes, lo_i, hi_i = [v], [], []
                else:
                    nv_ = ea[e] + eb[e] - v
                    lo = pA[e] if a[e] == v else pB[e]
                    hi = pA[e] + pB[e] - lo
                    if a[e] == b[e]:  # self-loop: either orientation
                        lo, hi = pA[e], pB[e]
                    nodes.append(nv_)
                    lo_i.append(lo)
                    hi_i.append(hi)
                    v = nv_
            if lo_i:
                trails.append((nodes, lo_i, hi_i))

    singles = list(sing_l)  # instance ids; ride an occurrence row
    rdeg = deg.copy()  # remaining real-edge degree
    total_rows = sum(len(n) for n, _, _ in trails)
    budget = NW * WCAP - 256  # headroom for orphan rows below
    # spill: pop trail-end pairs into singles until rows fit the windows
    guard = 0
    while total_rows > budget:
        progress = False
        for tr in trails:
            if total_rows <= budget:
                break
            nodes, lo_i, hi_i = tr
            if not lo_i:
                continue
            u_end = nodes[-1]
            u_prev = nodes[-2]
            d_end = rdeg[u_end] - (2 if u_end == u_prev else 1)
            if d_end < 1:
                continue
            if len(lo_i) == 1 and rdeg[u_prev] - 1 < 1:
                continue
            singles.append(lo_i.pop())
            singles.append(hi_i.pop())
            nodes.pop()
            rdeg[u_end] -= 1
            rdeg[u_prev] -= 1
            total_rows -= 1
            progress = True
        guard += 1
        assert progress and guard < 50, "L1 spill failed"
    trails = [t for t in trails if t[1]]

    # orphan singles: src with no remaining occurrence gets a 1-row trail
    occ_ok = np.zeros(V, dtype=bool)
    occ_ok[np.nonzero(rdeg > 0)[0]] = True
    vmap = {int(v): i for i, v in enumerate(verts)}
    fixed = []
    for s in singles:
        u = int(i_src[s])
        ui = vmap.get(u, -1)
        if ui >= 0 and occ_ok[ui]:
            fixed.append((s, ui))
        else:
            if ui < 0:
                vmap[u] = ui = V
                verts = np.append(verts, u)
                occ_ok = np.append(occ_ok, False)
                V += 1
            trails.append(([ui], [], []))  # 1-row trail, ridden by single
            occ_ok[ui] = True
            total_rows += 1
            fixed.append((s, ui))
    assert total_rows <= NW * WCAP, "L1 rows over budget after orphans"

    # window assignment: greedy vector packing on tile histograms
    tilecnt = np.bincount(i_tile, minlength=TILES).astype(np.float64)
    target = np.maximum(tilecnt / (2 * NW), 1.0)
    cellcnt = np.zeros((NW, TILES), dtype=np.int64)
    rows_used = np.zeros(NW, dtype=np.int64)
    tr_rows = np.array([len(n) for n, _, _ in trails])
    tr_order = np.argsort(-tr_rows, kind="stable")
    win_of = np.empty(len(trails), dtype=np.int64)
    for ti in tr_order:
        nodes, lo_i, hi_i = trails[ti]
        h = np.bincount(i_tile[lo_i], minlength=TILES) if lo_i else 0
        best_w, best_s = -1, None
        for w in range(NW):
            if rows_used[w] + len(nodes) > WCAP:
                continue
            s = ((cellcnt[w] + h) / target).max() if lo_i else (
                rows_used[w] / WCAP
            )
            if best_w < 0 or s < best_s:
                best_w, best_s = w, s
        assert best_w >= 0, "window overflow"
        win_of[ti] = best_w
        rows_used[best_w] += len(nodes)
        if lo_i:
            cellcnt[best_w] += h
    # build per-window row sequences + per-step descriptor records
    rows_w = [[] for _ in range(NW)]
    roff = np.zeros(NW, dtype=np.int64)
    d_key, d_idx = [], []
    d_dl = [[], []]
    d_nm = [[], []]
    first_occ = {}
    for ti, (nodes, lo_i, hi_i) in enumerate(trails):
        w = win_of[ti]
        base = roff[w]
        rows_w[w].append(np.asarray(nodes, dtype=np.int64))
        for k_ in range(len(nodes)):
            u = nodes[k_]
            if u not in first_occ:
                first_occ[u] = (w, base + k_)
        for k_, (lo, hi) in enumerate(zip(lo_i, hi_i)):
            d_key.append(w * TILES + int(i_tile[lo]))
            d_idx.append(base + k_)
            d_dl[0].append(i_dstl[lo])
            d_nm[0].append(i_norm[lo])
            d_dl[1].append(i_dstl[hi])
            d_nm[1].append(i_norm[hi])
        roff[w] += len(nodes)
    for s, ui in fixed:
        w, row = first_occ[ui]
        d_key.append(w * TILES + int(i_tile[s]))
        d_idx.append(row)
        d_dl[0].append(i_dstl[s])
        d_nm[0].append(i_norm[s])
        d_dl[1].append(-1.0)
        d_nm[1].append(0.0)
    key = np.asarray(d_key, dtype=np.int64)
    cell_final = np.bincount(key, minlength=NW * TILES).reshape(NW, TILES)
    return dict(
        cellcnt=cell_final,
        key=key,
        idxv=np.asarray(d_idx, dtype=np.int64),
        dstl=[np.asarray(d_dl[0], np.float32), np.asarray(d_dl[1], np.float32)],
        norm=[np.asarray(d_nm[0], np.float32), np.asarray(d_nm[1], np.float32)],
        rows_w=[
            verts[np.concatenate(rw)] if rw else np.zeros(0, dtype=np.int64)
            for rw in rows_w
        ],
    )


def _mk_meta_h(mmx, dstl_h, norm_h):
    """Meta for 6-column mm rows (b, t, c, lo, hi, half)."""
    M = mmx.shape[0]
    md = np.full((NC, M, 128), -1.0, dtype=np.float32)
    mn = np.zeros((NC, M, 128), dtype=np.float32)
    for i in range(M):
        b, t, c, lo, hi, h = mmx[i]
        base = int(c) * 128
        md[:, i, lo - base : hi - base] = dstl_h[h][:, lo:hi]
        mn[:, i, lo - base : hi - base] = norm_h[h][:, lo:hi]
    meta = np.empty((NC, 128, 2 * M), dtype=np.float32)
    meta[:, :, 0::2] = md.transpose(0, 2, 1)
    meta[:, :, 1::2] = mn.transpose(0, 2, 1)
    return meta


def _mk_meta(mm, dstl_all, norm_all):
    M = mm.shape[0]
    md = np.full((NC, M, 128), -1.0, dtype=np.float32)
    mn = np.zeros((NC, M, 128), dtype=np.float32)
    for i in range(M):
        b, t, c, lo, hi = mm[i]
        base = int(c) * 128
        md[:, i, lo - base : hi - base] = dstl_all[:, lo:hi]
        mn[:, i, lo - base : hi - base] = norm_all[:, lo:hi]
    meta = np.empty((NC, 128, 2 * M), dtype=np.float32)
    meta[:, :, 0::2] = md.transpose(0, 2, 1)
    meta[:, :, 1::2] = mn.transpose(0, 2, 1)
    return meta


def _schedule(edge_index, n_nodes):
    """Static SPMD schedule + per-core edge arrays for both layers."""
    N = n_nodes
    NSH = (N + NC - 1) // NC
    TILES = (NSH + 127) // 128
    BLK = 25000  # L1 x-table rows per block
    NB1 = -(-N // BLK)  # L1 source-range blocks
    UBLK = 25000  # L2 units per block (unit = 2 nodes)
    NU = (N + 1) // 2
    NB2u = -(-NU // UBLK)  # L2 unit-range blocks
    assert NSH % 2 == 0

    src = np.asarray(edge_index[0], dtype=np.int64)
    dst = np.asarray(edge_index[1], dtype=np.int64)
    E = src.shape[0]
    deg = np.bincount(dst, minlength=N).astype(np.float64) + 1.0
    dinv = 1.0 / np.sqrt(deg)
    w_all = (dinv[src] * dinv[dst]).astype(np.float32)

    core = dst // NSH
    j_all = dst - core * NSH
    blk1 = src // BLK

    # per-core per-node block in-degree vectors for balance:
    # L1 blocks (src range, self excluded) ++ L2 blocks (unit range x parity,
    # self excluded -- self parity depends on pos, handled by SELF streams)
    sc = src // NSH  # src core
    ub = sc // 4  # L2 unit-range block (aligned with shard pairs)
    nv1 = np.bincount(
        (core * NSH + j_all) * NB1 + blk1, minlength=NC * NSH * NB1
    ).reshape(NC, NSH, NB1)
    B1 = nv1.sum(axis=1)

    caps = np.full(TILES, 128, dtype=np.int64)
    caps[TILES - 1] = NSH - 128 * (TILES - 1)

    # exchange piece geometry (needed to label L2 blocks): tile groups
    PIECE_T = [0, 12, 24, 36, 48, 60, 72, 84, TILES]
    NP_ = len(PIECE_T) - 1
    pieces = []
    for p in range(NP_):
        t0, t1 = PIECE_T[p], PIECE_T[p + 1]
        r0 = int(caps[:t0].sum())
        r1 = int(caps[:t1].sum())
        assert (r1 - r0) % 2 == 0 and r0 % 2 == 0
        pieces.append([t0, t1, r0, r1 - r0, 0])
    prows = np.array([pc[3] for pc in pieces], dtype=np.int64)
    pstart = np.array([pc[2] for pc in pieces], dtype=np.int64)
    pbase = np.zeros(NP_, dtype=np.int64)
    pbase[1:] = np.cumsum(prows * NC)[:-1]
    for p in range(NP_):
        pieces[p][4] = int(pbase[p])
    piece_of_tile = np.zeros(TILES, dtype=np.int64)
    for p in range(NP_):
        piece_of_tile[PIECE_T[p] : PIECE_T[p + 1]] = p

    # The balance sees L1 source-range blocks; pass 2 adds L2 unit-range
    # columns labeled from pass-1 positions (the label of an edge depends
    # on where OTHER cores placed its src, so it needs a first pass).
    def spread(tot):
        tgt = tot * caps / NSH
        base = np.floor(tgt).astype(np.int64)
        rem = int(tot - base.sum())
        order = np.argsort(-(tgt - base))
        base[order[:rem]] += 1
        return base

    nv2u = None
    for pass_ in range(2):
        slack = 256
        for _attempt in range(8):
            TOT1 = ((B1.max(axis=0) + slack + 127) // 128) * 128  # [NB1]
            captb1 = np.stack([spread(t) for t in TOT1])  # [NB1, TILES]
            if nv2u is None:
                k_tb = captb1.T
                nv = nv1
            else:
                TOT2u = nv2u.sum(axis=1).max(axis=0) + 2 * slack
                captb2u = np.stack([spread(t) for t in TOT2u])
                k_tb = np.concatenate([captb1.T, captb2u.T], axis=1)
                nv = np.concatenate([nv1, nv2u], axis=2)
            pos_try = np.empty((NC, NSH), dtype=np.int64)
            ok = True
            for r in range(NC):
                pos = _balance(nv[r], caps, k_tb)
                if pos is None:
                    ok = False
                    break
                pos_try[r] = pos
            if ok:
                pos_all = pos_try
                break
            slack *= 2
        if not ok:
            assert pass_ == 1, "balance failed"
            break  # keep pass-1 positions
        if pass_ == 0:
            sp0 = pos_all[sc, src - sc * NSH]
            pp0 = piece_of_tile[sp0 >> 7]
            g0 = pbase[pp0] + sc * prows[pp0] + (sp0 - pstart[pp0])
            ub0 = (g0 >> 1) // UBLK
            nv2u = np.bincount(
                (core * NSH + j_all) * NB2u + ub0,
                minlength=NC * NSH * NB2u,
            ).reshape(NC, NSH, NB2u)

    # ---- explicit parity assignment (within-tile slot reshuffle) ----
    # A src node's position parity decides which L2 parity stream its
    # out-edges ride at every consumer.  Within each producer tile the
    # even/odd slot counts are fixed; which node takes which parity is
    # free.  Greedy discrepancy minimization over consumer (core, tile)
    # cells keeps every parity cell near half its unit-range cell.
    tile_of_e = pos_all[core, j_all] >> 7  # dst tile per edge (stable)
    cell_of_e = core * TILES + tile_of_e
    order_e = np.argsort(src, kind="stable")
    e_sorted = cell_of_e[order_e]
    src_sorted = src[order_e]
    starts = np.searchsorted(src_sorted, np.arange(N + 1))
    parity = np.zeros(N, dtype=np.int64)
    for g in range(NB2u):
        D = np.zeros(NC * TILES, dtype=np.int64)
        # nodes of producer cores 4g..4g+3, grouped by (core, tile)
        for r in range(4 * g, min(4 * g + 4, NC)):
            nval = min(NSH, N - r * NSH)
            p = pos_all[r, :nval]
            t = p >> 7
            for tt in range(TILES):
                nodes = np.nonzero(t == tt)[0] + r * NSH
                if nodes.size == 0:
                    continue
                degs = starts[nodes + 1] - starts[nodes]
                nodes = nodes[np.argsort(-degs, kind="stable")]
                n_even = (nodes.size + 1) // 2
                ev_left, od_left = n_even, nodes.size - n_even
                for n in nodes:
                    cells = e_sorted[starts[n] : starts[n + 1]]
                    if ev_left == 0:
                        p_n = 1
                    elif od_left == 0:
                        p_n = 0
                    else:
                        p_n = 0 if D[cells].sum() <= 0 else 1
                    parity[n] = p_n
                    if p_n == 0:
                        ev_left -= 1
                        np.add.at(D, cells, 1)
                    else:
                        od_left -= 1
                        np.add.at(D, cells, -1)
    # reassign within-tile slots by parity class
    for r in range(NC):
        nval = min(NSH, N - r * NSH)
        p = pos_all[r, :nval]
        t = p >> 7
        par_n = parity[r * NSH : r * NSH + nval]
        newpos = np.empty(nval, dtype=np.int64)
        for tt in range(TILES):
            nodes = np.nonzero(t == tt)[0]
            if nodes.size == 0:
                continue
            ev = nodes[par_n[nodes] == 0]
            od = nodes[par_n[nodes] == 1]
            newpos[ev] = tt * 128 + 2 * np.arange(ev.size)
            newpos[od] = tt * 128 + 2 * np.arange(od.size) + 1
        pos_all[r, :nval] = newpos

    pos_e = pos_all[core, j_all]
    tile_of = pos_e >> 7
    dstl = (pos_e & 127).astype(np.float32)

    # permuted g row of every src node -- PIECE-MAJOR g_full layout: the
    # AllGather is split into 4 tile-group pieces; piece p's region holds
    # all 8 ranks' rows for positions [pstart_p, pstart_p + prow_p), so
    # each piece collective writes one contiguous region.
    src_pos = pos_all[sc, src - sc * NSH]
    p_src = piece_of_tile[src_pos >> 7]
    g_row = pbase[p_src] + sc * prows[p_src] + (src_pos - pstart[p_src])
    par = (g_row & 1).astype(np.int64)
    ub = (g_row >> 1) // UBLK

    # ---------------- layer 1: paired descriptors ----------------
    # Each 512 B descriptor serves TWO same-tile edges: the per-core x2
    # table row i holds [x[a_i] | x[a_{i+1}]] along an Eulerian trail of
    # the chosen pair multigraph, so paired srcs sit in one row.  Blocks
    # are fixed 32760-row table windows (int16 idx).
    sel_by_core = [np.nonzero(core == r)[0] for r in range(NC)]
    WCAP = 32760
    NW = 4
    cores_l1 = []
    for r in range(NC):
        sel = sel_by_core[r]
        nval = min(NSH, N - r * NSH)
        jj = np.arange(nval, dtype=np.int64)
        p_self = pos_all[r, :nval]
        i_src = np.concatenate([src[sel], r * NSH + jj])
        i_tile = np.concatenate([tile_of[sel], p_self >> 7])
        i_dstl = np.concatenate(
            [dstl[sel], (p_self & 127).astype(np.float32)]
        )
        i_norm = np.concatenate(
            [w_all[sel], (dinv[r * NSH + jj] ** 2).astype(np.float32)]
        )
        cores_l1.append(
            _l1_pair_core(i_src, i_tile, i_dstl, i_norm, TILES, NW, WCAP)
        )
    # shared cell caps: max over cores, block sums chunk-aligned
    captb1 = np.zeros((NW, TILES), dtype=np.int64)
    for cd in cores_l1:
        np.maximum(captb1, cd["cellcnt"], out=captb1)
    for w in range(NW):
        rem = int(captb1[w].sum()) % 128
        if rem:
            captb1[w, TILES - 1] += 128 - rem
    mm1, off1, Kb1, C1 = _mk_mm(captb1, TILES)
    CAP1 = C1 * 128
    capf1 = captb1.reshape(-1)
    idx1 = np.zeros((NC, CAP1), dtype=np.int16)
    dstl1h = np.full((2, NC, CAP1), -1.0, dtype=np.float32)
    norm1h = np.zeros((2, NC, CAP1), dtype=np.float32)
    for r in range(NC):
        cd = cores_l1[r]
        k = cd["key"]
        order = np.argsort(k, kind="stable")
        k = k[order]
        nkey = capf1.shape[0]
        cr = np.bincount(k, minlength=nkey)
        grp_start = np.zeros(nkey, dtype=np.int64)
        np.cumsum(cr[:-1], out=grp_start[1:])
        rank = np.arange(k.shape[0], dtype=np.int64) - np.repeat(grp_start, cr)
        assert (rank < capf1[k]).all(), "L1 cell overflow"
        slots = off1[k] + rank
        idx1[r, slots] = cd["idxv"][order].astype(np.int16)
        for h in range(2):
            dstl1h[h, r, slots] = cd["dstl"][h][order]
            norm1h[h, r, slots] = cd["norm"][h][order]
    # expand mm rows: one matmul per (desc-chunk, half)
    mm1x = np.empty((mm1.shape[0] * 2, 6), dtype=np.int64)
    mm1x[0::2, :5] = mm1
    mm1x[1::2, :5] = mm1
    mm1x[0::2, 5] = 0
    mm1x[1::2, 5] = 1
    meta1 = _mk_meta_h(mm1x, dstl1h, norm1h)

    # ---------------- layer 2 cells ----------------
    # blocks 0..2*NB2u-1 = (unit range x parity) reading g_full; SELF is a
    # per-tile diag matmul against the SBUF-kept gs tile (block id NBm).
    # Parity cells sized exactly: max over cores + chunk-align block sums.
    NBm = 2 * NB2u
    cnt2 = np.bincount(
        (core * TILES + tile_of) * NBm + (ub * 2 + par),
        minlength=NC * TILES * NBm,
    ).reshape(NC, TILES, NBm)
    captb2m = cnt2.max(axis=0).T.copy()  # [NBm, TILES]
    for b in range(NBm):
        rem = int(captb2m[b].sum()) % 128
        if rem:
            captb2m[b, TILES - 1] += 128 - rem
    mm2, off2, Kb2, C2 = _mk_mm(captb2m, TILES)
    CAP2 = C2 * 128
    key2 = (ub * 2 + par) * TILES + tile_of
    val2 = (g_row >> 1) - ub * UBLK
    idx2, dstl2, norm2 = _fill_slots(
        sel_by_core, key2, val2, dstl, w_all, off2, captb2m.reshape(-1), CAP2
    )
    # self diag region: meta slots [CAP2 + t*128, +128), lane k = node at
    # pos t*128+k, value dinv^2 (dead lanes keep norm 0)
    dstl2s = np.tile(
        np.arange(128, dtype=np.float32)[None, :], (NC, TILES)
    ).reshape(NC, TILES * 128)
    norm2s = np.zeros((NC, TILES * 128), dtype=np.float32)
    for r in range(NC):
        nval = min(NSH, N - r * NSH)
        jj = np.arange(nval, dtype=np.int64)
        p = pos_all[r, :nval]
        norm2s[r, p] = (dinv[r * NSH + jj] ** 2).astype(np.float32)
    dstl2f = np.concatenate([dstl2, dstl2s], axis=1)
    norm2f = np.concatenate([norm2, norm2s], axis=1)
    # interleave: each tile's run starts with its self diag row
    mm2x_l = []
    cur_t = -1
    for row in mm2:
        t = int(row[1])
        if t != cur_t:
            mm2x_l.append(
                (NBm, t, C2 + t, CAP2 + t * 128, CAP2 + t * 128 + 128, 0)
            )
            cur_t = t
        mm2x_l.append((int(row[0]), t, int(row[2]), int(row[3]),
                       int(row[4]), 0))
    mm2x = np.array(mm2x_l, dtype=np.int64)
    meta2 = _mk_meta_h(
        mm2x,
        np.stack([dstl2f, dstl2f]),
        np.stack([norm2f, norm2f]),
    )

    l1_rows = [cores_l1[r]["rows_w"] for r in range(NC)]
    return dict(
        pieces=pieces,
        N=N, E=E, NSH=NSH, TILES=TILES, NB2u=NB2u, UBLK=UBLK,
        NW=NW, WCAP=WCAP, l1_rows=l1_rows,
        C1=C1, M1=mm1x.shape[0], mm1=mm1x, Kb1=Kb1,
        C2=C2, M2=mm2x.shape[0], mm2=mm2x, Kb2=Kb2,
        idx1_sb=_wrap_idx(idx1), idx2_sb=_wrap_idx(idx2),
        meta1=meta1, meta2=meta2,
        pos_all=pos_all,
    )


def _build_bass(sp, for_timing=False):
    import concourse.bass as bass
    import concourse.bacc as bacc
    import concourse.mybir as mybir
    import concourse.tile as tile

    f32 = mybir.dt.float32
    bf16 = mybir.dt.bfloat16
    i16 = mybir.dt.int16
    N, NSH, TILES = sp["N"], sp["NSH"], sp["TILES"]
    NW, WCAP = sp["NW"], sp["WCAP"]
    UBLK, NB2u = sp["UBLK"], sp["NB2u"]
    NU = (N + 1) // 2
    NPAD = TILES * 128

    def mk_calls(Kb, cc):
        calls = []
        chunk0 = 0
        for kb in (int(k) for k in Kb):
            lst = []
            s0, left = chunk0, kb
            while left > 0:
                cs = min(cc, left)
                lst.append((s0, cs))
                s0 += cs
                left -= cs
            calls.append(lst)
            chunk0 += kb
        return calls

    CC1, CC2 = 14, 20
    calls1 = mk_calls(sp["Kb1"], CC1)
    calls2 = mk_calls(sp["Kb2"], CC2)
    MW = 128  # meta window (matmuls per meta tile)

    nc = bacc.Bacc("TRN2", target_bir_lowering=False, debug=False, num_devices=NC)
    x2_in = nc.dram_tensor("x2", [NW * WCAP + 2, 256], bf16, kind="ExternalInput")
    idx1_in = nc.dram_tensor("idx1", [128, sp["C1"] * 8], i16, kind="ExternalInput")
    idx2_in = nc.dram_tensor("idx2", [128, sp["C2"] * 8], i16, kind="ExternalInput")
    meta1_in = nc.dram_tensor("meta1", [128, 2 * sp["M1"]], f32, kind="ExternalInput")
    meta2_in = nc.dram_tensor("meta2", [128, 2 * sp["M2"]], f32, kind="ExternalInput")
    wb_in = nc.dram_tensor("wb", [128, WBW], f32, kind="ExternalInput")
    outT = nc.dram_tensor("outT", [COUT, NPAD], f32, kind="ExternalOutput")

    with tile.TileContext(nc) as tc:
        with (
            tc.tile_pool(name="const", bufs=1) as constp,
            tc.tile_pool(name="stream", bufs=3) as streamp,
            tc.tile_pool(name="msgs1", bufs=3) as msgs1p,
            tc.tile_pool(name="msgs2", bufs=2) as msgs2p,
            tc.tile_pool(name="oh", bufs=3) as ohp,
            tc.tile_pool(name="gskeep", bufs=TILES) as gskeepp,
            tc.tile_pool(name="work", bufs=8) as workp,
            tc.tile_pool(name="pacc", bufs=PSUM_ACC_BUFS, space="PSUM") as paccp,
            tc.tile_pool(name="pproj", bufs=2, space="PSUM") as pprojp,
            tc.tile_pool(name="dram", bufs=1, space="DRAM") as dramp,
        ):
            wb = constp.tile([128, WBW], f32)
            nc.sync.dma_start(wb[:], wb_in[:])
            W1 = wb[:, 0:128]
            W2 = wb[:, 128 : 128 + COUT]
            b1 = wb[:, 384:385]
            b2 = wb[:64, 385:386]
            iota_bf = constp.tile([128, 128], bf16)
            nc.vector.tensor_copy(iota_bf[:], wb[:, 256:384])

            # idx tables live in SBUF for the whole run (one load each)
            idxt1 = constp.tile([128, sp["C1"] * 8], i16)
            nc.sync.dma_start(idxt1[:], idx1_in[:])
            idxt2 = constp.tile([128, sp["C2"] * 8], i16)
            nc.sync.dma_start(idxt2[:], idx2_in[:])

            g_shard = dramp.tile([NSH + 2, COUT], bf16)
            # piece-major g rows + pad (odd-parity streams overread 64
            # elements past the last unit)
            g_full = dramp.tile([N + 4, COUT], bf16)
            gsu = g_shard[:].flatten()
            gfu = g_full[:].flatten()

            def gf_view(off, rows):
                return gfu[off : off + rows * 128].rearrange(
                    "(r c) -> r c", c=128
                )

            def gs_view(off, rows):
                return gsu[off : off + rows * 128].rearrange(
                    "(r c) -> r c", c=128
                )

            # layer-2 stream tables: (unit-range x parity) + self even/odd
            l2_tables = []
            for u0 in range(NB2u):
                for q in range(2):
                    rows = min(UBLK, NU - u0 * UBLK)
                    l2_tables.append(gf_view(u0 * UBLK * 128 + q * 64, rows))
            l1_tables = [
                x2_in[w * WCAP : w * WCAP + WCAP + 1, :] for w in range(NW)
            ]

            def aggregate(tables, mm, calls, meta_in, idxt, nrow, elem,
                          cc, msgsp, epilogue, self_tiles=None,
                          pool_every=0, after_tile=None):
                """Tile-major consumption: each tile's cells (all blocks)
                accumulate in one PSUM bank; block-major gather streams
                feed the matmuls; `epilogue(t, pt)` consumes the full tile."""
                NBt = len(tables)
                s_ci = [0] * NBt
                s_cur = [(-1, 0)] * NBt
                s_mg = [None] * NBt
                mt = None
                cur_w = -1
                pt = None
                cur_t = -1
                oht = None  # grouped one-hot tile: 8 slots per pool alloc
                M = mm.shape[0]
                for i in range(M):
                    b, t, c, lo, hi, half = (int(v) for v in mm[i])
                    is_self = b >= NBt
                    if not is_self:
                        cur0, curk = s_cur[b]
                    if not is_self and (s_mg[b] is None or c >= cur0 + curk):
                        cur0, curk = calls[b][s_ci[b]]
                        s_ci[b] += 1
                        s_cur[b] = (cur0, curk)
                        assert cur0 <= c < cur0 + curk
                        mg = msgsp.tile(
                            [128, cc, elem], bf16, tag=f"msgs{elem}_{b}"
                        )
                        s_mg[b] = mg
                        nc.gpsimd.dma_gather(
                            mg[:, :curk, :],
                            tables[b],
                            idxt[:, cur0 * 8 : (cur0 + curk) * 8],
                            num_idxs=curk * 128,
                            num_idxs_reg=curk * 128,
                            elem_size=elem,
                            single_packet=False,
                        )
                    if i // MW != cur_w:
                        cur_w = i // MW
                        nmw = min(MW, M - cur_w * MW)
                        mt = streamp.tile([128, 2 * MW], f32, tag="meta")
                        nc.sync.dma_start(
                            mt[:, : 2 * nmw],
                            meta_in[:, 2 * cur_w * MW : 2 * (cur_w * MW + nmw)],
                        )
                    if t != cur_t:
                        if pt is not None:
                            epilogue(cur_t, pt)
                            if after_tile is not None:
                                after_tile(cur_t)
                        pt = paccp.tile([128, 128], f32, tag="pacc")
                        cur_t = t
                        first = True
                    else:
                        first = False
                    last = (i == M - 1) or (int(mm[i + 1][1]) != t)
                    mloc = i - cur_w * MW
                    osl = i % 8
                    if osl == 0 or oht is None:
                        oht = ohp.tile([128, 8 * 128], bf16, tag="oh")
                    oh = oht[:, osl * 128 : (osl + 1) * 128]
                    eng = (
                        nc.gpsimd
                        if pool_every and (i % pool_every == pool_every - 1)
                        else nc.vector
                    )
                    eng.tensor_scalar(
                        oh,
                        iota_bf[:],
                        mt[:, 2 * mloc : 2 * mloc + 1],
                        mt[:, 2 * mloc + 1 : 2 * mloc + 2],
                        mybir.AluOpType.is_equal,
                        mybir.AluOpType.mult,
                    )
                    if is_self:
                        lhsT = self_tiles[t][:, :nrow]
                    else:
                        cl = c - cur0
                        co = half * 128
                        lhsT = s_mg[b][:, cl, co : co + nrow]
                    nc.tensor.matmul(
                        pt[:nrow, :],
                        lhsT,
                        oh,
                        start=first,
                        stop=last,
                    )
                epilogue(cur_t, pt)
                if after_tile is not None:
                    after_tile(cur_t)

            # ---------------- layer 1 + piece-wise exchange ----------------
            # The AllGather is split into 4 tile-group pieces, each issued
            # right after its last epilogue: pieces 0-2 transfer during the
            # rest of layer 1 (their local SDMA send/recv work is charged in
            # the timing model via the ring-work copy); only the last
            # piece's link time is exposed (test.py adds it).
            pieces = sp["pieces"]
            if for_timing:
                ring_scratch = dramp.tile(
                    [NC * max(pc[3] for pc in pieces), COUT], bf16
                )

            def emit_piece(p):
                t0, t1, r0w, prow, base = pieces[p]
                if for_timing:
                    nc.gpsimd.dma_start(
                        g_full[base : base + prow, :],
                        g_shard[r0w : r0w + prow, :],
                    )
                    # ring work: 7/8 of the piece is received (written)
                    # and 7/8 forwarded (read) per core; one copy of a
                    # 7/8 region charges exactly that on the DMA engines
                    r78 = (NC - 1) * prow
                    nc.gpsimd.dma_start(
                        ring_scratch[:r78, :],
                        g_full[base : base + r78, :],
                    )
                else:
                    nc.gpsimd.collective_compute(
                        "AllGather",
                        mybir.AluOpType.bypass,
                        replica_groups=[list(range(NC))],
                        ins=[g_shard[r0w : r0w + prow, :]],
                        outs=[g_full[base : base + NC * prow, :]],
                    )

            piece_end = {pc[1] - 1: p for p, pc in enumerate(pieces)}

            def after_tile1(t):
                if t in piece_end:
                    emit_piece(piece_end[t])

            gs_tiles = []  # SBUF-kept per-tile g rows for the L2 self diag

            def epilogue1(t, pt):
                r0 = t * 128
                rows = min(128, NSH - r0)
                aggT = workp.tile([128, 128], f32, tag="aggT")
                nc.scalar.activation(
                    aggT[:], pt[:], mybir.ActivationFunctionType.Copy
                )
                hp = pprojp.tile([128, 128], f32, tag="proj")
                nc.tensor.matmul(hp[:], W1, aggT[:], start=True, stop=True)
                hs = workp.tile([128, 128], f32, tag="hs")
                nc.scalar.activation(
                    hs[:], hp[:], mybir.ActivationFunctionType.Relu,
                    bias=b1, scale=1.0,
                )
                gp = pprojp.tile([128, 128], f32, tag="proj")
                nc.tensor.matmul(gp[:, :COUT], hs[:], W2, start=True, stop=True)
                gs = gskeepp.tile([128, COUT], bf16, tag="gs")
                nc.scalar.activation(
                    gs[:], gp[:, :COUT], mybir.ActivationFunctionType.Copy
                )
                gs_tiles.append(gs)
                nc.sync.dma_start(g_shard[r0 : r0 + rows, :], gs[:rows, :])

            aggregate(l1_tables, sp["mm1"], calls1, meta1_in, idxt1, 128,
                      256, CC1, msgs1p, epilogue1, pool_every=8,
                      after_tile=after_tile1)

            # ---------------- layer 2 ----------------
            def epilogue2(t, pt):
                r0 = t * 128
                cols = min(128, NSH - r0)
                ob = workp.tile([64, 128], f32, tag="ob")
                nc.scalar.activation(
                    ob[:],
                    pt[:COUT, :],
                    mybir.ActivationFunctionType.Identity,
                    bias=b2,
                    scale=1.0,
                )
                nc.sync.dma_start(outT[:, r0 : r0 + cols], ob[:, :cols])

            aggregate(l2_tables, sp["mm2"], calls2, meta2_in, idxt2, COUT,
                      128, CC2, msgs2p, epilogue2, self_tiles=gs_tiles)

    nc.compile()
    return nc


_CACHE = {}


def _get_program(sp):
    key = (sp["N"], sp["C1"], sp["C2"], sp["mm1"].tobytes(), sp["mm2"].tobytes())
    if key not in _CACHE:
        _CACHE[key] = _build_bass(sp)
    return _CACHE[key]


def _make_wb(W1, b1, W2, b2):
    wb = np.zeros((128, WBW), dtype=np.float32)
    wb[:, 0:128] = np.asarray(W1, dtype=np.float32)
    wb[:, 128 : 128 + COUT] = np.asarray(W2, dtype=np.float32)
    wb[:, 256:384] = np.arange(128, dtype=np.float32)[None, :]
    wb[:, 384] = np.asarray(b1, dtype=np.float32)
    wb[:64, 385] = np.asarray(b2, dtype=np.float32)
    return wb


def make_in_maps(sp, x, W1, b1, W2, b2):
    xb = np.ascontiguousarray(np.asarray(x, dtype=np.float32).astype(BF16))
    wb = _make_wb(W1, b1, W2, b2)
    NW, WCAP = sp["NW"], sp["WCAP"]
    maps = []
    for r in range(NC):
        x2 = np.zeros((NW * WCAP + 2, 256), dtype=BF16)
        for w, nodes in enumerate(sp["l1_rows"][r]):
            R = nodes.shape[0]
            base = w * WCAP
            x2[base : base + R, 0:128] = xb[nodes]
            if R > 1:
                x2[base : base + R - 1, 128:256] = xb[nodes[1:]]
        maps.append(
            {
                "x2": x2,
                "idx1": sp["idx1_sb"][r],
                "idx2": sp["idx2_sb"][r],
                "meta1": sp["meta1"][r],
                "meta2": sp["meta2"][r],
                "wb": wb,
            }
        )
    return maps


def kernel(x, edge_index, W1, b1, W2, b2, _trace=False):
    from concourse.bass_utils import run_bass_kernel_spmd

    x = np.asarray(x, dtype=np.float32)
    N = x.shape[0]
    sp = _schedule(np.asarray(edge_index), N)
    nc = _get_program(sp)
    in_maps = make_in_maps(sp, x, W1, b1, W2, b2)
    res = run_bass_kernel_spmd(nc, in_maps, list(range(NC)), trace=_trace)

    NSH = sp["NSH"]
    out = np.empty((N, COUT), dtype=np.float32)
    for r in range(NC):
        lo = r * NSH
        hi = min(N, lo + NSH)
        out[lo:hi] = res.results[r]["outT"][:, sp["pos_all"][r, : hi - lo]].T
    if _trace:
        kernel.last_result = res
    return out


# revision 82
# speedup vs baseline: 1.6987x; 1.0059x over previous
"""2-layer GCN on 8 Trainium2 NeuronCores (Bass/Tile).

Math: gcn_conv(x, W, b) = D^-1/2 (A+I) D^-1/2 (x W) + b.  Propagation
commutes with the weight matmul, so layer 1 aggregates raw x
(h = relu((A_hat x) W1 + b1)) and layer 2 projects first
(out = A_hat (h W2) + b2), minimizing per-edge gather bytes.

Distribution: destination nodes sharded 8 ways (12500/core); each core
gathers source features for its own edges from a full local feature table
(x is an input; g = h W2 is assembled with one 8-rank AllGather).

g is stored at its true width (64 cols bf16 = 128 B rows) so the
AllGather moves half the bytes of a 128-padded layout.  dma_gather
requires 256 B-aligned elements, so layer-2 gathers fetch a 256 B *unit*
(two consecutive g rows) per edge: streams come in (unit-range, parity)
flavors whose table APs are offset by 64 elements so the wanted row
always lands in msgs cols 0:64.

Per-core aggregation, per layer:
- block-major dma_gather streams (int16 idx limit => <=25000-row/unit
  tables). L1: 4 source-range blocks + a SELF stream reading x_own.
  L2: 4 (unit-range x parity) blocks reading g_full + 2 parity SELF
  streams reading g_shard.
- Edges are laid out in fractional per-(tile, block) cells balanced by a
  per-core node permutation (greedy vector bin packing), ~2% padding;
  boundary chunks are consumed by two matmuls with foreign lanes zeroed.
- Consumption is tile-major: bf16 one-hot scatter matrices
  (tensor_scalar is_equal*norm) feed PE matmuls msgs^T @ onehot
  (bf16 -> fp32 PSUM); each destination tile accumulates all its cells
  in one PSUM bank, then a single ACT-engine escape feeds the inline
  projection (W1 -> relu -> W2 -> bf16 g row) or the bias+output write.
"""
import sys

sys.path.insert(0, "/opt/trn_rl_repo")
import numpy as np
import ml_dtypes

BF16 = ml_dtypes.bfloat16
NC = 8
CIN, CH, COUT = 128, 128, 64
CALL_CHUNKS = 36  # gather-call granularity (chunks of 128 edges)
PSUM_ACC_BUFS = 6  # concurrent per-tile accumulation banks
WBW = 386  # wb cols: W1[0:128] W2[128:192] iota[256:384] b1[384] b2[385]


def _balance(nv, caps, k_tb):
    """Best-fit-decreasing node->tile assignment for one core.

    nv: [NSH, NB] per-node block in-degree vectors (both layers' blocks).
    caps: [TILES] node slots per tile.  k_tb: [TILES, NB] cell targets.
    Returns pos[NSH] (node -> global slot = tile*128 + slot_in_tile) or None
    if infeasible.
    """
    NSH, NB = nv.shape
    TILES = caps.shape[0]
    capv = k_tb.astype(np.float64)
    np.maximum(capv, 1e-9, out=capv)
    usedv = np.zeros((TILES, NB), dtype=np.float64)
    usect = np.zeros(TILES, dtype=np.int64)
    pos = np.empty(NSH, dtype=np.int64)
    order = np.argsort(-nv.sum(1), kind="stable")
    for n in order:
        v = nv[n].astype(np.float64)
        post = (usedv + v[None, :]) / capv
        feas = (post <= 1.0).all(1) & (usect < caps)
        if not feas.any():
            return None
        score = np.maximum(post.max(1), (usect + 1) / caps)
        score[~feas] = np.inf
        t = int(np.argmin(score))
        usedv[t] += v
        pos[n] = t * 128 + usect[t]
        usect[t] += 1
    return pos


def _mk_mm(captb, TILES):
    """Tile-major matmul table for a cell-capacity matrix [NB, TILES].

    Returns (mm [(b,t,c,lo,hi)], off_flat, Kb, C).  Chunk numbering is
    global across blocks (block b's chunks follow block b-1's).
    """
    NB = captb.shape[0]
    nkey = NB * TILES
    off_flat = np.zeros(nkey + 1, dtype=np.int64)
    np.cumsum(captb.reshape(-1), out=off_flat[1:])
    CAP = int(off_flat[-1])
    assert CAP % 128 == 0
    # block starts must be chunk-aligned so gather streams stay block-major
    blk_cap = captb.sum(axis=1)
    assert (blk_cap % 128 == 0).all(), blk_cap
    Kb = blk_cap // 128
    C = int(Kb.sum())
    mm = []
    for t in range(TILES):
        for b in range(NB):
            o = int(off_flat[b * TILES + t])
            cap = int(captb[b, t])
            if cap == 0:
                continue
            c0, c1 = o >> 7, (o + cap - 1) >> 7
            for c in range(c0, c1 + 1):
                lo = max(o, c * 128)
                hi = min(o + cap, (c + 1) * 128)
                mm.append((b, t, c, lo, hi))
    return np.array(mm, dtype=np.int64), off_flat, Kb, C


def _fill_slots(sel_by_core, key_all, val_all, dstl_all, norm_all, off_flat,
                capf, CAP):
    """Scatter per-edge records into their cell slots for all cores.

    Returns (idx [NC, CAP] int16, dstl [NC, CAP] f32, norm [NC, CAP] f32).
    """
    idx = np.zeros((NC, CAP), dtype=np.int16)
    dst = np.full((NC, CAP), -1.0, dtype=np.float32)
    nrm = np.zeros((NC, CAP), dtype=np.float32)
    for r in range(NC):
        sel = sel_by_core[r]
        k = key_all[sel]
        order = np.argsort(k, kind="stable")
        sel = sel[order]
        k = k[order]
        nkey = capf.shape[0]
        cr = np.bincount(k, minlength=nkey)
        grp_start = np.zeros(nkey, dtype=np.int64)
        np.cumsum(cr[:-1], out=grp_start[1:])
        rank = np.arange(sel.shape[0], dtype=np.int64) - np.repeat(grp_start, cr)
        assert (rank < capf[k]).all(), "cell overflow"
        slots = off_flat[k] + rank
        idx[r, slots] = val_all[sel].astype(np.int16)
        dst[r, slots] = dstl_all[sel]
        nrm[r, slots] = norm_all[sel]
    return idx, dst, nrm


def _wrap_idx(a):
    NCc, CAP = a.shape
    return np.ascontiguousarray(
        np.tile(a.reshape(NCc, CAP // 16, 16).transpose(0, 2, 1), (1, 8, 1))
    )


def _l1_pair_core(i_src, i_tile, i_dstl, i_norm, TILES, NW, WCAP):
    """Pair one core's L1 edge instances and lay out its x2 trail table.

    Any two same-tile instances may pair (one 512 B descriptor).  Pairs
    form a multigraph over src nodes; Eulerian trails lay it out so each
    pair occupies one table row [x[u] | x[v]].  Returns per-descriptor
    records plus per-window row node sequences.
    """
    order = np.lexsort((i_src, i_tile))
    ts = i_tile[order]
    cnt = np.bincount(ts, minlength=TILES)
    starts = np.zeros(TILES + 1, dtype=np.int64)
    np.cumsum(cnt, out=starts[1:])
    pA_l, pB_l, sing_l = [], [], []
    for t in range(TILES):
        s, e = int(starts[t]), int(starts[t + 1])
        k = (e - s) // 2
        seg = order[s:e]
        pA_l.append(seg[0 : 2 * k : 2])
        pB_l.append(seg[1 : 2 * k : 2])
        if (e - s) % 2:
            sing_l.append(int(seg[-1]))
    pA = np.concatenate(pA_l)
    pB = np.concatenate(pB_l)
    P = pA.shape[0]

    verts, inv = np.unique(
        np.concatenate([i_src[pA], i_src[pB]]), return_inverse=True
    )
    a = inv[:P].astype(np.int64)
    b = inv[P:].astype(np.int64)
    V = verts.shape[0]

    # union-find for components
    parent = np.arange(V, dtype=np.int64)

    def find(x):
        root = x
        while parent[root] != root:
            root = parent[root]
        while parent[x] != root:
            parent[x], x = root, parent[x]
        return root

    for i in range(P):
        ra, rb = find(a[i]), find(b[i])
        if ra != rb:
            parent[ra] = rb
    comp = np.fromiter((find(i) for i in range(V)), np.int64, V)

    # pair odd-degree vertices within components with virtual edges
    deg = np.bincount(a, minlength=V) + np.bincount(b, minlength=V)
    odd = np.nonzero(deg % 2 == 1)[0]
    oorder = odd[np.argsort(comp[odd], kind="stable")]
    va, vb = oorder[0::2], oorder[1::2]
    NE = P + va.shape[0]
    ea = np.concatenate([a, va])
    eb = np.concatenate([b, vb])

    # CSR half-edge adjacency + iterative Hierholzer
    he_v = np.concatenate([ea, eb])
    hstart = np.zeros(V + 1, dtype=np.int64)
    np.cumsum(np.bincount(he_v, minlength=V), out=hstart[1:])
    hlist = np.argsort(he_v, kind="stable")
    ptr = hstart[:-1].copy()
    used = np.zeros(NE, dtype=bool)
    trails = []  # (nodes [k+1], lo_inst [k], hi_inst [k])
    for v0 in range(V):
        while True:
            while ptr[v0] < hstart[v0 + 1] and used[hlist[ptr[v0]] % NE]:
                ptr[v0] += 1
            if ptr[v0] >= hstart[v0 + 1]:
                break
            stack_v = [v0]
            stack_e = []
            circ = []  # edge ids, circuit order
            while stack_v:
                x = stack_v[-1]
                while ptr[x] < hstart[x + 1] and used[hlist[ptr[x]] % NE]:
                    ptr[x] += 1
                if ptr[x] < hstart[x + 1]:
                    e = hlist[ptr[x]] % NE
                    used[e] = True
                    stack_v.append(ea[e] + eb[e] - x)
                    stack_e.append(e)
                else:
                    stack_v.pop()
                    if stack_e:
                        circ.append(stack_e.pop())
            circ.reverse()
            # walk the circuit, splitting at virtual edges
            v = v0
            nodes, lo_i, hi_i = [v], [], []
            for e in circ:
                if e >= P:  # virtual: close current trail
                    if lo_i:
                        trails.append((nodes, lo_i, hi_i))
                    v = ea[e] + eb[e] - v
                    nodes, lo_i, hi_i = [v], [], []
                else:
                    nv_ = ea[e] + eb[e] - v
                    lo = pA[e] if a[e] == v else pB[e]
                    hi = pA[e] + pB[e] - lo
                    if a[e] == b[e]:  # self-loop: either orientation
                        lo, hi = pA[e], pB[e]
                    nodes.append(nv_)
                    lo_i.append(lo)
                    hi_i.append(hi)
                    v = nv_
            if lo_i:
                trails.append((nodes, lo_i, hi_i))

    singles = list(sing_l)  # instance ids; ride an occurrence row
    rdeg = deg.copy()  # remaining real-edge degree
    total_rows = sum(len(n) for n, _, _ in trails)
    budget = NW * WCAP - 256  # headroom for orphan rows below
    # spill: pop trail-end pairs into singles until rows fit the windows
    guard = 0
    while total_rows > budget:
        progress = False
        for tr in trails:
            if total_rows <= budget:
                break
            nodes, lo_i, hi_i = tr
            if not lo_i:
                continue
            u_end = nodes[-1]
            u_prev = nodes[-2]
            d_end = rdeg[u_end] - (2 if u_end == u_prev else 1)
            if d_end < 1:
                continue
            if len(lo_i) == 1 and rdeg[u_prev] - 1 < 1:
                continue
            singles.append(lo_i.pop())
            singles.append(hi_i.pop())
            nodes.pop()
            rdeg[u_end] -= 1
            rdeg[u_prev] -= 1
            total_rows -= 1
            progress = True
        guard += 1
        assert progress and guard < 50, "L1 spill failed"
    trails = [t for t in trails if t[1]]

    # orphan singles: src with no remaining occurrence gets a 1-row trail
    occ_ok = np.zeros(V, dtype=bool)
    occ_ok[np.nonzero(rdeg > 0)[0]] = True
    vmap = {int(v): i for i, v in enumerate(verts)}
    fixed = []
    for s in singles:
        u = int(i_src[s])
        ui = vmap.get(u, -1)
        if ui >= 0 and occ_ok[ui]:
            fixed.append((s, ui))
        else:
            if ui < 0:
                vmap[u] = ui = V
                verts = np.append(verts, u)
                occ_ok = np.append(occ_ok, False)
                V += 1
            trails.append(([ui], [], []))  # 1-row trail, ridden by single
            occ_ok[ui] = True
            total_rows += 1
            fixed.append((s, ui))
    assert total_rows <= NW * WCAP, "L1 rows over budget after orphans"

    # window assignment: greedy vector packing on tile histograms
    tilecnt = np.bincount(i_tile, minlength=TILES).astype(np.float64)
    target = np.maximum(tilecnt / (2 * NW), 1.0)
    cellcnt = np.zeros((NW, TILES), dtype=np.int64)
    rows_used = np.zeros(NW, dtype=np.int64)
    tr_rows = np.array([len(n) for n, _, _ in trails])
    tr_order = np.argsort(-tr_rows, kind="stable")
    win_of = np.empty(len(trails), dtype=np.int64)
    for ti in tr_order:
        nodes, lo_i, hi_i = trails[ti]
        h = np.bincount(i_tile[lo_i], minlength=TILES) if lo_i else 0
        best_w, best_s = -1, None
        for w in range(NW):
            if rows_used[w] + len(nodes) > WCAP:
                continue
            s = ((cellcnt[w] + h) / target).max() if lo_i else (
                rows_used[w] / WCAP
            )
            if best_w < 0 or s < best_s:
                best_w, best_s = w, s
        assert best_w >= 0, "window overflow"
        win_of[ti] = best_w
        rows_used[best_w] += len(nodes)
        if lo_i:
            cellcnt[best_w] += h
    # build per-window row sequences + per-step descriptor records
    rows_w = [[] for _ in range(NW)]
    roff = np.zeros(NW, dtype=np.int64)
    d_key, d_idx = [], []
    d_dl = [[], []]
    d_nm = [[], []]
    first_occ = {}
    for ti, (nodes, lo_i, hi_i) in enumerate(trails):
        w = win_of[ti]
        base = roff[w]
        rows_w[w].append(np.asarray(nodes, dtype=np.int64))
        for k_ in range(len(nodes)):
            u = nodes[k_]
            if u not in first_occ:
                first_occ[u] = (w, base + k_)
        for k_, (lo, hi) in enumerate(zip(lo_i, hi_i)):
            d_key.append(w * TILES + int(i_tile[lo]))
            d_idx.append(base + k_)
            d_dl[0].append(i_dstl[lo])
            d_nm[0].append(i_norm[lo])
            d_dl[1].append(i_dstl[hi])
            d_nm[1].append(i_norm[hi])
        roff[w] += len(nodes)
    for s, ui in fixed:
        w, row = first_occ[ui]
        d_key.append(w * TILES + int(i_tile[s]))
        d_idx.append(row)
        d_dl[0].append(i_dstl[s])
        d_nm[0].append(i_norm[s])
        d_dl[1].append(-1.0)
        d_nm[1].append(0.0)
    key = np.asarray(d_key, dtype=np.int64)
    cell_final = np.bincount(key, minlength=NW * TILES).reshape(NW, TILES)
    return dict(
        cellcnt=cell_final,
        key=key,
        idxv=np.asarray(d_idx, dtype=np.int64),
        dstl=[np.asarray(d_dl[0], np.float32), np.asarray(d_dl[1], np.float32)],
        norm=[np.asarray(d_nm[0], np.float32), np.asarray(d_nm[1], np.float32)],
        rows_w=[
            verts[np.concatenate(rw)] if rw else np.zeros(0, dtype=np.int64)
            for rw in rows_w
        ],
    )


def _mk_meta_h(mmx, dstl_h, norm_h):
    """Meta for 6-column mm rows (b, t, c, lo, hi, half)."""
    M = mmx.shape[0]
    md = np.full((NC, M, 128), -1.0, dtype=np.float32)
    mn = np.zeros((NC, M, 128), dtype=np.float32)
    for i in range(M):
        b, t, c, lo, hi, h = mmx[i]
        base = int(c) * 128
        md[:, i, lo - base : hi - base] = dstl_h[h][:, lo:hi]
        mn[:, i, lo - base : hi - base] = norm_h[h][:, lo:hi]
    meta = np.empty((NC, 128, 2 * M), dtype=np.float32)
    meta[:, :, 0::2] = md.transpose(0, 2, 1)
    meta[:, :, 1::2] = mn.transpose(0, 2, 1)
    return meta


def _mk_meta(mm, dstl_all, norm_all):
    M = mm.shape[0]
    md = np.full((NC, M, 128), -1.0, dtype=np.float32)
    mn = np.zeros((NC, M, 128), dtype=np.float32)
    for i in range(M):
        b, t, c, lo, hi = mm[i]
        base = int(c) * 128
        md[:, i, lo - base : hi - base] = dstl_all[:, lo:hi]
        mn[:, i, lo - base : hi - base] = norm_all[:, lo:hi]
    meta = np.empty((NC, 128, 2 * M), dtype=np.float32)
    meta[:, :, 0::2] = md.transpose(0, 2, 1)
    meta[:, :, 1::2] = mn.transpose(0, 2, 1)
    return meta


def _schedule(edge_index, n_nodes):
    """Static SPMD schedule + per-core edge arrays for both layers."""
    N = n_nodes
    NSH = (N + NC - 1) // NC
    TILES = (NSH + 127) // 128
    BLK = 25000  # L1 x-table rows per block
    NB1 = -(-N // BLK)  # L1 source-range blocks
    UBLK = 25000  # L2 units per block (unit = 2 nodes)
    NU = (N + 1) // 2
    NB2u = -(-NU // UBLK)  # L2 unit-range blocks
    assert NSH % 2 == 0

    src = np.asarray(edge_index[0], dtype=np.int64)
    dst = np.asarray(edge_index[1], dtype=np.int64)
    E = src.shape[0]
    deg = np.bincount(dst, minlength=N).astype(np.float64) + 1.0
    dinv = 1.0 / np.sqrt(deg)
    w_all = (dinv[src] * dinv[dst]).astype(np.float32)

    core = dst // NSH
    j_all = dst - core * NSH
    blk1 = src // BLK

    # per-core per-node block in-degree vectors for balance:
    # L1 blocks (src range, self excluded) ++ L2 blocks (unit range x parity,
    # self excluded -- self parity depends on pos, handled by SELF streams)
    sc = src // NSH  # src core
    ub = sc // 4  # L2 unit-range block (aligned with shard pairs)
    nv1 = np.bincount(
        (core * NSH + j_all) * NB1 + blk1, minlength=NC * NSH * NB1
    ).reshape(NC, NSH, NB1)
    B1 = nv1.sum(axis=1)

    caps = np.full(TILES, 128, dtype=np.int64)
    caps[TILES - 1] = NSH - 128 * (TILES - 1)

    # exchange piece geometry (needed to label L2 blocks): tile groups
    PIECE_T = [0, 12, 24, 36, 48, 60, 72, 84, TILES]
    NP_ = len(PIECE_T) - 1
    pieces = []
    for p in range(NP_):
        t0, t1 = PIECE_T[p], PIECE_T[p + 1]
        r0 = int(caps[:t0].sum())
        r1 = int(caps[:t1].sum())
        assert (r1 - r0) % 2 == 0 and r0 % 2 == 0
        pieces.append([t0, t1, r0, r1 - r0, 0])
    prows = np.array([pc[3] for pc in pieces], dtype=np.int64)
    pstart = np.array([pc[2] for pc in pieces], dtype=np.int64)
    pbase = np.zeros(NP_, dtype=np.int64)
    pbase[1:] = np.cumsum(prows * NC)[:-1]
    for p in range(NP_):
        pieces[p][4] = int(pbase[p])
    piece_of_tile = np.zeros(TILES, dtype=np.int64)
    for p in range(NP_):
        piece_of_tile[PIECE_T[p] : PIECE_T[p + 1]] = p

    # The balance sees L1 source-range blocks; pass 2 adds L2 unit-range
    # columns labeled from pass-1 positions (the label of an edge depends
    # on where OTHER cores placed its src, so it needs a first pass).
    def spread(tot):
        tgt = tot * caps / NSH
        base = np.floor(tgt).astype(np.int64)
        rem = int(tot - base.sum())
        order = np.argsort(-(tgt - base))
        base[order[:rem]] += 1
        return base

    nv2u = None
    for pass_ in range(2):
        slack = 256
        for _attempt in range(8):
            TOT1 = ((B1.max(axis=0) + slack + 127) // 128) * 128  # [NB1]
            captb1 = np.stack([spread(t) for t in TOT1])  # [NB1, TILES]
            if nv2u is None:
                k_tb = captb1.T
                nv = nv1
            else:
                TOT2u = nv2u.sum(axis=1).max(axis=0) + 2 * slack
                captb2u = np.stack([spread(t) for t in TOT2u])
                k_tb = np.concatenate([captb1.T, captb2u.T], axis=1)
                nv = np.concatenate([nv1, nv2u], axis=2)
            pos_try = np.empty((NC, NSH), dtype=np.int64)
            ok = True
            for r in range(NC):
                pos = _balance(nv[r], caps, k_tb)
                if pos is None:
                    ok = False
                    break
                pos_try[r] = pos
            if ok:
                pos_all = pos_try
                break
            slack *= 2
        if not ok:
            assert pass_ == 1, "balance failed"
            break  # keep pass-1 positions
        if pass_ == 0:
            sp0 = pos_all[sc, src - sc * NSH]
            pp0 = piece_of_tile[sp0 >> 7]
            g0 = pbase[pp0] + sc * prows[pp0] + (sp0 - pstart[pp0])
            ub0 = (g0 >> 1) // UBLK
            nv2u = np.bincount(
                (core * NSH + j_all) * NB2u + ub0,
                minlength=NC * NSH * NB2u,
            ).reshape(NC, NSH, NB2u)

    # ---- explicit parity assignment (within-tile slot reshuffle) ----
    # A src node's position parity decides which L2 parity stream its
    # out-edges ride at every consumer.  Within each producer tile the
    # even/odd slot counts are fixed; which node takes which parity is
    # free.  Greedy discrepancy minimization over consumer (core, tile)
    # cells keeps every parity cell near half its unit-range cell.
    tile_of_e = pos_all[core, j_all] >> 7  # dst tile per edge (stable)
    cell_of_e = core * TILES + tile_of_e
    order_e = np.argsort(src, kind="stable")
    e_sorted = cell_of_e[order_e]
    src_sorted = src[order_e]
    starts = np.searchsorted(src_sorted, np.arange(N + 1))
    parity = np.zeros(N, dtype=np.int64)
    for g in range(NB2u):
        D = np.zeros(NC * TILES, dtype=np.int64)
        # nodes of producer cores 4g..4g+3, grouped by (core, tile)
        for r in range(4 * g, min(4 * g + 4, NC)):
            nval = min(NSH, N - r * NSH)
            p = pos_all[r, :nval]
            t = p >> 7
            for tt in range(TILES):
                nodes = np.nonzero(t == tt)[0] + r * NSH
                if nodes.size == 0:
                    continue
                degs = starts[nodes + 1] - starts[nodes]
                nodes = nodes[np.argsort(-degs, kind="stable")]
                n_even = (nodes.size + 1) // 2
                ev_left, od_left = n_even, nodes.size - n_even
                for n in nodes:
                    cells = e_sorted[starts[n] : starts[n + 1]]
                    if ev_left == 0:
                        p_n = 1
                    elif od_left == 0:
                        p_n = 0
                    else:
                        p_n = 0 if D[cells].sum() <= 0 else 1
                    parity[n] = p_n
                    if p_n == 0:
                        ev_left -= 1
                        np.add.at(D, cells, 1)
                    else:
                        od_left -= 1
                        np.add.at(D, cells, -1)
    # reassign within-tile slots by parity class
    for r in range(NC):
        nval = min(NSH, N - r * NSH)
        p = pos_all[r, :nval]
        t = p >> 7
        par_n = parity[r * NSH : r * NSH + nval]
        newpos = np.empty(nval, dtype=np.int64)
        for tt in range(TILES):
            nodes = np.nonzero(t == tt)[0]
            if nodes.size == 0:
                continue
            ev = nodes[par_n[nodes] == 0]
            od = nodes[par_n[nodes] == 1]
            newpos[ev] = tt * 128 + 2 * np.arange(ev.size)
            newpos[od] = tt * 128 + 2 * np.arange(od.size) + 1
        pos_all[r, :nval] = newpos

    pos_e = pos_all[core, j_all]
    tile_of = pos_e >> 7
    dstl = (pos_e & 127).astype(np.float32)

    # permuted g row of every src node -- PIECE-MAJOR g_full layout: the
    # AllGather is split into 4 tile-group pieces; piece p's region holds
    # all 8 ranks' rows for positions [pstart_p, pstart_p + prow_p), so
    # each piece collective writes one contiguous region.
    src_pos = pos_all[sc, src - sc * NSH]
    p_src = piece_of_tile[src_pos >> 7]
    g_row = pbase[p_src] + sc * prows[p_src] + (src_pos - pstart[p_src])
    par = (g_row & 1).astype(np.int64)
    ub = (g_row >> 1) // UBLK

    # ---------------- layer 1: paired descriptors ----------------
    # Each 512 B descriptor serves TWO same-tile edges: the per-core x2
    # table row i holds [x[a_i] | x[a_{i+1}]] along an Eulerian trail of
    # the chosen pair multigraph, so paired srcs sit in one row.  Blocks
    # are fixed 32760-row table windows (int16 idx).
    sel_by_core = [np.nonzero(core == r)[0] for r in range(NC)]
    WCAP = 32760
    NW = 4
    cores_l1 = []
    for r in range(NC):
        sel = sel_by_core[r]
        nval = min(NSH, N - r * NSH)
        jj = np.arange(nval, dtype=np.int64)
        p_self = pos_all[r, :nval]
        i_src = np.concatenate([src[sel], r * NSH + jj])
        i_tile = np.concatenate([tile_of[sel], p_self >> 7])
        i_dstl = np.concatenate(
            [dstl[sel], (p_self & 127).astype(np.float32)]
        )
        i_norm = np.concatenate(
            [w_all[sel], (dinv[r * NSH + jj] ** 2).astype(np.float32)]
        )
        cores_l1.append(
            _l1_pair_core(i_src, i_tile, i_dstl, i_norm, TILES, NW, WCAP)
        )
    # shared cell caps: max over cores, block sums chunk-aligned
    captb1 = np.zeros((NW, TILES), dtype=np.int64)
    for cd in cores_l1:
        np.maximum(captb1, cd["cellcnt"], out=captb1)
    for w in range(NW):
        rem = int(captb1[w].sum()) % 128
        if rem:
            captb1[w, TILES - 1] += 128 - rem
    mm1, off1, Kb1, C1 = _mk_mm(captb1, TILES)
    CAP1 = C1 * 128
    capf1 = captb1.reshape(-1)
    idx1 = np.zeros((NC, CAP1), dtype=np.int16)
    dstl1h = np.full((2, NC, CAP1), -1.0, dtype=np.float32)
    norm1h = np.zeros((2, NC, CAP1), dtype=np.float32)
    for r in range(NC):
        cd = cores_l1[r]
        k = cd["key"]
        order = np.argsort(k, kind="stable")
        k = k[order]
        nkey = capf1.shape[0]
        cr = np.bincount(k, minlength=nkey)
        grp_start = np.zeros(nkey, dtype=np.int64)
        np.cumsum(cr[:-1], out=grp_start[1:])
        rank = np.arange(k.shape[0], dtype=np.int64) - np.repeat(grp_start, cr)
        assert (rank < capf1[k]).all(), "L1 cell overflow"
        slots = off1[k] + rank
        idx1[r, slots] = cd["idxv"][order].astype(np.int16)
        for h in range(2):
            dstl1h[h, r, slots] = cd["dstl"][h][order]
            norm1h[h, r, slots] = cd["norm"][h][order]
    # expand mm rows: one matmul per (desc-chunk, half)
    mm1x = np.empty((mm1.shape[0] * 2, 6), dtype=np.int64)
    mm1x[0::2, :5] = mm1
    mm1x[1::2, :5] = mm1
    mm1x[0::2, 5] = 0
    mm1x[1::2, 5] = 1
    meta1 = _mk_meta_h(mm1x, dstl1h, norm1h)

    # ---------------- layer 2 cells ----------------
    # blocks 0..2*NB2u-1 = (unit range x parity) reading g_full; SELF is a
    # per-tile diag matmul against the SBUF-kept gs tile (block id NBm).
    # Parity cells sized exactly: max over cores + chunk-align block sums.
    NBm = 2 * NB2u
    cnt2 = np.bincount(
        (core * TILES + tile_of) * NBm + (ub * 2 + par),
        minlength=NC * TILES * NBm,
    ).reshape(NC, TILES, NBm)
    captb2m = cnt2.max(axis=0).T.copy()  # [NBm, TILES]
    for b in range(NBm):
        rem = int(captb2m[b].sum()) % 128
        if rem:
            captb2m[b, TILES - 1] += 128 - rem
    mm2, off2, Kb2, C2 = _mk_mm(captb2m, TILES)
    CAP2 = C2 * 128
    key2 = (ub * 2 + par) * TILES + tile_of
    val2 = (g_row >> 1) - ub * UBLK
    idx2, dstl2, norm2 = _fill_slots(
        sel_by_core, key2, val2, dstl, w_all, off2, captb2m.reshape(-1), CAP2
    )
    # self diag region: meta slots [CAP2 + t*128, +128), lane k = node at
    # pos t*128+k, value dinv^2 (dead lanes keep norm 0)
    dstl2s = np.tile(
        np.arange(128, dtype=np.float32)[None, :], (NC, TILES)
    ).reshape(NC, TILES * 128)
    norm2s = np.zeros((NC, TILES * 128), dtype=np.float32)
    for r in range(NC):
        nval = min(NSH, N - r * NSH)
        jj = np.arange(nval, dtype=np.int64)
        p = pos_all[r, :nval]
        norm2s[r, p] = (dinv[r * NSH + jj] ** 2).astype(np.float32)
    dstl2f = np.concatenate([dstl2, dstl2s], axis=1)
    norm2f = np.concatenate([norm2, norm2s], axis=1)
    # interleave: each tile's run starts with its self diag row
    mm2x_l = []
    cur_t = -1
    for row in mm2:
        t = int(row[1])
        if t != cur_t:
            mm2x_l.append(
                (NBm, t, C2 + t, CAP2 + t * 128, CAP2 + t * 128 + 128, 0)
            )
            cur_t = t
        mm2x_l.append((int(row[0]), t, int(row[2]), int(row[3]),
                       int(row[4]), 0))
    mm2x = np.array(mm2x_l, dtype=np.int64)
    meta2 = _mk_meta_h(
        mm2x,
        np.stack([dstl2f, dstl2f]),
        np.stack([norm2f, norm2f]),
    )

    l1_rows = [cores_l1[r]["rows_w"] for r in range(NC)]
    return dict(
        pieces=pieces,
        N=N, E=E, NSH=NSH, TILES=TILES, NB2u=NB2u, UBLK=UBLK,
        NW=NW, WCAP=WCAP, l1_rows=l1_rows,
        C1=C1, M1=mm1x.shape[0], mm1=mm1x, Kb1=Kb1,
        C2=C2, M2=mm2x.shape[0], mm2=mm2x, Kb2=Kb2,
        idx1_sb=_wrap_idx(idx1), idx2_sb=_wrap_idx(idx2),
        meta1=meta1, meta2=meta2,
        pos_all=pos_all,
    )


def _build_bass(sp, for_timing=False):
    import concourse.bass as bass
    import concourse.bacc as bacc
    import concourse.mybir as mybir
    import concourse.tile as tile

    f32 = mybir.dt.float32
    bf16 = mybir.dt.bfloat16
    i16 = mybir.dt.int16
    N, NSH, TILES = sp["N"], sp["NSH"], sp["TILES"]
    NW, WCAP = sp["NW"], sp["WCAP"]
    UBLK, NB2u = sp["UBLK"], sp["NB2u"]
    NU = (N + 1) // 2
    NPAD = TILES * 128

    def mk_calls(Kb, cc):
        calls = []
        chunk0 = 0
        for kb in (int(k) for k in Kb):
            lst = []
            s0, left = chunk0, kb
            while left > 0:
                cs = min(cc, left)
                lst.append((s0, cs))
                s0 += cs
                left -= cs
            calls.append(lst)
            chunk0 += kb
        return calls

    CC1, CC2 = 14, 14
    calls1 = mk_calls(sp["Kb1"], CC1)
    calls2 = mk_calls(sp["Kb2"], CC2)
    MW = 128  # meta window (matmuls per meta tile)

    nc = bacc.Bacc("TRN2", target_bir_lowering=False, debug=False, num_devices=NC)
    x2_in = nc.dram_tensor("x2", [NW * WCAP + 2, 256], bf16, kind="ExternalInput")
    idx1_in = nc.dram_tensor("idx1", [128, sp["C1"] * 8], i16, kind="ExternalInput")
    idx2_in = nc.dram_tensor("idx2", [128, sp["C2"] * 8], i16, kind="ExternalInput")
    meta1_in = nc.dram_tensor("meta1", [128, 2 * sp["M1"]], f32, kind="ExternalInput")
    meta2_in = nc.dram_tensor("meta2", [128, 2 * sp["M2"]], f32, kind="ExternalInput")
    wb_in = nc.dram_tensor("wb", [128, WBW], f32, kind="ExternalInput")
    outT = nc.dram_tensor("outT", [COUT, NPAD], f32, kind="ExternalOutput")

    with tile.TileContext(nc) as tc:
        with (
            tc.tile_pool(name="const", bufs=1) as constp,
            tc.tile_pool(name="stream", bufs=3) as streamp,
            tc.tile_pool(name="msgs1", bufs=3) as msgs1p,
            tc.tile_pool(name="msgs2", bufs=3) as msgs2p,
            tc.tile_pool(name="oh", bufs=3) as ohp,
            tc.tile_pool(name="gskeep", bufs=TILES) as gskeepp,
            tc.tile_pool(name="work", bufs=8) as workp,
            tc.tile_pool(name="pacc", bufs=PSUM_ACC_BUFS, space="PSUM") as paccp,
            tc.tile_pool(name="pproj", bufs=2, space="PSUM") as pprojp,
            tc.tile_pool(name="dram", bufs=1, space="DRAM") as dramp,
        ):
            wb = constp.tile([128, WBW], f32)
            nc.sync.dma_start(wb[:], wb_in[:])
            W1 = wb[:, 0:128]
            W2 = wb[:, 128 : 128 + COUT]
            b1 = wb[:, 384:385]
            b2 = wb[:64, 385:386]
            iota_bf = constp.tile([128, 128], bf16)
            nc.vector.tensor_copy(iota_bf[:], wb[:, 256:384])

            # idx tables live in SBUF for the whole run (one load each)
            idxt1 = constp.tile([128, sp["C1"] * 8], i16)
            nc.sync.dma_start(idxt1[:], idx1_in[:])
            idxt2 = constp.tile([128, sp["C2"] * 8], i16)
            nc.sync.dma_start(idxt2[:], idx2_in[:])

            g_shard = dramp.tile([NSH + 2, COUT], bf16)
            # piece-major g rows + pad (odd-parity streams overread 64
            # elements past the last unit)
            g_full = dramp.tile([N + 4, COUT], bf16)
            gsu = g_shard[:].flatten()
            gfu = g_full[:].flatten()

            def gf_view(off, rows):
                return gfu[off : off + rows * 128].rearrange(
                    "(r c) -> r c", c=128
                )

            def gs_view(off, rows):
                return gsu[off : off + rows * 128].rearrange(
                    "(r c) -> r c", c=128
                )

            # layer-2 stream tables: (unit-range x parity) + self even/odd
            l2_tables = []
            for u0 in range(NB2u):
                for q in range(2):
                    rows = min(UBLK, NU - u0 * UBLK)
                    l2_tables.append(gf_view(u0 * UBLK * 128 + q * 64, rows))
            l1_tables = [
                x2_in[w * WCAP : w * WCAP + WCAP + 1, :] for w in range(NW)
            ]

            def aggregate(tables, mm, calls, meta_in, idxt, nrow, elem,
                          cc, msgsp, epilogue, self_tiles=None,
                          pool_every=0, after_tile=None):
                """Tile-major consumption: each tile's cells (all blocks)
                accumulate in one PSUM bank; block-major gather streams
                feed the matmuls; `epilogue(t, pt)` consumes the full tile."""
                NBt = len(tables)
                s_ci = [0] * NBt
                s_cur = [(-1, 0)] * NBt
                s_mg = [None] * NBt
                mt = None
                cur_w = -1
                pt = None
                cur_t = -1
                oht = None  # grouped one-hot tile: 8 slots per pool alloc
                M = mm.shape[0]
                for i in range(M):
                    b, t, c, lo, hi, half = (int(v) for v in mm[i])
                    is_self = b >= NBt
                    if not is_self:
                        cur0, curk = s_cur[b]
                    if not is_self and (s_mg[b] is None or c >= cur0 + curk):
                        cur0, curk = calls[b][s_ci[b]]
                        s_ci[b] += 1
                        s_cur[b] = (cur0, curk)
                        assert cur0 <= c < cur0 + curk
                        mg = msgsp.tile(
                            [128, cc, elem], bf16, tag=f"msgs{elem}_{b}"
                        )
                        s_mg[b] = mg
                        nc.gpsimd.dma_gather(
                            mg[:, :curk, :],
                            tables[b],
                            idxt[:, cur0 * 8 : (cur0 + curk) * 8],
                            num_idxs=curk * 128,
                            num_idxs_reg=curk * 128,
                            elem_size=elem,
                            single_packet=False,
                        )
                    if i // MW != cur_w:
                        cur_w = i // MW
                        nmw = min(MW, M - cur_w * MW)
                        mt = streamp.tile([128, 2 * MW], f32, tag="meta")
                        nc.sync.dma_start(
                            mt[:, : 2 * nmw],
                            meta_in[:, 2 * cur_w * MW : 2 * (cur_w * MW + nmw)],
                        )
                    if t != cur_t:
                        if pt is not None:
                            epilogue(cur_t, pt)
                            if after_tile is not None:
                                after_tile(cur_t)
                        pt = paccp.tile([128, 128], f32, tag="pacc")
                        cur_t = t
                        first = True
                    else:
                        first = False
                    last = (i == M - 1) or (int(mm[i + 1][1]) != t)
                    mloc = i - cur_w * MW
                    osl = i % 8
                    if osl == 0 or oht is None:
                        oht = ohp.tile([128, 8 * 128], bf16, tag="oh")
                    oh = oht[:, osl * 128 : (osl + 1) * 128]
                    eng = (
                        nc.gpsimd
                        if pool_every and (i % pool_every == pool_every - 1)
                        else nc.vector
                    )
                    eng.tensor_scalar(
                        oh,
                        iota_bf[:],
                        mt[:, 2 * mloc : 2 * mloc + 1],
                        mt[:, 2 * mloc + 1 : 2 * mloc + 2],
                        mybir.AluOpType.is_equal,
                        mybir.AluOpType.mult,
                    )
                    if is_self:
                        lhsT = self_tiles[t][:, :nrow]
                    else:
                        cl = c - cur0
                        co = half * 128
                        lhsT = s_mg[b][:, cl, co : co + nrow]
                    nc.tensor.matmul(
                        pt[:nrow, :],
                        lhsT,
                        oh,
                        start=first,
                        stop=last,
                    )
                epilogue(cur_t, pt)
                if after_tile is not None:
                    after_tile(cur_t)

            # ---------------- layer 1 + piece-wise exchange ----------------
            # The AllGather is split into 4 tile-group pieces, each issued
            # right after its last epilogue: pieces 0-2 transfer during the
            # rest of layer 1 (their local SDMA send/recv work is charged in
            # the timing model via the ring-work copy); only the last
            # piece's link time is exposed (test.py adds it).
            pieces = sp["pieces"]
            if for_timing:
                ring_scratch = dramp.tile(
                    [NC * max(pc[3] for pc in pieces), COUT], bf16
                )

            def emit_piece(p):
                t0, t1, r0w, prow, base = pieces[p]
                if for_timing:
                    nc.gpsimd.dma_start(
                        g_full[base : base + prow, :],
                        g_shard[r0w : r0w + prow, :],
                    )
                    # ring work: 7/8 of the piece is received (written)
                    # and 7/8 forwarded (read) per core; one copy of a
                    # 7/8 region charges exactly that on the DMA engines
                    r78 = (NC - 1) * prow
                    nc.gpsimd.dma_start(
                        ring_scratch[:r78, :],
                        g_full[base : base + r78, :],
                    )
                else:
                    nc.gpsimd.collective_compute(
                        "AllGather",
                        mybir.AluOpType.bypass,
                        replica_groups=[list(range(NC))],
                        ins=[g_shard[r0w : r0w + prow, :]],
                        outs=[g_full[base : base + NC * prow, :]],
                    )

            piece_end = {pc[1] - 1: p for p, pc in enumerate(pieces)}

            def after_tile1(t):
                if t in piece_end:
                    emit_piece(piece_end[t])

            gs_tiles = []  # SBUF-kept per-tile g rows for the L2 self diag

            def epilogue1(t, pt):
                r0 = t * 128
                rows = min(128, NSH - r0)
                aggT = workp.tile([128, 128], f32, tag="aggT")
                nc.scalar.activation(
                    aggT[:], pt[:], mybir.ActivationFunctionType.Copy
                )
                hp = pprojp.tile([128, 128], f32, tag="proj")
                nc.tensor.matmul(hp[:], W1, aggT[:], start=True, stop=True)
                hs = workp.tile([128, 128], f32, tag="hs")
                nc.scalar.activation(
                    hs[:], hp[:], mybir.ActivationFunctionType.Relu,
                    bias=b1, scale=1.0,
                )
                gp = pprojp.tile([128, 128], f32, tag="proj")
                nc.tensor.matmul(gp[:, :COUT], hs[:], W2, start=True, stop=True)
                gs = gskeepp.tile([128, COUT], bf16, tag="gs")
                nc.scalar.activation(
                    gs[:], gp[:, :COUT], mybir.ActivationFunctionType.Copy
                )
                gs_tiles.append(gs)
                nc.sync.dma_start(g_shard[r0 : r0 + rows, :], gs[:rows, :])

            aggregate(l1_tables, sp["mm1"], calls1, meta1_in, idxt1, 128,
                      256, CC1, msgs1p, epilogue1, pool_every=8,
                      after_tile=after_tile1)

            # ---------------- layer 2 ----------------
            def epilogue2(t, pt):
                r0 = t * 128
                cols = min(128, NSH - r0)
                ob = workp.tile([64, 128], f32, tag="ob")
                nc.scalar.activation(
                    ob[:],
                    pt[:COUT, :],
                    mybir.ActivationFunctionType.Identity,
                    bias=b2,
                    scale=1.0,
                )
                nc.sync.dma_start(outT[:, r0 : r0 + cols], ob[:, :cols])

            aggregate(l2_tables, sp["mm2"], calls2, meta2_in, idxt2, COUT,
                      128, CC2, msgs2p, epilogue2, self_tiles=gs_tiles)

    nc.compile()
    return nc


_CACHE = {}


def _get_program(sp):
    key = (sp["N"], sp["C1"], sp["C2"], sp["mm1"].tobytes(), sp["mm2"].tobytes())
    if key not in _CACHE:
        _CACHE[key] = _build_bass(sp)
    return _CACHE[key]


def _make_wb(W1, b1, W2, b2):
    wb = np.zeros((128, WBW), dtype=np.float32)
    wb[:, 0:128] = np.asarray(W1, dtype=np.float32)
    wb[:, 128 : 128 + COUT] = np.asarray(W2, dtype=np.float32)
    wb[:, 256:384] = np.arange(128, dtype=np.float32)[None, :]
    wb[:, 384] = np.asarray(b1, dtype=np.float32)
    wb[:64, 385] = np.asarray(b2, dtype=np.float32)
    return wb


def make_in_maps(sp, x, W1, b1, W2, b2):
    xb = np.ascontiguousarray(np.asarray(x, dtype=np.float32).astype(BF16))
    wb = _make_wb(W1, b1, W2, b2)
    NW, WCAP = sp["NW"], sp["WCAP"]
    maps = []
    for r in range(NC):
        x2 = np.zeros((NW * WCAP + 2, 256), dtype=BF16)
        for w, nodes in enumerate(sp["l1_rows"][r]):
            R = nodes.shape[0]
            base = w * WCAP
            x2[base : base + R, 0:128] = xb[nodes]
            if R > 1:
                x2[base : base + R - 1, 128:256] = xb[nodes[1:]]
        maps.append(
            {
                "x2": x2,
                "idx1": sp["idx1_sb"][r],
                "idx2": sp["idx2_sb"][r],
                "meta1": sp["meta1"][r],
                "meta2": sp["meta2"][r],
                "wb": wb,
            }
        )
    return maps


def kernel(x, edge_index, W1, b1, W2, b2, _trace=False):
    from concourse.bass_utils import run_bass_kernel_spmd

    x = np.asarray(x, dtype=np.float32)
    N = x.shape[0]
    sp = _schedule(np.asarray(edge_index), N)
    nc = _get_program(sp)
    in_maps = make_in_maps(sp, x, W1, b1, W2, b2)
    res = run_bass_kernel_spmd(nc, in_maps, list(range(NC)), trace=_trace)

    NSH = sp["NSH"]
    out = np.empty((N, COUT), dtype=np.float32)
    for r in range(NC):
        lo = r * NSH
        hi = min(N, lo + NSH)
        out[lo:hi] = res.results[r]["outT"][:, sp["pos_all"][r, : hi - lo]].T
    if _trace:
        kernel.last_result = res
    return out
